# revision 1
# baseline (speedup 1.0000x reference)
"""GQA attention block (dense_transformer) on 8 trn2 cores.

Sharding: tensor-parallel by kv-group. Core c owns kv-group c = 8 query
heads + 1 k + 1 v head (640 rows of W_qkv) and the matching 512 columns of
W_dense. hidden_states is replicated (passed transposed, bf16). Each core
returns a bf16 partial [4096, 2048] dense output; the host sums the 8
partials in f32.

v2 layout: per-head attention. PSUM = 5 qkv accumulators + cpsA/cpsB/aux
(8 banks total). Score chunks rotate through the spare slots ordered by
dependency depth (the slot whose previous epilogue frees last is used last);
in phase C batch-1 scores also rotate through qkv4 (dense needs qkv0..3
only), a 4-deep score pipeline. PV accumulates per 512-col half into new
cpsA/cpsB instances whose groups stop exactly at ki=3 / ki=7, where each
half's softmax epilogue (reciprocal straight off the PSUM ones-row, f32r
ones-matmul broadcast in aux) runs and frees the bank. The causal mask is a
0/1-triangle bf16 multiply on et AFTER the exp — off the score->exp chain
and out of PSUM. Dense borrows the qkv slots; the v-transpose borrows aux.
Emission is interleaved across batches — qkv(b1) fills attention(b0)'s PE
gaps and dense(b0) fills attention(b1)'s — since each engine executes its
queue in program order. Bulk DMAs trigger from the Pool engine (SWDGE,
alternating with SP) to keep them off the shared HWDGE dispatcher; weight
tiles load lazily at the hs prefetch cadence so the first matmul unblocks
early. Simulated per-core time (TimelineSim cost model): 327.7 us vs 605 us
for the session-start baseline; PE engine busy time (314.4 us) sits exactly
at the output-column floor of the 128x128 tiling, occupancy 95.9%.
"""
import numpy as np
import ml_dtypes
from contextlib import ExitStack

import bass_rust
import concourse.bass as bass
import concourse.mybir as mybir
from concourse import tile
from concourse.bass_utils import run_bass_kernel_spmd

dt = mybir.dt
bf16 = ml_dtypes.bfloat16

B, S, HID = 2, 1024, 4096
NKV, G, HD = 8, 8, 64
NPOS = B * S
INV = 0.125
NCORES = 8

# ---------------------------------------------------------------------------
# walrus in this container takes at most ONE sync-wait per instruction; Tile
# attaches several (tail drain especially). Split extras onto same-engine nops.
_orig_exit = tile.TileContext.__exit__


def _split_waits(nc):
    for bb in nc.m.functions[0].blocks:
        out, extra = [], 0
        for inst in bb.instructions:
            si = inst.sync_info
            if si is not None and len(si.on_wait) > 1:
                waits = list(si.on_wait)
                for w in waits[:-1]:
                    nop = mybir.InstNoOp(name=f"I-wsplit-{nc.next_id()}")
                    nop.engine = inst.engine
                    nop.sync_info = bass_rust.SyncInfo(on_wait=[w], on_update=[])
                    nc.register_instruction(nop, overwrite=True)
                    out.append(nop)
                    extra += 1
                inst.sync_info = bass_rust.SyncInfo(
                    on_wait=[waits[-1]], on_update=list(si.on_update)
                )
            out.append(inst)
        if extra:
            bb.instructions = out


def _patched_exit(self, exc_type, exc_val, exc_tb):
    r = _orig_exit(self, exc_type, exc_val, exc_tb)
    _split_waits(self.nc)
    return r


tile.TileContext.__exit__ = _patched_exit
# ---------------------------------------------------------------------------

_CACHED_NC = None


def _drive(*gens):
    live = list(gens)
    while live:
        for g in list(live):
            try:
                next(g)
            except StopIteration:
                live.remove(g)


def build_program():
    global _CACHED_NC
    if _CACHED_NC is not None:
        return _CACHED_NC
    nc = bass.Bass()
    hst_d = nc.declare_dram_parameter("hst", [32, 128, NPOS], dt.bfloat16, isOutput=False)
    wq_d = nc.declare_dram_parameter("wq", [32, 128, 640], dt.bfloat16, isOutput=False)
    wd_d = nc.declare_dram_parameter("wd", [4, 128, 4096], dt.bfloat16, isOutput=False)
    cst_d = nc.declare_dram_parameter("cst", [128, 2048], dt.bfloat16, isOutput=False)
    msk_d = nc.declare_dram_parameter("msk", [128, 128], dt.bfloat16, isOutput=False)
    ab_d = nc.declare_dram_parameter("ab", [128, 128], dt.float32, isOutput=False)
    idn_d = nc.declare_dram_parameter("idn", [64, 64], dt.bfloat16, isOutput=False)
    outp_d = nc.declare_dram_parameter("outp", [32, 128, NPOS], dt.bfloat16, isOutput=True)

    AF = mybir.ActivationFunctionType
    # packed causal offsets for et: block ki has width 1024-128*ki
    koff = [0] * 8
    for ki in range(1, 8):
        koff[ki] = koff[ki - 1] + (1024 - 128 * (ki - 1))
    ET_W = koff[7] + (1024 - 128 * 7)  # 4608

    with ExitStack() as ctx:
        tc = ctx.enter_context(tile.TileContext(nc))
        cpool = ctx.enter_context(tc.tile_pool(name="const", bufs=1))
        wq_sb = [cpool.tile([128, 640], dt.bfloat16, tag=f"wq{k}", name=f"wq{k}")
                 for k in range(32)]
        nc.sync.dma_start(wq_sb[0][:], wq_d[0])  # first matmul unblocks asap
        cst_sb = cpool.tile([128, 2048], dt.bfloat16)
        msk_sb = cpool.tile([128, 128], dt.bfloat16)
        ab_sb = cpool.tile([128, 128], dt.float32)
        idn_sb = cpool.tile([64, 64], dt.bfloat16)
        wd_sb = [cpool.tile([128, 4096], dt.bfloat16, tag=f"wd{kt}", name=f"wd{kt}")
                 for kt in range(4)]
        onesf = cpool.tile([1, 64], dt.float32)
        nc.vector.memset(onesf[:], 1.0)
        ones_r = cpool.tile([1, 64], dt.float32r)
        nc.scalar.copy(ones_r[:], onesf[:])

        hs_pool = ctx.enter_context(tc.tile_pool(name="hs", bufs=2))
        raw_pool = ctx.enter_context(tc.tile_pool(name="raw", bufs=2))
        tmp_pool = ctx.enter_context(tc.tile_pool(name="tmp", bufs=1))
        qp_pool = ctx.enter_context(tc.tile_pool(name="qp", bufs=2))
        kv_pool = ctx.enter_context(tc.tile_pool(name="kv", bufs=2))
        va_pool = ctx.enter_context(tc.tile_pool(name="va", bufs=2))
        et_pool = ctx.enter_context(tc.tile_pool(name="et", bufs=2))
        l_pool = ctx.enter_context(tc.tile_pool(name="l", bufs=1))
        rb_pool = ctx.enter_context(tc.tile_pool(name="rb", bufs=1))
        ctx_pool = ctx.enter_context(tc.tile_pool(name="ctx", bufs=2))
        dout_pool = ctx.enter_context(tc.tile_pool(name="dout", bufs=6))
        mm = ctx.enter_context(tc.tile_pool(name="mm", bufs=1, space="PSUM"))

        # per-batch SBUF state, filled by gen_qkv, read by gen_attn/gen_dense
        qp = {}   # (b, h) -> [64, 1024] bf16
        kk = {}   # b -> [64, 1024] bf16
        va = {}   # b -> [128, 8*72] bf16
        ctxt = {}  # (b, pr) -> [128, 1024] bf16

        def gen_qkv(b):
            for h in range(8):
                qp[(b, h)] = qp_pool.tile([64, 1024], dt.bfloat16, tag=f"qp{h}",
                                          name=f"qp{h}_{b}")
            kk[b] = kv_pool.tile([64, 1024], dt.bfloat16, tag="kk", name=f"kk{b}")
            vt = kv_pool.tile([64, 1024], dt.bfloat16, tag="vt", name=f"vt{b}")
            for n in range(2):
                pcol = b * 1024 + n * 512
                ncol = slice(n * 512, n * 512 + 512)
                ps = [mm.tile([128, 512], dt.float32, tag=f"qkv{m}", name=f"qkv{m}")
                      for m in range(5)]
                hs_t = {}

                def _load(k):
                    hs_t[k] = hs_pool.tile([128, 512], dt.bfloat16, tag=f"hs{k % 8}",
                                           name=f"hs{k}_{n}_{b}")
                    eng = nc.gpsimd if k % 2 == 0 else nc.sync
                    eng.dma_start(hs_t[k][:], hst_d[k][:, pcol:pcol + 512])
                    # lazy wq loads ride the same prefetch cadence on SP
                    if b == 0 and n == 0 and k > 0:
                        nc.sync.dma_start(wq_sb[k][:], wq_d[k])
                    if b == 0 and n == 0 and k == 12:
                        # small consts, needed from the RoPE/attention stages
                        nc.gpsimd.dma_start(cst_sb[:], cst_d[:])
                        nc.gpsimd.dma_start(msk_sb[:], msk_d[:])
                        nc.gpsimd.dma_start(ab_sb[:], ab_d[:])
                        nc.gpsimd.dma_start(idn_sb[:], idn_d[:])

                for k in range(8):
                    _load(k)
                raw = [raw_pool.tile([128, 512], dt.bfloat16, tag=f"raw{m}",
                                     name=f"raw{m}_{n}_{b}") for m in range(5)]
                for k in range(32):
                    if k + 8 < 32:
                        _load(k + 8)
                    for m in range(5):
                        nc.tensor.matmul(
                            ps[m][:],
                            wq_sb[k][:, m * 128:(m + 1) * 128],
                            hs_t[k][:],
                            start=(k == 0), stop=(k == 31),
                        )
                        if k == 31:
                            # drain each accumulator while PE finishes the rest
                            nc.scalar.copy(raw[m][:], ps[m][:])
                    if k % 2 == 1:
                        yield
                Cs = cst_sb[:, n * 512:(n + 1) * 512]
                Ss = cst_sb[:, 1024 + n * 512: 1024 + (n + 1) * 512]
                for grp in range(2):
                    A, Bb = raw[grp * 2], raw[grp * 2 + 1]
                    P1 = tmp_pool.tile([128, 512], dt.bfloat16, tag="P1")
                    P2 = tmp_pool.tile([128, 512], dt.bfloat16, tag="P2")
                    P3 = tmp_pool.tile([128, 512], dt.bfloat16, tag="P3")
                    P4 = tmp_pool.tile([128, 512], dt.bfloat16, tag="P4")
                    nc.vector.tensor_mul(P1[:], A[:], Cs)
                    nc.vector.tensor_mul(P2[:], Bb[:], Ss)
                    nc.vector.tensor_mul(P3[:], Bb[:], Cs)
                    nc.vector.tensor_mul(P4[:], A[:], Ss)
                    for i in range(4):
                        h = grp * 4 + i
                        sl = slice(32 * i, 32 * i + 32)
                        nc.vector.tensor_sub(qp[(b, h)][0:32, ncol], P1[sl, :], P2[sl, :])
                        nc.vector.tensor_add(qp[(b, h)][32:64, ncol], P3[sl, :], P4[sl, :])
                kvr = raw[4]
                pk1 = tmp_pool.tile([32, 512], dt.bfloat16, tag="pk1")
                pk2 = tmp_pool.tile([32, 512], dt.bfloat16, tag="pk2")
                pk3 = tmp_pool.tile([32, 512], dt.bfloat16, tag="pk3")
                pk4 = tmp_pool.tile([32, 512], dt.bfloat16, tag="pk4")
                nc.vector.tensor_mul(pk1[:], kvr[0:32, :], Cs[0:32, :])
                nc.vector.tensor_mul(pk2[:], kvr[32:64, :], Ss[32:64, :])
                nc.vector.tensor_mul(pk3[:], kvr[32:64, :], Cs[32:64, :])
                nc.vector.tensor_mul(pk4[:], kvr[0:32, :], Ss[0:32, :])
                nc.vector.tensor_sub(kk[b][0:32, ncol], pk1[:], pk2[:])
                nc.vector.tensor_add(kk[b][32:64, ncol], pk3[:], pk4[:])
                nc.vector.tensor_copy(vt[:, ncol], kvr[64:128, :])
                yield
            # V transpose + ones column (borrows the aux PSUM bank)
            va[b] = va_pool.tile([128, 8 * 72], dt.bfloat16, tag="va", name=f"va{b}")
            for ki in range(8):
                slot = mm.tile([128, 512], dt.float32, tag="aux", name=f"vps{ki}_{b}")
                vps = slot[:, 0:32].bitcast(dt.bfloat16)
                nc.tensor.transpose(vps, vt[0:64, ki * 128:(ki + 1) * 128],
                                    idn_sb[:, :])
                nc.vector.tensor_copy(va[b][:, ki * 72: ki * 72 + 64], vps)
                nc.vector.memset(va[b][:, ki * 72 + 64: ki * 72 + 65], 1.0)
            yield

        def gen_attn(b):
            if b == 0:
                for kt in range(4):  # wd needed from phase C; load during B
                    nc.sync.dma_start(wd_sb[kt][:], wd_d[kt])
            for pr in range(4):
                ctxt[(b, pr)] = ctx_pool.tile([128, 1024], dt.bfloat16,
                                              tag=f"ctxt{pr}", name=f"ctxt{pr}_{b}")
            # b=0 (phase B): qkv(1) owns all 5 qkv slots, so PV shares the
            # cpsA/cpsB score slots and runs after all 12 exps of the head.
            # b=1 (phase C): dense only uses qkv0/1, so PV accumulates in
            # qkv2/qkv3 and interleaves per-ki into the score stream.
            cps_tags = ("cpsA", "cpsB")
            pv_inline = False
            # phase C: dense only touches qkv0..3, so qkv4 is a free 4th
            # score slot; putting it first unblocks the b=1 head-0 chunk
            # before b=0's last epilogue releases cpsA/cpsB/aux
            rot = ("cpsA", "aux", "cpsB") if b == 0 else ("cpsA", "qkv4", "aux", "cpsB")
            for h in range(8):
                et = et_pool.tile([128, ET_W], dt.bfloat16, tag="et", name=f"et{h}_{b}")
                pr, hh = h // 2, h % 2
                rr = l_pool.tile([1, 1024], dt.float32r, tag="rr")
                rb = rb_pool.tile([64, 1024], dt.float32, tag="rb")
                cph = [mm.tile([128, 512], dt.float32, tag=cps_tags[0], name=f"cpA{h}_{b}"),
                       mm.tile([128, 512], dt.float32, tag=cps_tags[1], name=f"cpB{h}_{b}")]
                ci = 0

                def _pv(ki):
                    g0 = ki * 128
                    while g0 < 1024:
                        half = g0 // 512
                        g1 = min(1024, (half + 1) * 512)
                        loc = slice(g0 - half * 512, g1 - half * 512)
                        nc.tensor.matmul(
                            cph[half][0:65, loc],
                            va[b][:, ki * 72: ki * 72 + 65],
                            et[:, koff[ki] + g0 - ki * 128: koff[ki] + g1 - ki * 128],
                            start=(ki == 0), stop=(ki == (3 if half == 0 else 7)),
                            skip_group_check=True,
                        )
                        g0 = g1

                def _epi(ki):
                    # epilogue for the finished half: A after ki=3, B after 7
                    half = 0 if ki == 3 else 1
                    hs_ = slice(half * 512, half * 512 + 512)
                    # reciprocal straight from the PSUM ones-row
                    # (f32r is fp32-width; the gate only knows dtype != f32)
                    with nc.allow_low_precision(reason="f32r output is fp32-width"):
                        nc.vector.reciprocal(rr[0:1, hs_], cph[half][64:65, 0:512])
                    slot = mm.tile([128, 512], dt.float32, tag="aux",
                                   name=f"rps{h}{half}_{b}")
                    nc.tensor.matmul(slot[0:64, :], ones_r[:], rr[:, hs_],
                                     start=True, stop=True)
                    nc.vector.tensor_copy(rb[:, hs_], slot[0:64, :])
                    nc.vector.tensor_mul(
                        ctxt[(b, pr)][hh * 64:(hh + 1) * 64, hs_],
                        cph[half][0:64, 0:512], rb[:, hs_])

                for ki in range(8):
                    base = ki * 128
                    nchunks = (1024 - base + 511) // 512
                    for cj in range(nchunks):
                        c0 = base + cj * 512
                        cw = min(512, 1024 - c0)
                        sc = mm.tile([128, 512], dt.float32,
                                     tag=rot[ci % len(rot)],
                                     name=f"sc{h}{ki}{cj}_{b}")
                        ci += 1
                        nc.tensor.matmul(
                            sc[:, 0:cw],
                            kk[b][0:64, base:base + 128],
                            qp[(b, h)][0:64, c0:c0 + cw],
                            start=True, stop=True,
                        )
                        abc = b * 64 + ki * 8 + h
                        nc.scalar.activation(
                            et[:, koff[ki] + (c0 - base): koff[ki] + (c0 - base) + cw],
                            sc[:, 0:cw], AF.Exp,
                            bias=ab_sb[:, abc:abc + 1], scale=INV)
                        if cj == 0:
                            # causal mask: zero the upper triangle of the diag
                            # block via a 0/1 multiply (off the sc->exp chain)
                            nc.vector.tensor_mul(
                                et[:, koff[ki]: koff[ki] + 128],
                                et[:, koff[ki]: koff[ki] + 128], msk_sb[:])
                    if pv_inline:
                        _pv(ki)
                        if ki == 3 or ki == 7:
                            _epi(ki)
                            yield
                    if ki == 3:
                        yield
                if not pv_inline:
                    yield
                    for ki in range(8):
                        _pv(ki)
                        if ki == 3 or ki == 7:
                            _epi(ki)
                            yield

        def gen_dense(b):
            # b=1 runs after attention is done, so the score-rotation banks
            # are free for deeper accumulate/drain pipelining
            slots = ("qkv0", "qkv1", "qkv2", "qkv3") if b == 0 else (
                "qkv0", "qkv1", "qkv2", "qkv3", "aux", "cpsA")
            for mt in range(32):
                dsb = dout_pool.tile([128, 1024], dt.bfloat16, tag="dsb",
                                     name=f"dsb{mt}_{b}")
                for n2 in range(2):
                    dps = mm.tile([128, 512], dt.float32,
                                  tag=slots[(mt * 2 + n2) % len(slots)],
                                  name=f"d{mt}{n2}_{b}")
                    for kt in range(4):
                        nc.tensor.matmul(
                            dps[:],
                            wd_sb[kt][:, mt * 128:(mt + 1) * 128],
                            ctxt[(b, kt)][:, n2 * 512:(n2 + 1) * 512],
                            start=(kt == 0), stop=(kt == 3),
                        )
                    if n2 == 0:
                        nc.scalar.copy(dsb[:, 0:512], dps[:])
                    else:
                        nc.vector.tensor_copy(dsb[:, 512:1024], dps[:])
                    if b == 1 and mt == 31:
                        # last tile: ship halves separately to shorten the
                        # final copy->DMA drain chain
                        nc.sync.dma_start(
                            outp_d[mt][:, b * 1024 + n2 * 512:
                                        b * 1024 + n2 * 512 + 512],
                            dsb[:, n2 * 512:(n2 + 1) * 512])
                if not (b == 1 and mt == 31):
                    eng = nc.gpsimd
                    eng.dma_start(
                        outp_d[mt][:, b * 1024: b * 1024 + 1024], dsb[:])
                yield

        _drive(gen_qkv(0))
        _drive(gen_qkv(1), gen_attn(0))
        _drive(gen_dense(0), gen_attn(1))
        _drive(gen_dense(1))

    _CACHED_NC = nc
    return nc


def host_prep(hidden_states, alibi, attention_mask, W_qkv, W_dense):
    hsT = np.ascontiguousarray(hidden_states.reshape(NPOS, HID).T).astype(bf16)
    hsT = hsT.reshape(32, 128, NPOS)

    j32 = np.arange(32)
    inv_freq = 1.0 / (10000.0 ** (2 * j32 / HD))
    t = np.arange(S, dtype=np.float64)
    fr = np.outer(inv_freq, t)                       # [32, S]
    cst = np.zeros((128, 2048), np.float32)
    cst[:, 0:1024] = np.tile(np.cos(fr), (4, 1))
    cst[:, 1024:2048] = np.tile(np.sin(fr), (4, 1))
    cst = cst.astype(bf16)

    # single causal diag block, [kpos, q] layout: 0 where kpos > q, else 1
    mf = np.where(attention_mask[0, 0, 0:128, 0:128], 0.0, 1.0).astype(np.float32)
    msk = np.ascontiguousarray(mf.T).astype(bf16)    # [kpos, q]

    al = alibi.reshape(B, NKV * G, S) * INV          # [B, 64, S]

    perm = []
    for i in range(4):
        perm += [i * 64 + d for d in range(32)]
    for i in range(4):
        perm += [i * 64 + 32 + d for d in range(32)]
    for i in range(4, 8):
        perm += [i * 64 + d for d in range(32)]
    for i in range(4, 8):
        perm += [i * 64 + 32 + d for d in range(32)]
    perm += [512 + d for d in range(64)] + [576 + d for d in range(64)]
    perm = np.array(perm)

    idn = np.eye(64, dtype=np.float32).astype(bf16)
    in_maps = []
    for c in range(NCORES):
        Wg = W_qkv[c * 640:(c + 1) * 640][perm]       # [640, 4096]
        wq = np.ascontiguousarray(Wg.T).astype(bf16).reshape(32, 128, 640)
        Wd = W_dense[:, c * 512:(c + 1) * 512]        # [4096, 512]
        wd = np.ascontiguousarray(Wd.T).astype(bf16).reshape(4, 128, 4096)
        ab = np.zeros((128, 128), np.float32)
        for b in range(2):
            for ki in range(8):
                for h in range(8):
                    ab[:, b * 64 + ki * 8 + h] = al[b, c * 8 + h,
                                                    ki * 128:(ki + 1) * 128]
        in_maps.append({
            "hst": hsT, "wq": wq, "wd": wd, "cst": cst,
            "msk": msk, "ab": ab, "idn": idn,
        })
    return in_maps


def kernel(hidden_states, alibi, attention_mask, W_qkv, W_dense, _want_time=False):
    nc = build_program()
    in_maps = host_prep(np.asarray(hidden_states), np.asarray(alibi),
                        np.asarray(attention_mask), np.asarray(W_qkv),
                        np.asarray(W_dense))
    res = run_bass_kernel_spmd(nc, in_maps, list(range(NCORES)))
    acc = np.zeros((32, 128, NPOS), np.float32)
    for c in range(NCORES):
        acc += res.results[c]["outp"].astype(np.float32)
    out = acc.reshape(4096, NPOS).T.reshape(B, S, HID)
    if _want_time:
        return np.ascontiguousarray(out), res
    return np.ascontiguousarray(out)



# revision 11
# speedup vs baseline: 1.0382x; 1.0382x over previous
"""GQA attention block (dense_transformer) on 8 trn2 cores.

Sharding: tensor-parallel by kv-group. Core c owns kv-group c = 8 query
heads + 1 k + 1 v head (640 rows of W_qkv) and the matching 512 columns of
W_dense. hidden_states is replicated. Each core returns a bf16 partial
[4096, 2048] dense output; the host sums the 8 partials in f32.

v3: split-precision fp8 DoubleRow for the two big GEMMs. QKV and dense
weights/activations are decomposed host-side (and ctx on-chip) into
hi+lo fp8e4 planes; each K=256 pair runs as 3 DoubleRow matmuls
(hi*hi, lo*hi, hi*lo) at 0.5 cycles/col = 0.75x the bf16 PE cost with
bf16-level accuracy (dropped lo*lo term ~2^-8). Attention internals
(RoPE, scores, softmax, PV, epilogue) stay bf16 exactly as v2: per-head
scores with cpsA/cpsB/aux PSUM rotation, packed causal et layout, 0/1
triangle mask multiply on DVE, reciprocal-of-ones-row epilogue with
f32r ones-matmul broadcast. Scales: W_qkv and W_dense x64 host-side
(q,k,v 64x), exp activation scale INV/4096, va ones column 4.0 (ctx
16x true), final dense drain x2^-10. The wq pool (40KB hi+lo) is
released after phase B and the dense weights reuse its zone.
"""
import numpy as np
import ml_dtypes
from contextlib import ExitStack

import bass_rust
import concourse.bass as bass
import concourse.mybir as mybir
from concourse import tile
from concourse.bass_utils import run_bass_kernel_spmd

dt = mybir.dt
bf16 = ml_dtypes.bfloat16
f8 = ml_dtypes.float8_e4m3

B, S, HID = 2, 1024, 4096
NKV, G, HD = 8, 8, 64
NPOS = B * S
INV = 0.125
WS = 64.0                      # host-side weight scale (q,k,v come out 64x)
EXP_SCALE = INV / (WS * WS)    # PSUM scores are 4096x true
ONES_VAL = 4.0                 # va ones column -> ctx = 16x true
DRAIN_SCALE = 1.0 / 1024.0     # dense psum = 16 * 64 = 1024x true
NCORES = 8
DR = mybir.MatmulPerfMode.DoubleRow

# ---------------------------------------------------------------------------
# walrus in this container takes at most ONE sync-wait per instruction; Tile
# attaches several (tail drain especially). Split extras onto same-engine nops.
_orig_exit = tile.TileContext.__exit__


def _split_waits(nc):
    for bb in nc.m.functions[0].blocks:
        out, extra = [], 0
        for inst in bb.instructions:
            si = inst.sync_info
            if si is not None and len(si.on_wait) > 1:
                waits = list(si.on_wait)
                for w in waits[:-1]:
                    nop = mybir.InstNoOp(name=f"I-wsplit-{nc.next_id()}")
                    nop.engine = inst.engine
                    nop.sync_info = bass_rust.SyncInfo(on_wait=[w], on_update=[])
                    nc.register_instruction(nop, overwrite=True)
                    out.append(nop)
                    extra += 1
                inst.sync_info = bass_rust.SyncInfo(
                    on_wait=[waits[-1]], on_update=list(si.on_update)
                )
            out.append(inst)
        if extra:
            bb.instructions = out


def _patched_exit(self, exc_type, exc_val, exc_tb):
    r = _orig_exit(self, exc_type, exc_val, exc_tb)
    _split_waits(self.nc)
    return r


tile.TileContext.__exit__ = _patched_exit
# ---------------------------------------------------------------------------

_CACHED_NC = None


def _drive(*gens):
    live = list(gens)
    while live:
        for g in list(live):
            try:
                next(g)
            except StopIteration:
                live.remove(g)


def build_program():
    global _CACHED_NC
    if _CACHED_NC is not None:
        return _CACHED_NC
    nc = bass.Bass()
    # per chunk c (=b*2+n, 512 positions): [hi 32x512 | lo 32x512] fp8 planes
    hsx_d = nc.declare_dram_parameter("hsx", [4, 128, 32768], dt.float8e4, isOutput=False)
    # [p][k][hi 640 | lo 640]
    wqx_d = nc.declare_dram_parameter("wqx", [128, 32 * 1280], dt.float8e4, isOutput=False)
    # [p][kt][hi 4096 | lo 4096]
    wdx_d = nc.declare_dram_parameter("wdx", [128, 4 * 8192], dt.float8e4, isOutput=False)
    cst_d = nc.declare_dram_parameter("cst", [128, 2048], dt.bfloat16, isOutput=False)
    msk_d = nc.declare_dram_parameter("msk", [128, 128], dt.bfloat16, isOutput=False)
    ab_d = nc.declare_dram_parameter("ab", [128, 128], dt.float32, isOutput=False)
    idn_d = nc.declare_dram_parameter("idn", [64, 64], dt.bfloat16, isOutput=False)
    outp_d = nc.declare_dram_parameter("outp", [32, 128, NPOS], dt.bfloat16, isOutput=True)

    AF = mybir.ActivationFunctionType
    # packed causal offsets for et: block ki has width 1024-128*ki
    koff = [0] * 8
    for ki in range(1, 8):
        koff[ki] = koff[ki - 1] + (1024 - 128 * (ki - 1))
    ET_W = koff[7] + (1024 - 128 * 7)  # 4608

    with ExitStack() as ctx:
        tc = ctx.enter_context(tile.TileContext(nc))
        cpool = ctx.enter_context(tc.tile_pool(name="const", bufs=1))
        cst_sb = cpool.tile([128, 2048], dt.bfloat16)
        msk_sb = cpool.tile([128, 128], dt.bfloat16)
        ab_sb = cpool.tile([128, 128], dt.float32)
        idn_sb = cpool.tile([64, 64], dt.bfloat16)
        onesf = cpool.tile([1, 64], dt.float32)
        nc.vector.memset(onesf[:], 1.0)
        ones_r = cpool.tile([1, 64], dt.float32r)
        nc.scalar.copy(ones_r[:], onesf[:])

        hs_pool = ctx.enter_context(tc.tile_pool(name="hs", bufs=4))
        raw_pool = ctx.enter_context(tc.tile_pool(name="raw", bufs=2))
        tmp_pool = ctx.enter_context(tc.tile_pool(name="tmp", bufs=1))
        qp_pool = ctx.enter_context(tc.tile_pool(name="qp", bufs=2))
        kv_pool = ctx.enter_context(tc.tile_pool(name="kv", bufs=2))
        va_pool = ctx.enter_context(tc.tile_pool(name="va", bufs=2))
        et_pool = ctx.enter_context(tc.tile_pool(name="et", bufs=2))
        l_pool = ctx.enter_context(tc.tile_pool(name="l", bufs=1))
        rb_pool = ctx.enter_context(tc.tile_pool(name="rb", bufs=1))
        ctx_pool = ctx.enter_context(tc.tile_pool(name="ctx", bufs=2))
        cxl_pool = ctx.enter_context(tc.tile_pool(name="cxl", bufs=2))
        dout_pool = ctx.enter_context(tc.tile_pool(name="dout", bufs=4))
        mm = ctx.enter_context(tc.tile_pool(name="mm", bufs=1, space="PSUM"))
        wd_pool = []  # created after wq_pool release so it reuses that zone

        # wq pool is created LAST so it sits on top of the SBUF stack and can
        # be released (LIFO) after phase B; the dense weights reuse its zone.
        wq_cm = tc.tile_pool(name="wqp", bufs=1)
        wq_pool = wq_cm.__enter__()
        # [128, k=32, hi|lo 1280] fp8
        wqx = wq_pool.tile([128, 32, 1280], dt.float8e4, tag="wqx", name="wqx")
        # first quarter unblocks the first matmuls asap
        nc.sync.dma_start(wqx[:, 0:8, :], wqx_d[:, 0:8 * 1280])

        # per-batch SBUF state, filled by gen_qkv, read by gen_attn/gen_dense
        qp = {}   # (b, h) -> [64, 1024] bf16
        kk = {}   # b -> [64, 1024] bf16
        va = {}   # b -> [128, 8*72] bf16
        ctxt = {}  # (b, pr) -> [128, 1024] bf16
        cxh = {}  # b -> [128, 4, 1024] fp8 hi
        cxl = {}  # b -> [128, 4, 1024] fp8 lo
        wdx_t = []

        def gen_qkv(b):
            for h in range(8):
                qp[(b, h)] = qp_pool.tile([64, 1024], dt.bfloat16, tag=f"qp{h}",
                                          name=f"qp{h}_{b}")
            kk[b] = kv_pool.tile([64, 1024], dt.bfloat16, tag="kk", name=f"kk{b}")
            vt = kv_pool.tile([64, 1024], dt.bfloat16, tag="vt", name=f"vt{b}")
            for n in range(2):
                c = b * 2 + n
                ncol = slice(n * 512, n * 512 + 512)
                ps = [mm.tile([128, 512], dt.float32, tag=f"qkv{m}", name=f"qkv{m}")
                      for m in range(5)]
                # 4 hs tiles per chunk: hiH0 (k0-15), hiH1 (k16-31), loH0, loH1
                hst = {}

                def _load(part, engs):
                    # part: 0=hiH0 1=hiH1 2=loH0 3=loH1
                    t = hs_pool.tile([128, 16, 512], dt.float8e4, tag="hs",
                                     name=f"hs{part}_{c}")
                    if c == 0 and part == 0:
                        # split first load so the first matmul unblocks early
                        for g in range(4):
                            nc.gpsimd.dma_start(
                                t[:, 4 * g:4 * g + 4, :],
                                hsx_d[c][:, g * 2048:(g + 1) * 2048])
                    else:
                        engs.dma_start(t[:], hsx_d[c][:, part * 8192:(part + 1) * 8192])
                    hst[part] = t

                _load(0, nc.gpsimd)
                if c == 0:
                    # small consts, needed from the RoPE/attention stages
                    nc.gpsimd.dma_start(cst_sb[:], cst_d[:])
                    nc.gpsimd.dma_start(msk_sb[:], msk_d[:])
                    nc.gpsimd.dma_start(ab_sb[:], ab_d[:])
                    nc.gpsimd.dma_start(idn_sb[:], idn_d[:])
                    nc.sync.dma_start(wqx[:, 8:16, :],
                                      wqx_d[:, 8 * 1280:16 * 1280])
                _load(1, nc.sync)
                raw = [raw_pool.tile([128, 512], dt.bfloat16, tag=f"raw{m}",
                                     name=f"raw{m}_{n}_{b}") for m in range(5)]
                # pass 1: hi moving planes -> M1 (hi wts) + M2 (lo wts)
                for kp in range(16):
                    if c == 0 and kp == 2:
                        nc.sync.dma_start(wqx[:, 16:24, :],
                                          wqx_d[:, 16 * 1280:24 * 1280])
                    if c == 0 and kp == 6:
                        nc.sync.dma_start(wqx[:, 24:32, :],
                                          wqx_d[:, 24 * 1280:32 * 1280])
                    if kp == 4:
                        _load(2, nc.gpsimd)
                    if kp == 10:
                        _load(3, nc.sync)
                    half, i = kp // 8, kp % 8
                    hi_mv = hst[half][:, 2 * i:2 * i + 2, :]
                    for m in range(5):
                        wsl_hi = wqx[:, 2 * kp:2 * kp + 2, m * 128:(m + 1) * 128]
                        wsl_lo = wqx[:, 2 * kp:2 * kp + 2,
                                     640 + m * 128:640 + (m + 1) * 128]
                        nc.tensor.matmul(ps[m][:], wsl_hi, hi_mv,
                                         start=(kp == 0), stop=False, perf_mode=DR)
                        nc.tensor.matmul(ps[m][:], wsl_lo, hi_mv,
                                         start=False, stop=False, perf_mode=DR)
                    if kp % 2 == 1:
                        yield
                # pass 2: lo moving planes -> M3 (hi wts)
                for kp in range(16):
                    half, i = kp // 8, kp % 8
                    lo_mv = hst[2 + half][:, 2 * i:2 * i + 2, :]
                    for m in range(5):
                        wsl_hi = wqx[:, 2 * kp:2 * kp + 2, m * 128:(m + 1) * 128]
                        nc.tensor.matmul(ps[m][:], wsl_hi, lo_mv,
                                         start=False, stop=(kp == 15), perf_mode=DR)
                        if kp == 15:
                            # drain each accumulator while PE finishes the rest
                            nc.scalar.copy(raw[m][:], ps[m][:])
                    if kp % 4 == 3:
                        yield
                Cs = cst_sb[:, n * 512:(n + 1) * 512]
                Ss = cst_sb[:, 1024 + n * 512: 1024 + (n + 1) * 512]
                for grp in range(2):
                    A, Bb = raw[grp * 2], raw[grp * 2 + 1]
                    P1 = tmp_pool.tile([128, 512], dt.bfloat16, tag="P1")
                    P2 = tmp_pool.tile([128, 512], dt.bfloat16, tag="P2")
                    P3 = tmp_pool.tile([128, 512], dt.bfloat16, tag="P3")
                    P4 = tmp_pool.tile([128, 512], dt.bfloat16, tag="P4")
                    nc.vector.tensor_mul(P1[:], A[:], Cs)
                    nc.vector.tensor_mul(P2[:], Bb[:], Ss)
                    nc.vector.tensor_mul(P3[:], Bb[:], Cs)
                    nc.vector.tensor_mul(P4[:], A[:], Ss)
                    for i in range(4):
                        h = grp * 4 + i
                        sl = slice(32 * i, 32 * i + 32)
                        nc.vector.tensor_sub(qp[(b, h)][0:32, ncol], P1[sl, :], P2[sl, :])
                        nc.vector.tensor_add(qp[(b, h)][32:64, ncol], P3[sl, :], P4[sl, :])
                kvr = raw[4]
                pk1 = tmp_pool.tile([32, 512], dt.bfloat16, tag="pk1")
                pk2 = tmp_pool.tile([32, 512], dt.bfloat16, tag="pk2")
                pk3 = tmp_pool.tile([32, 512], dt.bfloat16, tag="pk3")
                pk4 = tmp_pool.tile([32, 512], dt.bfloat16, tag="pk4")
                nc.vector.tensor_mul(pk1[:], kvr[0:32, :], Cs[0:32, :])
                nc.vector.tensor_mul(pk2[:], kvr[32:64, :], Ss[32:64, :])
                nc.vector.tensor_mul(pk3[:], kvr[32:64, :], Cs[32:64, :])
                nc.vector.tensor_mul(pk4[:], kvr[0:32, :], Ss[0:32, :])
                nc.vector.tensor_sub(kk[b][0:32, ncol], pk1[:], pk2[:])
                nc.vector.tensor_add(kk[b][32:64, ncol], pk3[:], pk4[:])
                nc.vector.tensor_copy(vt[:, ncol], kvr[64:128, :])
                yield
            # V transpose + ones column (borrows the aux PSUM bank)
            va[b] = va_pool.tile([128, 8 * 72], dt.bfloat16, tag="va", name=f"va{b}")
            for ki in range(8):
                slot = mm.tile([128, 512], dt.float32, tag="aux", name=f"vps{ki}_{b}")
                vps = slot[:, 0:32].bitcast(dt.bfloat16)
                nc.tensor.transpose(vps, vt[0:64, ki * 128:(ki + 1) * 128],
                                    idn_sb[:, :])
                nc.vector.tensor_copy(va[b][:, ki * 72: ki * 72 + 64], vps)
                nc.vector.memset(va[b][:, ki * 72 + 64: ki * 72 + 65], ONES_VAL)
            yield

        def gen_attn(b):
            for pr in range(4):
                ctxt[(b, pr)] = ctx_pool.tile([128, 1024], dt.bfloat16,
                                              tag=f"ctxt{pr}", name=f"ctxt{pr}_{b}")
            cxh[b] = ctx_pool.tile([128, 4, 1024], dt.float8e4, tag="cxh",
                                   name=f"cxh{b}")
            cxl[b] = cxl_pool.tile([128, 4, 1024], dt.float8e4, tag="cxl",
                                   name=f"cxl{b}")
            # b=0 (phase B): qkv(1) owns all 5 qkv slots, so PV shares the
            # cpsA/cpsB score slots and runs after all 12 exps of the head.
            # b=1 (phase C): dense only uses qkv0/1, so PV accumulates in
            # qkv2/qkv3 and interleaves per-ki into the score stream.
            cps_tags = ("cpsA", "cpsB")
            pv_inline = False
            rot = ("cpsA", "aux", "cpsB") if b == 0 else ("cpsA", "qkv4", "aux", "cpsB")
            for h in range(8):
                et = et_pool.tile([128, ET_W], dt.bfloat16, tag="et", name=f"et{h}_{b}")
                pr, hh = h // 2, h % 2
                rr = l_pool.tile([1, 1024], dt.float32r, tag="rr")
                rb = rb_pool.tile([64, 1024], dt.float32, tag="rb")
                cph = [mm.tile([128, 512], dt.float32, tag=cps_tags[0], name=f"cpA{h}_{b}"),
                       mm.tile([128, 512], dt.float32, tag=cps_tags[1], name=f"cpB{h}_{b}")]
                ci = 0

                def _pv(ki):
                    g0 = ki * 128
                    while g0 < 1024:
                        half = g0 // 512
                        g1 = min(1024, (half + 1) * 512)
                        loc = slice(g0 - half * 512, g1 - half * 512)
                        nc.tensor.matmul(
                            cph[half][0:65, loc],
                            va[b][:, ki * 72: ki * 72 + 65],
                            et[:, koff[ki] + g0 - ki * 128: koff[ki] + g1 - ki * 128],
                            start=(ki == 0), stop=(ki == (3 if half == 0 else 7)),
                            skip_group_check=True,
                        )
                        g0 = g1

                def _epi(ki):
                    # epilogue for the finished half: A after ki=3, B after 7
                    half = 0 if ki == 3 else 1
                    hs_ = slice(half * 512, half * 512 + 512)
                    # reciprocal straight from the PSUM ones-row
                    # (f32r is fp32-width; the gate only knows dtype != f32)
                    with nc.allow_low_precision(reason="f32r output is fp32-width"):
                        nc.vector.reciprocal(rr[0:1, hs_], cph[half][64:65, 0:512])
                    slot = mm.tile([128, 512], dt.float32, tag="aux",
                                   name=f"rps{h}{half}_{b}")
                    nc.tensor.matmul(slot[0:64, :], ones_r[:], rr[:, hs_],
                                     start=True, stop=True)
                    nc.vector.tensor_copy(rb[:, hs_], slot[0:64, :])
                    nc.vector.tensor_mul(
                        ctxt[(b, pr)][hh * 64:(hh + 1) * 64, hs_],
                        cph[half][0:64, 0:512], rb[:, hs_])
                    if ki == 7 and hh == 1:
                        # head pair done: split ctx into fp8 hi+lo planes for
                        # the DoubleRow dense
                        nc.scalar.copy(cxh[b][:, pr, :], ctxt[(b, pr)][:])
                        nc.vector.tensor_sub(cxl[b][:, pr, :], ctxt[(b, pr)][:],
                                             cxh[b][:, pr, :])

                for ki in range(8):
                    base = ki * 128
                    nchunks = (1024 - base + 511) // 512
                    for cj in range(nchunks):
                        c0 = base + cj * 512
                        cw = min(512, 1024 - c0)
                        sc = mm.tile([128, 512], dt.float32,
                                     tag=rot[ci % len(rot)],
                                     name=f"sc{h}{ki}{cj}_{b}")
                        ci += 1
                        nc.tensor.matmul(
                            sc[:, 0:cw],
                            kk[b][0:64, base:base + 128],
                            qp[(b, h)][0:64, c0:c0 + cw],
                            start=True, stop=True,
                        )
                        abc = b * 64 + ki * 8 + h
                        nc.scalar.activation(
                            et[:, koff[ki] + (c0 - base): koff[ki] + (c0 - base) + cw],
                            sc[:, 0:cw], AF.Exp,
                            bias=ab_sb[:, abc:abc + 1], scale=EXP_SCALE)
                        if cj == 0:
                            # causal mask: zero the upper triangle of the diag
                            # block via a 0/1 multiply (off the sc->exp chain)
                            nc.vector.tensor_mul(
                                et[:, koff[ki]: koff[ki] + 128],
                                et[:, koff[ki]: koff[ki] + 128], msk_sb[:])
                    if pv_inline:
                        _pv(ki)
                        if ki == 3 or ki == 7:
                            _epi(ki)
                            yield
                    if ki == 3:
                        yield
                if not pv_inline:
                    yield
                    for ki in range(8):
                        _pv(ki)
                        if ki == 3 or ki == 7:
                            _epi(ki)
                            yield

        def gen_dense(b):
            if b == 0:
                wdx = wd_pool[0].tile([128, 4, 8192], dt.float8e4, tag="wdx",
                                      name="wdx")
                wdx_t.append(wdx)
                nc.sync.dma_start(wdx[:, 0:2, :], wdx_d[:, 0:2 * 8192])
                nc.sync.dma_start(wdx[:, 2:4, :], wdx_d[:, 2 * 8192:4 * 8192])
            wdx = wdx_t[0]
            # b=1 runs after attention is done, so the score-rotation banks
            # are free for deeper accumulate/drain pipelining
            slots = ("qkv0", "qkv1", "qkv2", "qkv3") if b == 0 else (
                "qkv0", "qkv1", "qkv2", "qkv3", "aux", "cpsA")
            for mt in range(32):
                dsb = dout_pool.tile([128, 1024], dt.bfloat16, tag="dsb",
                                     name=f"dsb{mt}_{b}")
                for n2 in range(2):
                    dps = mm.tile([128, 512], dt.float32,
                                  tag=slots[(mt * 2 + n2) % len(slots)],
                                  name=f"d{mt}{n2}_{b}")
                    n2s = slice(n2 * 512, (n2 + 1) * 512)
                    for t in range(2):
                        ks = slice(2 * t, 2 * t + 2)
                        w_hi = wdx[:, ks, mt * 128:(mt + 1) * 128]
                        w_lo = wdx[:, ks, 4096 + mt * 128:4096 + (mt + 1) * 128]
                        nc.tensor.matmul(dps[:], w_hi, cxh[b][:, ks, n2s],
                                         start=(t == 0), stop=False, perf_mode=DR)
                        nc.tensor.matmul(dps[:], w_lo, cxh[b][:, ks, n2s],
                                         start=False, stop=False, perf_mode=DR)
                        nc.tensor.matmul(dps[:], w_hi, cxl[b][:, ks, n2s],
                                         start=False, stop=(t == 1), perf_mode=DR)
                    if n2 == 0:
                        nc.scalar.mul(dsb[:, 0:512], dps[:], DRAIN_SCALE)
                    else:
                        nc.vector.tensor_scalar_mul(dsb[:, 512:1024], dps[:],
                                                    DRAIN_SCALE)
                    if b == 1 and mt == 31:
                        # last tile: ship halves separately to shorten the
                        # final copy->DMA drain chain
                        nc.sync.dma_start(
                            outp_d[mt][:, b * 1024 + n2 * 512:
                                        b * 1024 + n2 * 512 + 512],
                            dsb[:, n2 * 512:(n2 + 1) * 512])
                if not (b == 1 and mt == 31):
                    eng = nc.gpsimd
                    eng.dma_start(
                        outp_d[mt][:, b * 1024: b * 1024 + 1024], dsb[:])
                yield

        _drive(gen_qkv(0))
        _drive(gen_qkv(1), gen_attn(0))
        wq_cm.__exit__(None, None, None)
        wd_pool.append(ctx.enter_context(tc.tile_pool(name="wdp", bufs=1)))
        _drive(gen_dense(0), gen_attn(1))
        _drive(gen_dense(1))

    _CACHED_NC = nc
    return nc


def _split8(x):
    """x (f32) -> (hi, lo) fp8e4 with x ~= hi + lo."""
    hi = x.astype(f8)
    lo = (x - hi.astype(np.float32)).astype(f8)
    return hi, lo


def host_prep(hidden_states, alibi, attention_mask, W_qkv, W_dense):
    hsT = np.ascontiguousarray(hidden_states.reshape(NPOS, HID).T)  # [4096, 2048]
    hh, hl = _split8(hsT.astype(np.float32))
    # hsx[c][p][k*512+j] (hi) / 16384 + same (lo) = hsT[k*128+p, c*512+j]
    def _arr(x8):
        return np.ascontiguousarray(
            x8.reshape(32, 128, 4, 512).transpose(2, 1, 0, 3).reshape(4, 128, 16384))
    hsx = np.concatenate([_arr(hh), _arr(hl)], axis=2)  # [4, 128, 32768]

    j32 = np.arange(32)
    inv_freq = 1.0 / (10000.0 ** (2 * j32 / HD))
    t = np.arange(S, dtype=np.float64)
    fr = np.outer(inv_freq, t)                       # [32, S]
    cst = np.zeros((128, 2048), np.float32)
    cst[:, 0:1024] = np.tile(np.cos(fr), (4, 1))
    cst[:, 1024:2048] = np.tile(np.sin(fr), (4, 1))
    cst = cst.astype(bf16)

    # single causal diag block, [kpos, q] layout: 0 where kpos > q, else 1
    mf = np.where(attention_mask[0, 0, 0:128, 0:128], 0.0, 1.0).astype(np.float32)
    msk = np.ascontiguousarray(mf.T).astype(bf16)    # [kpos, q]

    al = alibi.reshape(B, NKV * G, S) * INV          # [B, 64, S]

    perm = []
    for i in range(4):
        perm += [i * 64 + d for d in range(32)]
    for i in range(4):
        perm += [i * 64 + 32 + d for d in range(32)]
    for i in range(4, 8):
        perm += [i * 64 + d for d in range(32)]
    for i in range(4, 8):
        perm += [i * 64 + 32 + d for d in range(32)]
    perm += [512 + d for d in range(64)] + [576 + d for d in range(64)]
    perm = np.array(perm)

    idn = np.eye(64, dtype=np.float32).astype(bf16)
    in_maps = []
    for c in range(NCORES):
        Wg = (W_qkv[c * 640:(c + 1) * 640][perm] * WS).astype(np.float32)
        WgT = np.ascontiguousarray(Wg.T)              # [4096, 640]
        wh, wl = _split8(WgT)
        # wqx[p][k][0:640]=hi, [640:1280]=lo ; [k][p] from [4096,640]
        wqx = np.concatenate(
            [wh.reshape(32, 128, 640).transpose(1, 0, 2),
             wl.reshape(32, 128, 640).transpose(1, 0, 2)], axis=2)  # [128,32,1280]
        wqx = np.ascontiguousarray(wqx).reshape(128, 32 * 1280)

        Wd = (W_dense[:, c * 512:(c + 1) * 512] * WS).astype(np.float32)
        WdT = np.ascontiguousarray(Wd.T)              # [512, 4096]
        dh, dl = _split8(WdT)
        wdx = np.concatenate(
            [dh.reshape(4, 128, 4096).transpose(1, 0, 2),
             dl.reshape(4, 128, 4096).transpose(1, 0, 2)], axis=2)  # [128,4,8192]
        wdx = np.ascontiguousarray(wdx).reshape(128, 4 * 8192)

        ab = np.zeros((128, 128), np.float32)
        for b in range(2):
            for ki in range(8):
                for h in range(8):
                    ab[:, b * 64 + ki * 8 + h] = al[b, c * 8 + h,
                                                    ki * 128:(ki + 1) * 128]
        in_maps.append({
            "hsx": hsx, "wqx": wqx, "wdx": wdx, "cst": cst,
            "msk": msk, "ab": ab, "idn": idn,
        })
    return in_maps


def kernel(hidden_states, alibi, attention_mask, W_qkv, W_dense, _want_time=False):
    nc = build_program()
    in_maps = host_prep(np.asarray(hidden_states), np.asarray(alibi),
                        np.asarray(attention_mask), np.asarray(W_qkv),
                        np.asarray(W_dense))
    res = run_bass_kernel_spmd(nc, in_maps, list(range(NCORES)))
    acc = np.zeros((32, 128, NPOS), np.float32)
    for c in range(NCORES):
        acc += res.results[c]["outp"].astype(np.float32)
    out = acc.reshape(4096, NPOS).T.reshape(B, S, HID)
    if _want_time:
        return np.ascontiguousarray(out), res
    return np.ascontiguousarray(out)


# revision 55
# speedup vs baseline: 1.0699x; 1.0305x over previous
"""GQA attention block (dense_transformer) on 8 trn2 cores.

Sharding: tensor-parallel by kv-group. Core c owns kv-group c = 8 query
heads + 1 k + 1 v head (640 rows of W_qkv) and the matching 512 columns of
W_dense. hidden_states is replicated. Each core returns a bf16 partial
[4096, 2048] dense output; the host sums the 8 partials in f32.

v3: split-precision fp8 DoubleRow for the two big GEMMs. QKV and dense
weights/activations are decomposed host-side (and ctx on-chip) into
hi+lo fp8e4 planes; each K=256 pair runs as 3 DoubleRow matmuls
(hi*hi, lo*hi, hi*lo) at 0.5 cycles/col = 0.75x the bf16 PE cost with
bf16-level accuracy (dropped lo*lo term ~2^-8). Attention internals
(RoPE, scores, softmax, PV, epilogue) stay bf16 exactly as v2: per-head
scores with cpsA/cpsB/aux PSUM rotation, packed causal et layout, 0/1
triangle mask multiply on DVE, reciprocal-of-ones-row epilogue with
f32r ones-matmul broadcast. Scales: W_qkv and W_dense x64 host-side
(q,k,v 64x), exp activation scale INV/4096, va ones column 4.0 (ctx
16x true), final dense drain x2^-10. The wq pool (40KB hi+lo) is
released after phase B and the dense weights reuse its zone.
"""
import numpy as np
import ml_dtypes
from contextlib import ExitStack

import bass_rust
import concourse.bass as bass
import concourse.mybir as mybir
from concourse import tile
from concourse.bass_utils import run_bass_kernel_spmd

dt = mybir.dt
bf16 = ml_dtypes.bfloat16
f8 = ml_dtypes.float8_e4m3

B, S, HID = 2, 1024, 4096
NKV, G, HD = 8, 8, 64
NPOS = B * S
INV = 0.125
WS = 64.0                      # host-side weight scale (q,k,v come out 64x)
EXP_SCALE = INV / (WS * WS)    # PSUM scores are 4096x true
ONES_VAL = 4.0                 # va ones column -> ctx = 16x true
DRAIN_SCALE = 1.0 / 1024.0     # dense psum = 16 * 64 = 1024x true
NCORES = 8
DR = mybir.MatmulPerfMode.DoubleRow

# ---------------------------------------------------------------------------
# walrus in this container takes at most ONE sync-wait per instruction; Tile
# attaches several (tail drain especially). Split extras onto same-engine nops.
_orig_exit = tile.TileContext.__exit__


def _split_waits(nc):
    for bb in nc.m.functions[0].blocks:
        out, extra = [], 0
        for inst in bb.instructions:
            si = inst.sync_info
            if si is not None and len(si.on_wait) > 1:
                waits = list(si.on_wait)
                for w in waits[:-1]:
                    nop = mybir.InstNoOp(name=f"I-wsplit-{nc.next_id()}")
                    nop.engine = inst.engine
                    nop.sync_info = bass_rust.SyncInfo(on_wait=[w], on_update=[])
                    nc.register_instruction(nop, overwrite=True)
                    out.append(nop)
                    extra += 1
                inst.sync_info = bass_rust.SyncInfo(
                    on_wait=[waits[-1]], on_update=list(si.on_update)
                )
            out.append(inst)
        if extra:
            bb.instructions = out


def _patched_exit(self, exc_type, exc_val, exc_tb):
    r = _orig_exit(self, exc_type, exc_val, exc_tb)
    _split_waits(self.nc)
    return r


tile.TileContext.__exit__ = _patched_exit
# ---------------------------------------------------------------------------

_CACHED_NC = None


def _drive(*gens):
    live = list(gens)
    while live:
        for g in list(live):
            try:
                next(g)
            except StopIteration:
                live.remove(g)


def build_program():
    global _CACHED_NC
    if _CACHED_NC is not None:
        return _CACHED_NC
    nc = bass.Bass()
    # per chunk c (=b*2+n, 512 positions): [hi 32x512 | lo 32x512] fp8 planes
    hsx_d = nc.declare_dram_parameter("hsx", [4, 128, 32768], dt.float8e4, isOutput=False)
    # [p][k][hi 640 | lo 640]
    wqx_d = nc.declare_dram_parameter("wqx", [128, 32 * 1280], dt.float8e4, isOutput=False)
    # [p][kt][hi 4096 | lo 4096]
    wdx_d = nc.declare_dram_parameter("wdx", [128, 4 * 8192], dt.float8e4, isOutput=False)
    cst_d = nc.declare_dram_parameter("cst", [128, 2048], dt.bfloat16, isOutput=False)
    msk_d = nc.declare_dram_parameter("msk", [128, 128], dt.bfloat16, isOutput=False)
    ab_d = nc.declare_dram_parameter("ab", [128, 128], dt.float32, isOutput=False)
    idn_d = nc.declare_dram_parameter("idn", [64, 64], dt.bfloat16, isOutput=False)
    outp_d = nc.declare_dram_parameter("outp", [32, 128, NPOS], dt.bfloat16, isOutput=True)

    AF = mybir.ActivationFunctionType
    # packed causal offsets for et: block ki has width 1024-128*ki
    koff = [0] * 8
    for ki in range(1, 8):
        koff[ki] = koff[ki - 1] + (1024 - 128 * (ki - 1))
    ET_W = koff[7] + (1024 - 128 * 7)  # 4608

    with ExitStack() as ctx:
        tc = ctx.enter_context(tile.TileContext(nc))
        cpool = ctx.enter_context(tc.tile_pool(name="const", bufs=1))
        cst_sb = cpool.tile([128, 2048], dt.bfloat16)
        msk_sb = cpool.tile([128, 128], dt.bfloat16)
        ab_sb = cpool.tile([128, 128], dt.float32)
        idn_sb = cpool.tile([64, 64], dt.bfloat16)
        ones_b = cpool.tile([1, 64], dt.bfloat16)
        nc.vector.memset(ones_b[:], 1.0)

        hs_pool = ctx.enter_context(tc.tile_pool(name="hs", bufs=4))
        raw_pool = ctx.enter_context(tc.tile_pool(name="raw", bufs=2))
        tmp_pool = ctx.enter_context(tc.tile_pool(name="tmp", bufs=1))
        qp_pool = ctx.enter_context(tc.tile_pool(name="qp", bufs=2))
        kv_pool = ctx.enter_context(tc.tile_pool(name="kv", bufs=2))
        va_pool = ctx.enter_context(tc.tile_pool(name="va", bufs=2))
        et_pool = ctx.enter_context(tc.tile_pool(name="et", bufs=2))
        l_pool = ctx.enter_context(tc.tile_pool(name="l", bufs=1))
        rb_pool = ctx.enter_context(tc.tile_pool(name="rb", bufs=1))
        ctx_pool = ctx.enter_context(tc.tile_pool(name="ctx", bufs=2))
        cxl_pool = ctx.enter_context(tc.tile_pool(name="cxl", bufs=2))
        dout_pool = ctx.enter_context(tc.tile_pool(name="dout", bufs=3))
        wd_pool = ctx.enter_context(tc.tile_pool(name="wdp", bufs=1))
        wq_pool = ctx.enter_context(tc.tile_pool(name="wqp", bufs=1))
        mm = ctx.enter_context(tc.tile_pool(name="mm", bufs=1, space="PSUM"))

        # [128, k=32, hi|lo 1280] fp8
        wqx = wq_pool.tile([128, 32, 1280], dt.float8e4, tag="wqx", name="wqx")
        # small first slices unblock the first matmuls asap
        nc.sync.dma_start(wqx[:, 0:4, :], wqx_d[:, 0:4 * 1280])
        nc.sync.dma_start(wqx[:, 4:8, :], wqx_d[:, 4 * 1280:8 * 1280])

        # per-batch SBUF state, filled by gen_qkv, read by gen_attn/gen_dense
        qp = {}   # (b, h) -> [64, 1024] bf16
        kk = {}   # b -> [64, 1024] bf16
        va = {}   # b -> [128, 8*72] bf16
        ctxt = {}  # (b, pr) -> [128, 1024] bf16
        cxh = {}  # b -> [128, 4, 1024] fp8 hi
        cxl = {}  # b -> [128, 4, 1024] fp8 lo
        wdx_t = []

        def gen_qkv(b):
            # two heads share one 128-partition tile (h even: rows 0-63,
            # h odd: rows 64-127); kk is duplicated into both halves so the
            # odd-head score matmuls use matching base_partition 64
            for pr in range(4):
                qph = qp_pool.tile([128, 1024], dt.bfloat16, tag=f"qph{pr}",
                                   name=f"qph{pr}_{b}")
                qp[(b, 2 * pr)] = qph[0:64, :]
                qp[(b, 2 * pr + 1)] = qph[64:128, :]
            kk[b] = kv_pool.tile([128, 1024], dt.bfloat16, tag="kk", name=f"kk{b}")
            vt = kv_pool.tile([64, 1024], dt.bfloat16, tag="vt", name=f"vt{b}")
            for n in range(2):
                c = b * 2 + n
                ncol = slice(n * 512, n * 512 + 512)
                # 4 hs tiles per chunk: hiH0 (k0-15), hiH1 (k16-31), loH0, loH1
                hst = {}

                def _load(part, engs):
                    # part: 0=hiH0 1=hiH1 2=loH0 3=loH1
                    t = hs_pool.tile([128, 16, 512], dt.float8e4, tag="hs",
                                     name=f"hs{part}_{c}")
                    if c == 0 and part == 0:
                        # split first load so the first matmul unblocks early
                        for g in range(4):
                            nc.gpsimd.dma_start(
                                t[:, 4 * g:4 * g + 4, :],
                                hsx_d[c][:, g * 2048:(g + 1) * 2048])
                    else:
                        engs.dma_start(t[:], hsx_d[c][:, part * 8192:(part + 1) * 8192])
                    hst[part] = t

                _load(0, nc.gpsimd)
                if c == 0:
                    # small consts, needed from the RoPE/attention stages
                    nc.gpsimd.dma_start(cst_sb[:], cst_d[:])
                    nc.gpsimd.dma_start(msk_sb[:], msk_d[:])
                    nc.gpsimd.dma_start(ab_sb[:], ab_d[:])
                    nc.gpsimd.dma_start(idn_sb[:], idn_d[:])
                    nc.sync.dma_start(wqx[:, 8:16, :],
                                      wqx_d[:, 8 * 1280:16 * 1280])
                _load(1, nc.sync)
                raw = [raw_pool.tile([128, 512], dt.bfloat16, tag=f"raw{m}",
                                     name=f"raw{m}_{n}_{b}") for m in range(5)]
                # two sweeps over the same hs tiles (m 0-2, then m 3-4) so
                # QKV only ever holds 3 PSUM banks; attention keeps
                # qkv3/qkv4/aux + cpsA/cpsB at all times. Same PE cycles
                # (cost is output columns only).
                for ms in ((0, 1, 2), (3, 4)):
                    ps = {m: mm.tile([128, 512], dt.float32, tag=f"qkv{j}",
                                     name=f"qkv{m}_{n}_{b}")
                          for j, m in enumerate(ms)}
                    # pass 1: hi moving planes -> M1 (hi wts) + M2 (lo wts)
                    for kp in range(16):
                        if ms[0] == 0:
                            if c == 0 and kp == 2:
                                nc.sync.dma_start(wqx[:, 16:24, :],
                                                  wqx_d[:, 16 * 1280:24 * 1280])
                            if c == 0 and kp == 6:
                                nc.sync.dma_start(wqx[:, 24:32, :],
                                                  wqx_d[:, 24 * 1280:32 * 1280])
                            if kp == 4:
                                _load(2, nc.gpsimd)
                            if kp == 10:
                                _load(3, nc.sync)
                        half, i = kp // 8, kp % 8
                        hi_mv = hst[half][:, 2 * i:2 * i + 2, :]
                        for m in ms:
                            wsl_hi = wqx[:, 2 * kp:2 * kp + 2,
                                         m * 128:(m + 1) * 128]
                            wsl_lo = wqx[:, 2 * kp:2 * kp + 2,
                                         640 + m * 128:640 + (m + 1) * 128]
                            nc.tensor.matmul(ps[m][:], wsl_hi, hi_mv,
                                             start=(kp == 0), stop=False,
                                             perf_mode=DR)
                            nc.tensor.matmul(ps[m][:], wsl_lo, hi_mv,
                                             start=False, stop=False,
                                             perf_mode=DR)
                        yield
                    # pass 2: lo moving planes -> M3 (hi wts)
                    for kp in range(16):
                        half, i = kp // 8, kp % 8
                        lo_mv = hst[2 + half][:, 2 * i:2 * i + 2, :]
                        for m in ms:
                            wsl_hi = wqx[:, 2 * kp:2 * kp + 2,
                                         m * 128:(m + 1) * 128]
                            nc.tensor.matmul(ps[m][:], wsl_hi, lo_mv,
                                             start=False, stop=(kp == 15),
                                             perf_mode=DR)
                            if kp == 15:
                                # drain while PE finishes the rest
                                nc.scalar.copy(raw[m][:], ps[m][:])
                        if kp % 2 == 1:
                            yield
                Cs = cst_sb[:, n * 512:(n + 1) * 512]
                Ss = cst_sb[:, 1024 + n * 512: 1024 + (n + 1) * 512]
                for grp in range(2):
                    A, Bb = raw[grp * 2], raw[grp * 2 + 1]
                    P1 = tmp_pool.tile([128, 512], dt.bfloat16, tag="P1")
                    P2 = tmp_pool.tile([128, 512], dt.bfloat16, tag="P2")
                    P3 = tmp_pool.tile([128, 512], dt.bfloat16, tag="P3")
                    P4 = tmp_pool.tile([128, 512], dt.bfloat16, tag="P4")
                    nc.vector.tensor_mul(P1[:], A[:], Cs)
                    nc.vector.tensor_mul(P2[:], Bb[:], Ss)
                    nc.vector.tensor_mul(P3[:], Bb[:], Cs)
                    nc.vector.tensor_mul(P4[:], A[:], Ss)
                    for i in range(4):
                        h = grp * 4 + i
                        sl = slice(32 * i, 32 * i + 32)
                        nc.vector.tensor_sub(qp[(b, h)][0:32, ncol], P1[sl, :], P2[sl, :])
                        nc.vector.tensor_add(qp[(b, h)][32:64, ncol], P3[sl, :], P4[sl, :])
                kvr = raw[4]
                # reuse the P tiles' space for the k-RoPE temporaries
                pk1 = tmp_pool.tile([128, 512], dt.bfloat16, tag="P1",
                                    name=f"pk1_{n}_{b}")[0:32, :]
                pk2 = tmp_pool.tile([128, 512], dt.bfloat16, tag="P2",
                                    name=f"pk2_{n}_{b}")[0:32, :]
                pk3 = tmp_pool.tile([128, 512], dt.bfloat16, tag="P3",
                                    name=f"pk3_{n}_{b}")[0:32, :]
                pk4 = tmp_pool.tile([128, 512], dt.bfloat16, tag="P4",
                                    name=f"pk4_{n}_{b}")[0:32, :]
                nc.vector.tensor_mul(pk1[:], kvr[0:32, :], Cs[0:32, :])
                nc.vector.tensor_mul(pk2[:], kvr[32:64, :], Ss[32:64, :])
                nc.vector.tensor_mul(pk3[:], kvr[32:64, :], Cs[32:64, :])
                nc.vector.tensor_mul(pk4[:], kvr[0:32, :], Ss[0:32, :])
                nc.vector.tensor_sub(kk[b][0:32, ncol], pk1[:], pk2[:])
                nc.vector.tensor_add(kk[b][32:64, ncol], pk3[:], pk4[:])
                nc.vector.tensor_copy(vt[:, ncol], kvr[64:128, :])
                # duplicate k into rows 64-127 for the odd (base-64) heads
                nc.gpsimd.tensor_copy(kk[b][64:128, ncol], kk[b][0:64, ncol])
                yield
            # V transpose + ones column (borrows the aux PSUM bank)
            va[b] = va_pool.tile([128, 8 * 72], dt.bfloat16, tag="va", name=f"va{b}")
            for ki in range(8):
                slot = mm.tile([128, 512], dt.float32, tag="aux", name=f"vps{ki}_{b}")
                vps = slot[:, 0:32].bitcast(dt.bfloat16)
                nc.tensor.transpose(vps, vt[0:64, ki * 128:(ki + 1) * 128],
                                    idn_sb[:, :])
                nc.vector.tensor_copy(va[b][:, ki * 72: ki * 72 + 64], vps)
                nc.vector.memset(va[b][:, ki * 72 + 64: ki * 72 + 65], ONES_VAL)
            yield

        def gen_attn(b):
            for pr in range(4):
                ctxt[(b, pr)] = ctx_pool.tile([128, 1024], dt.bfloat16,
                                              tag=f"ctxt{pr}", name=f"ctxt{pr}_{b}")
            cxh[b] = ctx_pool.tile([128, 4, 1024], dt.float8e4, tag="cxh",
                                   name=f"cxh{b}")
            cxl[b] = cxl_pool.tile([128, 4, 1024], dt.float8e4, tag="cxl",
                                   name=f"cxl{b}")
            # Both batches: heads are software-pipelined (scores of head h
            # interleave with PV/epi of head h-1) so the exp round-trip is
            # hidden; the co-running generator (qkv(1) in phase B, dense(0)
            # in phase C) packs the remaining PE gaps. Scores rotate through
            # qkv3/qkv4/aux, PV owns cpsA/cpsB, qkv/dense use qkv0..2.
            rot = ("qkv3", "qkv4", "aux")

            def make_head(h):
                st = {}
                st['et'] = et_pool.tile([128, ET_W], dt.bfloat16, tag="et",
                                        name=f"et{h}_{b}")
                rrb = rb_pool.tile([128, 1024], dt.bfloat16, tag="rb",
                                   name=f"rrb{h}_{b}")
                st['rr'] = rrb[0:1, :]
                st['rb'] = rrb[64:128, :]
                st['cph'] = [
                    mm.tile([128, 512], dt.float32, tag="cpsA", name=f"cpA{h}_{b}"),
                    mm.tile([128, 512], dt.float32, tag="cpsB", name=f"cpB{h}_{b}")]
                st['ci'] = 0
                return st

            def score_ki(st, h, ki):
                et = st['et']
                base = ki * 128
                nchunks = (1024 - base + 511) // 512
                row0 = 64 * (h % 2)
                for cj in range(nchunks):
                    c0 = base + cj * 512
                    cw = min(512, 1024 - c0)
                    sc = mm.tile([128, 512], dt.float32,
                                 tag=rot[st['ci'] % len(rot)],
                                 name=f"sc{h}{ki}{cj}_{b}")
                    st['ci'] += 1
                    nc.tensor.matmul(
                        sc[:, 0:cw],
                        kk[b][row0:row0 + 64, base:base + 128],
                        qp[(b, h)][0:64, c0:c0 + cw],
                        start=True, stop=True,
                    )
                    abc = b * 64 + ki * 8 + h
                    nc.scalar.activation(
                        et[:, koff[ki] + (c0 - base): koff[ki] + (c0 - base) + cw],
                        sc[:, 0:cw], AF.Exp,
                        bias=ab_sb[:, abc:abc + 1], scale=EXP_SCALE)
                    if cj == 0:
                        # causal mask: zero the upper triangle of the diag
                        # block via a 0/1 multiply (off the sc->exp chain).
                        # b=0 runs it on Pool so the phase-B tail leaves no
                        # DVE backlog to stall attn(1)'s chain.
                        meng = nc.gpsimd if b == 0 else nc.vector
                        meng.tensor_mul(
                            et[:, koff[ki]: koff[ki] + 128],
                            et[:, koff[ki]: koff[ki] + 128], msk_sb[:])

            def pv_ki(st, h, ki):
                et, cph = st['et'], st['cph']
                g0 = ki * 128
                while g0 < 1024:
                    half = g0 // 512
                    g1 = min(1024, (half + 1) * 512)
                    loc = slice(g0 - half * 512, g1 - half * 512)
                    nc.tensor.matmul(
                        cph[half][0:65, loc],
                        va[b][:, ki * 72: ki * 72 + 65],
                        et[:, koff[ki] + g0 - ki * 128: koff[ki] + g1 - ki * 128],
                        start=(ki == 0), stop=(ki == (3 if half == 0 else 7)),
                        skip_group_check=True,
                    )
                    g0 = g1

            def epi(st, h, ki):
                # epilogue for the finished half: A after ki=3, B after 7
                rr, rb, cph = st['rr'], st['rb'], st['cph']
                pr, hh = h // 2, h % 2
                half = 0 if ki == 3 else 1
                hs_ = slice(half * 512, half * 512 + 512)
                # reciprocal straight from the PSUM ones-row
                # (f32r is fp32-width; the gate only knows dtype != f32)
                with nc.allow_low_precision(reason="1/l in bf16 is accurate enough"):
                    nc.vector.reciprocal(rr[0:1, hs_], cph[half][64:65, 0:512])
                slot = mm.tile([128, 512], dt.float32, tag="aux",
                               name=f"rps{h}{half}_{b}")
                nc.tensor.matmul(slot[0:64, :], ones_b[:], rr[:, hs_],
                                 start=True, stop=True)
                nc.vector.tensor_copy(rb[:, hs_], slot[0:64, :])
                nc.vector.tensor_mul(
                    ctxt[(b, pr)][hh * 64:(hh + 1) * 64, hs_],
                    cph[half][0:64, 0:512], rb[:, hs_])
                if ki == 7 and hh == 1:
                    # head pair done: split ctx into fp8 hi+lo planes for the
                    # DoubleRow dense. b=1's sub runs on Pool: it's only
                    # needed in phase D, keep it off the chain-critical DVE.
                    nc.scalar.copy(cxh[b][:, pr, :], ctxt[(b, pr)][:])
                    seng = nc.vector if b == 0 else nc.gpsimd
                    seng.tensor_sub(cxl[b][:, pr, :], ctxt[(b, pr)][:],
                                    cxh[b][:, pr, :])

            if b == 0:
                wdx = wd_pool.tile([128, 4, 8192], dt.float8e4, tag="wdx",
                                   name="wdx")
                wdx_t.append(wdx)
                # dense weights stream during phase B (DMA is idle here)
                nc.scalar.dma_start(wdx[:, 0:2, :], wdx_d[:, 0:2 * 8192])
                nc.scalar.dma_start(wdx[:, 2:4, :], wdx_d[:, 2 * 8192:4 * 8192])
            prev = None
            for slot in range(9):
                cur = make_head(slot) if slot < 8 else None
                for ki in range(8):
                    if cur is not None:
                        score_ki(cur, slot, ki)
                    if prev is not None:
                        pv_ki(prev, slot - 1, ki)
                        if ki == 3 or ki == 7:
                            epi(prev, slot - 1, ki)
                    yield
                prev = cur

        def gen_dense(b):
            wdx = wdx_t[0]
            # b=1 runs after attention is done, so the score-rotation banks
            # are free for deeper accumulate/drain pipelining
            slots = ("qkv0", "qkv1", "qkv2") if b == 0 else (
                "qkv0", "qkv1", "qkv2", "qkv3", "aux", "cpsA")
            for mt in range(32):
                dsb = dout_pool.tile([128, 1024], dt.bfloat16, tag="dsb",
                                     name=f"dsb{mt}_{b}")
                for n2 in range(2):
                    dps = mm.tile([128, 512], dt.float32,
                                  tag=slots[(mt * 2 + n2) % len(slots)],
                                  name=f"d{mt}{n2}_{b}")
                    n2s = slice(n2 * 512, (n2 + 1) * 512)
                    for t in range(2):
                        ks = slice(2 * t, 2 * t + 2)
                        w_hi = wdx[:, ks, mt * 128:(mt + 1) * 128]
                        w_lo = wdx[:, ks, 4096 + mt * 128:4096 + (mt + 1) * 128]
                        nc.tensor.matmul(dps[:], w_hi, cxh[b][:, ks, n2s],
                                         start=(t == 0), stop=False, perf_mode=DR)
                        nc.tensor.matmul(dps[:], w_lo, cxh[b][:, ks, n2s],
                                         start=False, stop=False, perf_mode=DR)
                        nc.tensor.matmul(dps[:], w_hi, cxl[b][:, ks, n2s],
                                         start=False, stop=(t == 1), perf_mode=DR)
                        if b == 0:
                            # fine-grained quanta so dense matmuls pack into
                            # the gaps of attn(1)'s serial score->exp chain
                            yield
                    if n2 == 0:
                        nc.scalar.mul(dsb[:, 0:512], dps[:], DRAIN_SCALE)
                    else:
                        nc.vector.tensor_scalar_mul(dsb[:, 512:1024], dps[:],
                                                    DRAIN_SCALE)
                    if b == 1 and mt == 31:
                        # last tile: ship halves separately to shorten the
                        # final copy->DMA drain chain
                        nc.sync.dma_start(
                            outp_d[mt][:, b * 1024 + n2 * 512:
                                        b * 1024 + n2 * 512 + 512],
                            dsb[:, n2 * 512:(n2 + 1) * 512])
                    yield
                if not (b == 1 and mt == 31):
                    # SP queue is idle through C/D; keep Pool free for drains
                    nc.sync.dma_start(
                        outp_d[mt][:, b * 1024: b * 1024 + 1024], dsb[:])

        def _chain(*gens):
            for g in gens:
                yield from g

        _drive(gen_qkv(0))
        # merge phases B and C: as soon as qkv(1) finishes emitting, attn(1)
        # interleaves with attn(0)'s tail; when attn(0) ends, dense(0)
        # interleaves with attn(1)'s tail.
        _drive(_chain(gen_qkv(1), gen_attn(1)),
               _chain(gen_attn(0), gen_dense(0)))
        _drive(gen_dense(1))

    _CACHED_NC = nc
    return nc


def _split8(x):
    """x (f32) -> (hi, lo) fp8e4 with x ~= hi + lo."""
    hi = x.astype(f8)
    lo = (x - hi.astype(np.float32)).astype(f8)
    return hi, lo


def host_prep(hidden_states, alibi, attention_mask, W_qkv, W_dense):
    hsT = np.ascontiguousarray(hidden_states.reshape(NPOS, HID).T)  # [4096, 2048]
    hh, hl = _split8(hsT.astype(np.float32))
    # hsx[c][p][k*512+j] (hi) / 16384 + same (lo) = hsT[k*128+p, c*512+j]
    def _arr(x8):
        return np.ascontiguousarray(
            x8.reshape(32, 128, 4, 512).transpose(2, 1, 0, 3).reshape(4, 128, 16384))
    hsx = np.concatenate([_arr(hh), _arr(hl)], axis=2)  # [4, 128, 32768]

    j32 = np.arange(32)
    inv_freq = 1.0 / (10000.0 ** (2 * j32 / HD))
    t = np.arange(S, dtype=np.float64)
    fr = np.outer(inv_freq, t)                       # [32, S]
    cst = np.zeros((128, 2048), np.float32)
    cst[:, 0:1024] = np.tile(np.cos(fr), (4, 1))
    cst[:, 1024:2048] = np.tile(np.sin(fr), (4, 1))
    cst = cst.astype(bf16)

    # single causal diag block, [kpos, q] layout: 0 where kpos > q, else 1
    mf = np.where(attention_mask[0, 0, 0:128, 0:128], 0.0, 1.0).astype(np.float32)
    msk = np.ascontiguousarray(mf.T).astype(bf16)    # [kpos, q]

    al = alibi.reshape(B, NKV * G, S) * INV          # [B, 64, S]

    perm = []
    for i in range(4):
        perm += [i * 64 + d for d in range(32)]
    for i in range(4):
        perm += [i * 64 + 32 + d for d in range(32)]
    for i in range(4, 8):
        perm += [i * 64 + d for d in range(32)]
    for i in range(4, 8):
        perm += [i * 64 + 32 + d for d in range(32)]
    perm += [512 + d for d in range(64)] + [576 + d for d in range(64)]
    perm = np.array(perm)

    idn = np.eye(64, dtype=np.float32).astype(bf16)
    in_maps = []
    for c in range(NCORES):
        Wg = (W_qkv[c * 640:(c + 1) * 640][perm] * WS).astype(np.float32)
        WgT = np.ascontiguousarray(Wg.T)              # [4096, 640]
        wh, wl = _split8(WgT)
        # wqx[p][k][0:640]=hi, [640:1280]=lo ; [k][p] from [4096,640]
        wqx = np.concatenate(
            [wh.reshape(32, 128, 640).transpose(1, 0, 2),
             wl.reshape(32, 128, 640).transpose(1, 0, 2)], axis=2)  # [128,32,1280]
        wqx = np.ascontiguousarray(wqx).reshape(128, 32 * 1280)

        Wd = (W_dense[:, c * 512:(c + 1) * 512] * WS).astype(np.float32)
        WdT = np.ascontiguousarray(Wd.T)              # [512, 4096]
        dh, dl = _split8(WdT)
        wdx = np.concatenate(
            [dh.reshape(4, 128, 4096).transpose(1, 0, 2),
             dl.reshape(4, 128, 4096).transpose(1, 0, 2)], axis=2)  # [128,4,8192]
        wdx = np.ascontiguousarray(wdx).reshape(128, 4 * 8192)

        ab = np.zeros((128, 128), np.float32)
        for b in range(2):
            for ki in range(8):
                for h in range(8):
                    ab[:, b * 64 + ki * 8 + h] = al[b, c * 8 + h,
                                                    ki * 128:(ki + 1) * 128]
        in_maps.append({
            "hsx": hsx, "wqx": wqx, "wdx": wdx, "cst": cst,
            "msk": msk, "ab": ab, "idn": idn,
        })
    return in_maps


def kernel(hidden_states, alibi, attention_mask, W_qkv, W_dense, _want_time=False):
    nc = build_program()
    in_maps = host_prep(np.asarray(hidden_states), np.asarray(alibi),
                        np.asarray(attention_mask), np.asarray(W_qkv),
                        np.asarray(W_dense))
    res = run_bass_kernel_spmd(nc, in_maps, list(range(NCORES)))
    acc = np.zeros((32, 128, NPOS), np.float32)
    for c in range(NCORES):
        acc += res.results[c]["outp"].astype(np.float32)
    out = acc.reshape(4096, NPOS).T.reshape(B, S, HID)
    if _want_time:
        return np.ascontiguousarray(out), res
    return np.ascontiguousarray(out)


# revision 77
# speedup vs baseline: 1.0721x; 1.0020x over previous
"""GQA attention block (dense_transformer) on 8 trn2 cores.

Sharding: tensor-parallel by kv-group. Core c owns kv-group c = 8 query
heads + 1 k + 1 v head (640 rows of W_qkv) and the matching 512 columns of
W_dense. hidden_states is replicated. Each core returns a bf16 partial
[4096, 2048] dense output; the host sums the 8 partials in f32.

v3: split-precision fp8 DoubleRow for the two big GEMMs. QKV and dense
weights/activations are decomposed host-side (and ctx on-chip) into
hi+lo fp8e4 planes; each K=256 pair runs as 3 DoubleRow matmuls
(hi*hi, lo*hi, hi*lo) at 0.5 cycles/col = 0.75x the bf16 PE cost with
bf16-level accuracy (dropped lo*lo term ~2^-8). Attention internals
(RoPE, scores, softmax, PV, epilogue) stay bf16 exactly as v2: per-head
scores with cpsA/cpsB/aux PSUM rotation, packed causal et layout, 0/1
triangle mask multiply on DVE, reciprocal-of-ones-row epilogue with
f32r ones-matmul broadcast. Scales: W_qkv and W_dense x64 host-side
(q,k,v 64x), exp activation scale INV/4096, va ones column 4.0 (ctx
16x true), final dense drain x2^-10. The wq pool (40KB hi+lo) is
released after phase B and the dense weights reuse its zone.
"""
import numpy as np
import ml_dtypes
from contextlib import ExitStack

import bass_rust
import concourse.bass as bass
import concourse.mybir as mybir
from concourse import tile
from concourse.bass_utils import run_bass_kernel_spmd

dt = mybir.dt
bf16 = ml_dtypes.bfloat16
f8 = ml_dtypes.float8_e4m3

B, S, HID = 2, 1024, 4096
NKV, G, HD = 8, 8, 64
NPOS = B * S
INV = 0.125
WS = 64.0                      # host-side weight scale (q,k,v come out 64x)
EXP_SCALE = INV / (WS * WS)    # PSUM scores are 4096x true
ONES_VAL = 4.0                 # va ones column -> ctx = 16x true
DRAIN_SCALE = 1.0 / 1024.0     # dense psum = 16 * 64 = 1024x true
NCORES = 8
DR = mybir.MatmulPerfMode.DoubleRow

# ---------------------------------------------------------------------------
# walrus in this container takes at most ONE sync-wait per instruction; Tile
# attaches several (tail drain especially). Split extras onto same-engine nops.
_orig_exit = tile.TileContext.__exit__


def _split_waits(nc):
    for bb in nc.m.functions[0].blocks:
        out, extra = [], 0
        for inst in bb.instructions:
            si = inst.sync_info
            if si is not None and len(si.on_wait) > 1:
                waits = list(si.on_wait)
                for w in waits[:-1]:
                    nop = mybir.InstNoOp(name=f"I-wsplit-{nc.next_id()}")
                    nop.engine = inst.engine
                    nop.sync_info = bass_rust.SyncInfo(on_wait=[w], on_update=[])
                    nc.register_instruction(nop, overwrite=True)
                    out.append(nop)
                    extra += 1
                inst.sync_info = bass_rust.SyncInfo(
                    on_wait=[waits[-1]], on_update=list(si.on_update)
                )
            out.append(inst)
        if extra:
            bb.instructions = out


def _patched_exit(self, exc_type, exc_val, exc_tb):
    r = _orig_exit(self, exc_type, exc_val, exc_tb)
    _split_waits(self.nc)
    return r


tile.TileContext.__exit__ = _patched_exit
# ---------------------------------------------------------------------------

_CACHED_NC = None


def _drive(*gens):
    live = list(gens)
    while live:
        for g in list(live):
            try:
                next(g)
            except StopIteration:
                live.remove(g)


def build_program():
    global _CACHED_NC
    if _CACHED_NC is not None:
        return _CACHED_NC
    nc = bass.Bass()
    # per chunk c (=b*2+n, 512 positions): [hi 32x512 | lo 32x512] fp8 planes
    hsx_d = nc.declare_dram_parameter("hsx", [4, 128, 32768], dt.float8e4, isOutput=False)
    # [p][k][640] hi and lo planes as separate params (hi loads first)
    wqh_d = nc.declare_dram_parameter("wqh", [128, 32 * 640], dt.float8e4, isOutput=False)
    wql_d = nc.declare_dram_parameter("wql", [128, 32 * 640], dt.float8e4, isOutput=False)
    # [p][kt][hi 4096 | lo 4096]
    wdx_d = nc.declare_dram_parameter("wdx", [128, 4 * 8192], dt.float8e4, isOutput=False)
    cst_d = nc.declare_dram_parameter("cst", [128, 2048], dt.bfloat16, isOutput=False)
    msk_d = nc.declare_dram_parameter("msk", [128, 128], dt.bfloat16, isOutput=False)
    ab_d = nc.declare_dram_parameter("ab", [128, 128], dt.float32, isOutput=False)
    idn_d = nc.declare_dram_parameter("idn", [64, 64], dt.bfloat16, isOutput=False)
    outp_d = nc.declare_dram_parameter("outp", [32, 128, NPOS], dt.bfloat16, isOutput=True)

    AF = mybir.ActivationFunctionType
    # packed causal offsets for et: block ki has width 1024-128*ki
    koff = [0] * 8
    for ki in range(1, 8):
        koff[ki] = koff[ki - 1] + (1024 - 128 * (ki - 1))
    ET_W = koff[7] + (1024 - 128 * 7)  # 4608

    with ExitStack() as ctx:
        tc = ctx.enter_context(tile.TileContext(nc))
        cpool = ctx.enter_context(tc.tile_pool(name="const", bufs=1))
        cst_sb = cpool.tile([128, 2048], dt.bfloat16)
        msk_sb = cpool.tile([128, 128], dt.bfloat16)
        ab_sb = cpool.tile([128, 128], dt.float32)
        idn_sb = cpool.tile([64, 64], dt.bfloat16)
        ones_b = cpool.tile([1, 64], dt.bfloat16)
        nc.vector.memset(ones_b[:], 1.0)

        hs_pool = ctx.enter_context(tc.tile_pool(name="hs", bufs=4))
        raw_pool = ctx.enter_context(tc.tile_pool(name="raw", bufs=2))
        tmp_pool = ctx.enter_context(tc.tile_pool(name="tmp", bufs=1))
        qp_pool = ctx.enter_context(tc.tile_pool(name="qp", bufs=2))
        kv_pool = ctx.enter_context(tc.tile_pool(name="kv", bufs=2))
        va_pool = ctx.enter_context(tc.tile_pool(name="va", bufs=2))
        et_pool = ctx.enter_context(tc.tile_pool(name="et", bufs=2))
        l_pool = ctx.enter_context(tc.tile_pool(name="l", bufs=1))
        rb_pool = ctx.enter_context(tc.tile_pool(name="rb", bufs=1))
        ctx_pool = ctx.enter_context(tc.tile_pool(name="ctx", bufs=2))
        cxl_pool = ctx.enter_context(tc.tile_pool(name="cxl", bufs=2))
        dout_pool = ctx.enter_context(tc.tile_pool(name="dout", bufs=3))
        wd_pool = ctx.enter_context(tc.tile_pool(name="wdp", bufs=1))
        wq_pool = ctx.enter_context(tc.tile_pool(name="wqp", bufs=1))
        mm = ctx.enter_context(tc.tile_pool(name="mm", bufs=1, space="PSUM"))

        # [128, k=32, hi|lo 1280] fp8
        # separate tiles per lazily-loaded slice: a shared tile would add
        # false write-after-read deps from in-flight matmuls to later loads
        wqh = [wq_pool.tile([128, 8, 640], dt.float8e4, tag=f"wqh{q}",
                            name=f"wqh{q}") for q in range(4)]
        wql = [wq_pool.tile([128, 16, 640], dt.float8e4, tag=f"wql{q}",
                            name=f"wql{q}") for q in range(2)]
        # small first slices unblock the first matmuls asap
        nc.sync.dma_start(wqh[0][:, 0:4, :], wqh_d[:, 0:4 * 640])
        nc.sync.dma_start(wqh[0][:, 4:8, :], wqh_d[:, 4 * 640:8 * 640])

        # per-batch SBUF state, filled by gen_qkv, read by gen_attn/gen_dense
        qp = {}   # (b, h) -> [64, 1024] bf16
        kk = {}   # b -> [64, 1024] bf16
        va = {}   # b -> [128, 8*72] bf16
        ctxt = {}  # (b, pr) -> [128, 1024] bf16
        cxh = {}  # b -> [128, 4, 1024] fp8 hi
        cxl = {}  # b -> [128, 4, 1024] fp8 lo
        wdx_t = []

        def gen_qkv(b):
            # two heads share one 128-partition tile (h even: rows 0-63,
            # h odd: rows 64-127); kk is duplicated into both halves so the
            # odd-head score matmuls use matching base_partition 64
            for pr in range(4):
                qph = qp_pool.tile([128, 1024], dt.bfloat16, tag=f"qph{pr}",
                                   name=f"qph{pr}_{b}")
                qp[(b, 2 * pr)] = qph[0:64, :]
                qp[(b, 2 * pr + 1)] = qph[64:128, :]
            kk[b] = kv_pool.tile([128, 1024], dt.bfloat16, tag="kk", name=f"kk{b}")
            vt = kv_pool.tile([64, 1024], dt.bfloat16, tag="vt", name=f"vt{b}")
            for n in range(2):
                c = b * 2 + n
                ncol = slice(n * 512, n * 512 + 512)
                # 4 hs tiles per chunk: hiH0 (k0-15), hiH1 (k16-31), loH0, loH1
                hst = {}

                def _load(part, engs):
                    # part: 0=hiH0 1=hiH1 2=loH0 3=loH1
                    t = hs_pool.tile([128, 16, 512], dt.float8e4, tag="hs",
                                     name=f"hs{part}_{c}")
                    if c == 0 and part == 0:
                        # split first load so the first matmul unblocks early
                        for g in range(4):
                            nc.gpsimd.dma_start(
                                t[:, 4 * g:4 * g + 4, :],
                                hsx_d[c][:, g * 2048:(g + 1) * 2048])
                    else:
                        engs.dma_start(t[:], hsx_d[c][:, part * 8192:(part + 1) * 8192])
                    hst[part] = t

                _load(0, nc.gpsimd)
                if c == 0:
                    # small consts, needed from the RoPE/attention stages
                    nc.scalar.dma_start(cst_sb[:], cst_d[:])
                    nc.scalar.dma_start(msk_sb[:], msk_d[:])
                    nc.scalar.dma_start(ab_sb[:], ab_d[:])
                    nc.scalar.dma_start(idn_sb[:], idn_d[:])
                    nc.sync.dma_start(wqh[1][:], wqh_d[:, 8 * 640:16 * 640])
                _load(1, nc.sync)
                raw = [raw_pool.tile([128, 512], dt.bfloat16, tag=f"raw{m}",
                                     name=f"raw{m}_{n}_{b}") for m in range(5)]
                # b=0 (phase A, attention not running): one 5-bank sweep.
                # b=1 (phase B): two sweeps (m 0-2, then 3-4) so QKV holds
                # only qkv0..2 and attention keeps qkv3/qkv4/aux+cpsA/cpsB.
                # Either way 3 passes (M1 hi*hi, M2 lo-w*hi-x, M3 hi-w*lo-x):
                # the cold start only needs hi weights + hi activations.
                tag5 = ("qkv0", "qkv1", "qkv2", "cpsA", "cpsB")
                sweeps = ((0, 1, 2, 3, 4),) if b == 0 else ((0, 1, 2), (3, 4))
                for ms in sweeps:
                    ps = {m: mm.tile([128, 512], dt.float32, tag=tag5[j],
                                     name=f"qkv{m}_{n}_{b}")
                          for j, m in enumerate(ms)}
                    for pa in range(3):
                        for kp in range(16):
                            if ms[0] == 0 and pa == 0:
                                if c == 0 and kp == 2:
                                    nc.sync.dma_start(wqh[2][:],
                                                      wqh_d[:, 16 * 640:24 * 640])
                                    nc.gpsimd.dma_start(wql[0][:],
                                                        wql_d[:, 0:16 * 640])
                                if c == 0 and kp == 4:
                                    nc.sync.dma_start(wqh[3][:],
                                                      wqh_d[:, 24 * 640:32 * 640])
                                if c == 0 and kp == 6:
                                    nc.gpsimd.dma_start(wql[1][:],
                                                        wql_d[:, 16 * 640:32 * 640])
                                if kp == (10 if c == 0 else 4):
                                    _load(2, nc.gpsimd)
                                if kp == (12 if c == 0 else 10):
                                    _load(3, nc.sync)
                            half, i = kp // 8, kp % 8
                            mv = hst[(2 if pa == 2 else 0) + half][:, 2 * i:2 * i + 2, :]
                            for m in ms:
                                if pa == 1:
                                    wsl = wql[kp // 8][:, 2 * (kp % 8):2 * (kp % 8) + 2,
                                              m * 128:(m + 1) * 128]
                                else:
                                    wsl = wqh[kp // 4][:, 2 * (kp % 4):2 * (kp % 4) + 2,
                                              m * 128:(m + 1) * 128]
                                nc.tensor.matmul(ps[m][:], wsl, mv,
                                                 start=(pa == 0 and kp == 0),
                                                 stop=(pa == 2 and kp == 15),
                                                 perf_mode=DR)
                                if pa == 2 and kp == 15:
                                    # drain while PE finishes the rest
                                    nc.scalar.copy(raw[m][:], ps[m][:])
                            if kp % 2 == 1:
                                yield
                Cs = cst_sb[:, n * 512:(n + 1) * 512]
                Ss = cst_sb[:, 1024 + n * 512: 1024 + (n + 1) * 512]
                for grp in range(2):
                    A, Bb = raw[grp * 2], raw[grp * 2 + 1]
                    P1 = tmp_pool.tile([128, 512], dt.bfloat16, tag="P1")
                    P2 = tmp_pool.tile([128, 512], dt.bfloat16, tag="P2")
                    P3 = tmp_pool.tile([128, 512], dt.bfloat16, tag="P3")
                    P4 = tmp_pool.tile([128, 512], dt.bfloat16, tag="P4")
                    nc.vector.tensor_mul(P1[:], A[:], Cs)
                    nc.vector.tensor_mul(P2[:], Bb[:], Ss)
                    nc.vector.tensor_mul(P3[:], Bb[:], Cs)
                    nc.vector.tensor_mul(P4[:], A[:], Ss)
                    for i in range(4):
                        h = grp * 4 + i
                        sl = slice(32 * i, 32 * i + 32)
                        nc.vector.tensor_sub(qp[(b, h)][0:32, ncol], P1[sl, :], P2[sl, :])
                        nc.vector.tensor_add(qp[(b, h)][32:64, ncol], P3[sl, :], P4[sl, :])
                kvr = raw[4]
                # reuse the P tiles' space for the k-RoPE temporaries
                pk1 = tmp_pool.tile([128, 512], dt.bfloat16, tag="P1",
                                    name=f"pk1_{n}_{b}")[0:32, :]
                pk2 = tmp_pool.tile([128, 512], dt.bfloat16, tag="P2",
                                    name=f"pk2_{n}_{b}")[0:32, :]
                pk3 = tmp_pool.tile([128, 512], dt.bfloat16, tag="P3",
                                    name=f"pk3_{n}_{b}")[0:32, :]
                pk4 = tmp_pool.tile([128, 512], dt.bfloat16, tag="P4",
                                    name=f"pk4_{n}_{b}")[0:32, :]
                nc.vector.tensor_mul(pk1[:], kvr[0:32, :], Cs[0:32, :])
                nc.vector.tensor_mul(pk2[:], kvr[32:64, :], Ss[32:64, :])
                nc.vector.tensor_mul(pk3[:], kvr[32:64, :], Cs[32:64, :])
                nc.vector.tensor_mul(pk4[:], kvr[0:32, :], Ss[0:32, :])
                nc.vector.tensor_sub(kk[b][0:32, ncol], pk1[:], pk2[:])
                nc.vector.tensor_add(kk[b][32:64, ncol], pk3[:], pk4[:])
                nc.vector.tensor_copy(vt[:, ncol], kvr[64:128, :])
                # duplicate k into rows 64-127 for the odd (base-64) heads
                nc.gpsimd.tensor_copy(kk[b][64:128, ncol], kk[b][0:64, ncol])
                yield
            # V transpose + ones column (borrows the aux PSUM bank)
            va[b] = va_pool.tile([128, 8 * 72], dt.bfloat16, tag="va", name=f"va{b}")
            for ki in range(8):
                slot = mm.tile([128, 512], dt.float32, tag="aux", name=f"vps{ki}_{b}")
                vps = slot[:, 0:32].bitcast(dt.bfloat16)
                nc.tensor.transpose(vps, vt[0:64, ki * 128:(ki + 1) * 128],
                                    idn_sb[:, :])
                nc.vector.tensor_copy(va[b][:, ki * 72: ki * 72 + 64], vps)
                nc.vector.memset(va[b][:, ki * 72 + 64: ki * 72 + 65], ONES_VAL)
            yield

        def gen_attn(b):
            for pr in range(4):
                ctxt[(b, pr)] = ctx_pool.tile([128, 1024], dt.bfloat16,
                                              tag=f"ctxt{pr}", name=f"ctxt{pr}_{b}")
            cxh[b] = ctx_pool.tile([128, 4, 1024], dt.float8e4, tag="cxh",
                                   name=f"cxh{b}")
            cxl[b] = cxl_pool.tile([128, 4, 1024], dt.float8e4, tag="cxl",
                                   name=f"cxl{b}")
            # Both batches: heads are software-pipelined (scores of head h
            # interleave with PV/epi of head h-1) so the exp round-trip is
            # hidden; the co-running generator (qkv(1) in phase B, dense(0)
            # in phase C) packs the remaining PE gaps. Scores rotate through
            # qkv3/qkv4/aux, PV owns cpsA/cpsB, qkv/dense use qkv0..2.
            rot = ("qkv3", "qkv4", "aux")

            def make_head(h):
                st = {}
                st['et'] = et_pool.tile([128, ET_W], dt.bfloat16, tag="et",
                                        name=f"et{h}_{b}")
                rrb = rb_pool.tile([128, 1024], dt.bfloat16, tag="rb",
                                   name=f"rrb{h}_{b}")
                st['rr'] = rrb[0:1, :]
                st['rb'] = rrb[64:128, :]
                st['cph'] = [
                    mm.tile([128, 512], dt.float32, tag="cpsA", name=f"cpA{h}_{b}"),
                    mm.tile([128, 512], dt.float32, tag="cpsB", name=f"cpB{h}_{b}")]
                st['ci'] = 0
                return st

            def score_ki(st, h, ki):
                et = st['et']
                base = ki * 128
                nchunks = (1024 - base + 511) // 512
                row0 = 64 * (h % 2)
                for cj in range(nchunks):
                    c0 = base + cj * 512
                    cw = min(512, 1024 - c0)
                    sc = mm.tile([128, 512], dt.float32,
                                 tag=rot[st['ci'] % len(rot)],
                                 name=f"sc{h}{ki}{cj}_{b}")
                    st['ci'] += 1
                    nc.tensor.matmul(
                        sc[:, 0:cw],
                        kk[b][row0:row0 + 64, base:base + 128],
                        qp[(b, h)][0:64, c0:c0 + cw],
                        start=True, stop=True,
                    )
                    abc = b * 64 + ki * 8 + h
                    nc.scalar.activation(
                        et[:, koff[ki] + (c0 - base): koff[ki] + (c0 - base) + cw],
                        sc[:, 0:cw], AF.Exp,
                        bias=ab_sb[:, abc:abc + 1], scale=EXP_SCALE)
                    if cj == 0:
                        # causal mask: zero the upper triangle of the diag
                        # block via a 0/1 multiply (off the sc->exp chain).
                        # Pool is otherwise idle and ACT/DVE are saturated.
                        nc.gpsimd.tensor_mul(
                            et[:, koff[ki]: koff[ki] + 128],
                            et[:, koff[ki]: koff[ki] + 128], msk_sb[:])

            def pv_ki(st, h, ki):
                et, cph = st['et'], st['cph']
                g0 = ki * 128
                while g0 < 1024:
                    half = g0 // 512
                    g1 = min(1024, (half + 1) * 512)
                    loc = slice(g0 - half * 512, g1 - half * 512)
                    nc.tensor.matmul(
                        cph[half][0:65, loc],
                        va[b][:, ki * 72: ki * 72 + 65],
                        et[:, koff[ki] + g0 - ki * 128: koff[ki] + g1 - ki * 128],
                        start=(ki == 0), stop=(ki == (3 if half == 0 else 7)),
                        skip_group_check=True,
                    )
                    g0 = g1

            def epi(st, h, ki):
                # epilogue for the finished half: A after ki=3, B after 7
                rr, rb, cph = st['rr'], st['rb'], st['cph']
                pr, hh = h // 2, h % 2
                half = 0 if ki == 3 else 1
                hs_ = slice(half * 512, half * 512 + 512)
                # reciprocal straight from the PSUM ones-row
                # (f32r is fp32-width; the gate only knows dtype != f32)
                with nc.allow_low_precision(reason="1/l in bf16 is accurate enough"):
                    nc.vector.reciprocal(rr[0:1, hs_], cph[half][64:65, 0:512])
                slot = mm.tile([128, 512], dt.float32, tag="aux",
                               name=f"rps{h}{half}_{b}")
                nc.tensor.matmul(slot[0:64, :], ones_b[:], rr[:, hs_],
                                 start=True, stop=True)
                nc.vector.tensor_copy(rb[:, hs_], slot[0:64, :])
                nc.vector.tensor_mul(
                    ctxt[(b, pr)][hh * 64:(hh + 1) * 64, hs_],
                    cph[half][0:64, 0:512], rb[:, hs_])
                if ki == 7 and hh == 1:
                    # head pair done: split ctx into fp8 hi+lo planes for the
                    # DoubleRow dense. Both SBUF-only ops go to the idle Pool
                    # engine; ACT (exp) and DVE (recip/epi) are saturated.
                    nc.gpsimd.tensor_copy(cxh[b][:, pr, :], ctxt[(b, pr)][:])
                    nc.gpsimd.tensor_sub(cxl[b][:, pr, :], ctxt[(b, pr)][:],
                                         cxh[b][:, pr, :])

            if b == 0:
                wdx = wd_pool.tile([128, 4, 8192], dt.float8e4, tag="wdx",
                                   name="wdx")
                wdx_t.append(wdx)
            prev = None
            for slot in range(9):
                if b == 0 and 1 <= slot <= 4:
                    # dense weights stream during phase B; emitting them
                    # inside the slot loop keeps them queued behind phase-B
                    # work so they don't steal DMA bandwidth from phase A
                    kq = slot - 1
                    nc.scalar.dma_start(wdx_t[0][:, kq, :],
                                        wdx_d[:, kq * 8192:(kq + 1) * 8192])
                cur = make_head(slot) if slot < 8 else None
                for ki in range(8):
                    if cur is not None:
                        score_ki(cur, slot, ki)
                    if prev is not None:
                        pv_ki(prev, slot - 1, ki)
                        if ki == 3 or ki == 7:
                            epi(prev, slot - 1, ki)
                    yield
                prev = cur

        def gen_dense(b):
            wdx = wdx_t[0]
            # b=1 runs after attention is done, so the score-rotation banks
            # are free for deeper accumulate/drain pipelining
            slots = ("qkv0", "qkv1", "qkv2") if b == 0 else (
                "qkv0", "qkv1", "qkv2", "qkv3", "aux", "cpsA")
            for mt in range(32):
                dsb = dout_pool.tile([128, 1024], dt.bfloat16, tag="dsb",
                                     name=f"dsb{mt}_{b}")
                for n2 in range(2):
                    dps = mm.tile([128, 512], dt.float32,
                                  tag=slots[(mt * 2 + n2) % len(slots)],
                                  name=f"d{mt}{n2}_{b}")
                    n2s = slice(n2 * 512, (n2 + 1) * 512)
                    for t in range(2):
                        ks = slice(2 * t, 2 * t + 2)
                        w_hi = wdx[:, ks, mt * 128:(mt + 1) * 128]
                        w_lo = wdx[:, ks, 4096 + mt * 128:4096 + (mt + 1) * 128]
                        nc.tensor.matmul(dps[:], w_hi, cxh[b][:, ks, n2s],
                                         start=(t == 0), stop=False, perf_mode=DR)
                        nc.tensor.matmul(dps[:], w_lo, cxh[b][:, ks, n2s],
                                         start=False, stop=False, perf_mode=DR)
                        nc.tensor.matmul(dps[:], w_hi, cxl[b][:, ks, n2s],
                                         start=False, stop=(t == 1), perf_mode=DR)
                        if b == 0:
                            # fine-grained quanta so dense matmuls pack into
                            # the gaps of attn(1)'s serial score->exp chain
                            yield
                    # phase C: ACT is exp-bound, keep all drains on DVE;
                    # phase D: ACT is idle, split by n2
                    if b == 1 and n2 == 0:
                        nc.scalar.mul(dsb[:, 0:512], dps[:], DRAIN_SCALE)
                    else:
                        nc.vector.tensor_scalar_mul(
                            dsb[:, n2 * 512:(n2 + 1) * 512], dps[:], DRAIN_SCALE)
                    if b == 1 and mt == 31:
                        # last tile: ship halves separately to shorten the
                        # final copy->DMA drain chain
                        nc.sync.dma_start(
                            outp_d[mt][:, b * 1024 + n2 * 512:
                                        b * 1024 + n2 * 512 + 512],
                            dsb[:, n2 * 512:(n2 + 1) * 512])
                    yield
                if not (b == 1 and mt == 31):
                    # SP queue is idle through C/D; keep Pool free for drains
                    nc.sync.dma_start(
                        outp_d[mt][:, b * 1024: b * 1024 + 1024], dsb[:])

        def _chain(*gens):
            for g in gens:
                yield from g

        _drive(gen_qkv(0))
        # merge phases B and C: as soon as qkv(1) finishes emitting, attn(1)
        # interleaves with attn(0)'s tail; when attn(0) ends, dense(0)
        # interleaves with attn(1)'s tail.
        _drive(_chain(gen_qkv(1), gen_attn(1)),
               _chain(gen_attn(0), gen_dense(0)))
        _drive(gen_dense(1))

    _CACHED_NC = nc
    return nc


def _split8(x):
    """x (f32) -> (hi, lo) fp8e4 with x ~= hi + lo."""
    hi = x.astype(f8)
    lo = (x - hi.astype(np.float32)).astype(f8)
    return hi, lo


def host_prep(hidden_states, alibi, attention_mask, W_qkv, W_dense):
    hsT = np.ascontiguousarray(hidden_states.reshape(NPOS, HID).T)  # [4096, 2048]
    hh, hl = _split8(hsT.astype(np.float32))
    # hsx[c][p][k*512+j] (hi) / 16384 + same (lo) = hsT[k*128+p, c*512+j]
    def _arr(x8):
        return np.ascontiguousarray(
            x8.reshape(32, 128, 4, 512).transpose(2, 1, 0, 3).reshape(4, 128, 16384))
    hsx = np.concatenate([_arr(hh), _arr(hl)], axis=2)  # [4, 128, 32768]

    j32 = np.arange(32)
    inv_freq = 1.0 / (10000.0 ** (2 * j32 / HD))
    t = np.arange(S, dtype=np.float64)
    fr = np.outer(inv_freq, t)                       # [32, S]
    cst = np.zeros((128, 2048), np.float32)
    cst[:, 0:1024] = np.tile(np.cos(fr), (4, 1))
    cst[:, 1024:2048] = np.tile(np.sin(fr), (4, 1))
    cst = cst.astype(bf16)

    # single causal diag block, [kpos, q] layout: 0 where kpos > q, else 1
    mf = np.where(attention_mask[0, 0, 0:128, 0:128], 0.0, 1.0).astype(np.float32)
    msk = np.ascontiguousarray(mf.T).astype(bf16)    # [kpos, q]

    al = alibi.reshape(B, NKV * G, S) * INV          # [B, 64, S]

    perm = []
    for i in range(4):
        perm += [i * 64 + d for d in range(32)]
    for i in range(4):
        perm += [i * 64 + 32 + d for d in range(32)]
    for i in range(4, 8):
        perm += [i * 64 + d for d in range(32)]
    for i in range(4, 8):
        perm += [i * 64 + 32 + d for d in range(32)]
    perm += [512 + d for d in range(64)] + [576 + d for d in range(64)]
    perm = np.array(perm)

    idn = np.eye(64, dtype=np.float32).astype(bf16)
    in_maps = []
    for c in range(NCORES):
        Wg = (W_qkv[c * 640:(c + 1) * 640][perm] * WS).astype(np.float32)
        WgT = np.ascontiguousarray(Wg.T)              # [4096, 640]
        wh, wl = _split8(WgT)
        wqh = np.ascontiguousarray(
            wh.reshape(32, 128, 640).transpose(1, 0, 2)).reshape(128, 32 * 640)
        wql = np.ascontiguousarray(
            wl.reshape(32, 128, 640).transpose(1, 0, 2)).reshape(128, 32 * 640)

        Wd = (W_dense[:, c * 512:(c + 1) * 512] * WS).astype(np.float32)
        WdT = np.ascontiguousarray(Wd.T)              # [512, 4096]
        dh, dl = _split8(WdT)
        wdx = np.concatenate(
            [dh.reshape(4, 128, 4096).transpose(1, 0, 2),
             dl.reshape(4, 128, 4096).transpose(1, 0, 2)], axis=2)  # [128,4,8192]
        wdx = np.ascontiguousarray(wdx).reshape(128, 4 * 8192)

        ab = np.zeros((128, 128), np.float32)
        for b in range(2):
            for ki in range(8):
                for h in range(8):
                    ab[:, b * 64 + ki * 8 + h] = al[b, c * 8 + h,
                                                    ki * 128:(ki + 1) * 128]
        in_maps.append({
            "hsx": hsx, "wqh": wqh, "wql": wql, "wdx": wdx, "cst": cst,
            "msk": msk, "ab": ab, "idn": idn,
        })
    return in_maps


def kernel(hidden_states, alibi, attention_mask, W_qkv, W_dense, _want_time=False):
    nc = build_program()
    in_maps = host_prep(np.asarray(hidden_states), np.asarray(alibi),
                        np.asarray(attention_mask), np.asarray(W_qkv),
                        np.asarray(W_dense))
    res = run_bass_kernel_spmd(nc, in_maps, list(range(NCORES)))
    acc = np.zeros((32, 128, NPOS), np.float32)
    for c in range(NCORES):
        acc += res.results[c]["outp"].astype(np.float32)
    out = acc.reshape(4096, NPOS).T.reshape(B, S, HID)
    if _want_time:
        return np.ascontiguousarray(out), res
    return np.ascontiguousarray(out)


# revision 81
# speedup vs baseline: 1.0738x; 1.0016x over previous
"""GQA attention block (dense_transformer) on 8 trn2 cores.

Sharding: tensor-parallel by kv-group. Core c owns kv-group c = 8 query
heads + 1 k + 1 v head (640 rows of W_qkv) and the matching 512 columns of
W_dense. hidden_states is replicated. Each core returns a bf16 partial
[4096, 2048] dense output; the host sums the 8 partials in f32.

v3: split-precision fp8 DoubleRow for the two big GEMMs. QKV and dense
weights/activations are decomposed host-side (and ctx on-chip) into
hi+lo fp8e4 planes; each K=256 pair runs as 3 DoubleRow matmuls
(hi*hi, lo-w*hi-x, hi-w*lo-x) at 0.5 cycles/col = 0.75x the bf16 PE
cost with bf16-level accuracy (dropped lo*lo term ~2^-8). Attention
internals (RoPE, scores, softmax, PV, epilogue) stay bf16. Scales:
W_qkv and W_dense x64 host-side (q,k,v 64x), exp activation scale
INV/4096, va ones column 4.0 (ctx 16x true), dense drains x2^-10.

Schedule: QKV holds only PSUM banks qkv0..2 (two m-sweeps for b=1, one
5-bank sweep in phase A) so attention permanently owns qkv3/qkv4/aux
score rotation + cpsA/cpsB PV accumulators. Attention heads are
software-pipelined (scores of head h emit interleaved with PV/epi of
head h-1) to hide the exp round-trip, and phases are fused by chaining
generators [qkv(1)->attn(1)] || [attn(0)->dense(0)] so a compute-dense
partner always fills the serial softmax chain's PE bubbles in the
in-order queues. Two heads share each 128-partition qp tile (kk is
duplicated to rows 64-127 so odd heads use base_partition 64). Engine
balance: exps+b1 drains on ACT, recip/epi/drains on DVE, masks and
ctx hi/lo splits on the otherwise idle Pool, output DMAs on SP.
Measured (TimelineSim): 305.6us vs 327.6us for the session baseline;
rel err 0.0035 (gate 2e-2). PE busy 261.7us (86% occupancy); the
residual idle is the DMA-bound phase-A cold start (~25us) and the
attention ramp at the B/C seam.
"""
import numpy as np
import ml_dtypes
from contextlib import ExitStack

import bass_rust
import concourse.bass as bass
import concourse.mybir as mybir
from concourse import tile
from concourse.bass_utils import run_bass_kernel_spmd

dt = mybir.dt
bf16 = ml_dtypes.bfloat16
f8 = ml_dtypes.float8_e4m3

B, S, HID = 2, 1024, 4096
NKV, G, HD = 8, 8, 64
NPOS = B * S
INV = 0.125
WS = 64.0                      # host-side weight scale (q,k,v come out 64x)
EXP_SCALE = INV / (WS * WS)    # PSUM scores are 4096x true
ONES_VAL = 4.0                 # va ones column -> ctx = 16x true
DRAIN_SCALE = 1.0 / 1024.0     # dense psum = 16 * 64 = 1024x true
NCORES = 8
DR = mybir.MatmulPerfMode.DoubleRow

# ---------------------------------------------------------------------------
# walrus in this container takes at most ONE sync-wait per instruction; Tile
# attaches several (tail drain especially). Split extras onto same-engine nops.
_orig_exit = tile.TileContext.__exit__


def _split_waits(nc):
    for bb in nc.m.functions[0].blocks:
        out, extra = [], 0
        for inst in bb.instructions:
            si = inst.sync_info
            if si is not None and len(si.on_wait) > 1:
                waits = list(si.on_wait)
                for w in waits[:-1]:
                    nop = mybir.InstNoOp(name=f"I-wsplit-{nc.next_id()}")
                    nop.engine = inst.engine
                    nop.sync_info = bass_rust.SyncInfo(on_wait=[w], on_update=[])
                    nc.register_instruction(nop, overwrite=True)
                    out.append(nop)
                    extra += 1
                inst.sync_info = bass_rust.SyncInfo(
                    on_wait=[waits[-1]], on_update=list(si.on_update)
                )
            out.append(inst)
        if extra:
            bb.instructions = out


def _patched_exit(self, exc_type, exc_val, exc_tb):
    r = _orig_exit(self, exc_type, exc_val, exc_tb)
    _split_waits(self.nc)
    return r


tile.TileContext.__exit__ = _patched_exit
# ---------------------------------------------------------------------------

_CACHED_NC = None


def _drive(*gens):
    live = list(gens)
    while live:
        for g in list(live):
            try:
                next(g)
            except StopIteration:
                live.remove(g)


def build_program():
    global _CACHED_NC
    if _CACHED_NC is not None:
        return _CACHED_NC
    nc = bass.Bass()
    # per chunk c (=b*2+n, 512 positions): [hi 32x512 | lo 32x512] fp8 planes
    hsx_d = nc.declare_dram_parameter("hsx", [4, 128, 32768], dt.float8e4, isOutput=False)
    # [p][k][640] hi and lo planes as separate params (hi loads first)
    wqh_d = nc.declare_dram_parameter("wqh", [128, 32 * 640], dt.float8e4, isOutput=False)
    wql_d = nc.declare_dram_parameter("wql", [128, 32 * 640], dt.float8e4, isOutput=False)
    # [p][kt][hi 4096 | lo 4096]
    wdx_d = nc.declare_dram_parameter("wdx", [128, 4 * 8192], dt.float8e4, isOutput=False)
    cst_d = nc.declare_dram_parameter("cst", [128, 2048], dt.bfloat16, isOutput=False)
    msk_d = nc.declare_dram_parameter("msk", [128, 128], dt.bfloat16, isOutput=False)
    ab_d = nc.declare_dram_parameter("ab", [128, 128], dt.float32, isOutput=False)
    idn_d = nc.declare_dram_parameter("idn", [64, 64], dt.bfloat16, isOutput=False)
    outp_d = nc.declare_dram_parameter("outp", [32, 128, NPOS], dt.bfloat16, isOutput=True)

    AF = mybir.ActivationFunctionType
    # packed causal offsets for et: block ki has width 1024-128*ki
    koff = [0] * 8
    for ki in range(1, 8):
        koff[ki] = koff[ki - 1] + (1024 - 128 * (ki - 1))
    ET_W = koff[7] + (1024 - 128 * 7)  # 4608

    with ExitStack() as ctx:
        tc = ctx.enter_context(tile.TileContext(nc))
        cpool = ctx.enter_context(tc.tile_pool(name="const", bufs=1))
        cst_sb = cpool.tile([128, 2048], dt.bfloat16)
        msk_sb = cpool.tile([128, 128], dt.bfloat16)
        ab_sb = cpool.tile([128, 128], dt.float32)
        idn_sb = cpool.tile([64, 64], dt.bfloat16)
        ones_b = cpool.tile([1, 64], dt.bfloat16)
        nc.vector.memset(ones_b[:], 1.0)

        hs_pool = ctx.enter_context(tc.tile_pool(name="hs", bufs=4))
        raw_pool = ctx.enter_context(tc.tile_pool(name="raw", bufs=2))
        tmp_pool = ctx.enter_context(tc.tile_pool(name="tmp", bufs=1))
        qp_pool = ctx.enter_context(tc.tile_pool(name="qp", bufs=2))
        kv_pool = ctx.enter_context(tc.tile_pool(name="kv", bufs=2))
        va_pool = ctx.enter_context(tc.tile_pool(name="va", bufs=2))
        et_pool = ctx.enter_context(tc.tile_pool(name="et", bufs=2))
        l_pool = ctx.enter_context(tc.tile_pool(name="l", bufs=1))
        rb_pool = ctx.enter_context(tc.tile_pool(name="rb", bufs=1))
        ctx_pool = ctx.enter_context(tc.tile_pool(name="ctx", bufs=2))
        cxl_pool = ctx.enter_context(tc.tile_pool(name="cxl", bufs=2))
        dout_pool = ctx.enter_context(tc.tile_pool(name="dout", bufs=3))
        wd_pool = ctx.enter_context(tc.tile_pool(name="wdp", bufs=1))
        wq_pool = ctx.enter_context(tc.tile_pool(name="wqp", bufs=1))
        mm = ctx.enter_context(tc.tile_pool(name="mm", bufs=1, space="PSUM"))

        # [128, k=32, hi|lo 1280] fp8
        # separate tiles per lazily-loaded slice: a shared tile would add
        # false write-after-read deps from in-flight matmuls to later loads
        wqh = [wq_pool.tile([128, 8, 640], dt.float8e4, tag=f"wqh{q}",
                            name=f"wqh{q}") for q in range(4)]
        wql = [wq_pool.tile([128, 16, 640], dt.float8e4, tag=f"wql{q}",
                            name=f"wql{q}") for q in range(2)]
        # small first slices unblock the first matmuls asap
        nc.sync.dma_start(wqh[0][:, 0:4, :], wqh_d[:, 0:4 * 640])
        nc.sync.dma_start(wqh[0][:, 4:8, :], wqh_d[:, 4 * 640:8 * 640])

        # per-batch SBUF state, filled by gen_qkv, read by gen_attn/gen_dense
        qp = {}   # (b, h) -> [64, 1024] bf16
        kk = {}   # b -> [64, 1024] bf16
        va = {}   # b -> [128, 8*72] bf16
        ctxt = {}  # (b, pr) -> [128, 1024] bf16
        cxh = {}  # b -> [128, 4, 1024] fp8 hi
        cxl = {}  # b -> [128, 4, 1024] fp8 lo
        wdx_t = []

        def gen_qkv(b):
            # two heads share one 128-partition tile (h even: rows 0-63,
            # h odd: rows 64-127); kk is duplicated into both halves so the
            # odd-head score matmuls use matching base_partition 64
            for pr in range(4):
                qph = qp_pool.tile([128, 1024], dt.bfloat16, tag=f"qph{pr}",
                                   name=f"qph{pr}_{b}")
                qp[(b, 2 * pr)] = qph[0:64, :]
                qp[(b, 2 * pr + 1)] = qph[64:128, :]
            kk[b] = kv_pool.tile([128, 1024], dt.bfloat16, tag="kk", name=f"kk{b}")
            vt = kv_pool.tile([64, 1024], dt.bfloat16, tag="vt", name=f"vt{b}")
            for n in range(2):
                c = b * 2 + n
                ncol = slice(n * 512, n * 512 + 512)
                # 4 hs tiles per chunk: hiH0 (k0-15), hiH1 (k16-31), loH0, loH1
                hst = {}

                def _load(part, engs):
                    # part: 0=hiH0 1=hiH1 2=loH0 3=loH1
                    t = hs_pool.tile([128, 16, 512], dt.float8e4, tag="hs",
                                     name=f"hs{part}_{c}")
                    if c == 0 and part == 0:
                        # split first load so the first matmul unblocks early
                        for g in range(4):
                            nc.gpsimd.dma_start(
                                t[:, 4 * g:4 * g + 4, :],
                                hsx_d[c][:, g * 2048:(g + 1) * 2048])
                    else:
                        engs.dma_start(t[:], hsx_d[c][:, part * 8192:(part + 1) * 8192])
                    hst[part] = t

                _load(0, nc.gpsimd)
                if c == 0:
                    # chunk 0 is DMA-bound: emit everything on ONE queue in
                    # exact demand order (M1: wqh+hs-hi, M2: wql, M3: hs-lo)
                    # so the shared DMA engines serve it in priority order.
                    nc.gpsimd.dma_start(wqh[1][:], wqh_d[:, 8 * 640:16 * 640])
                    nc.gpsimd.dma_start(wqh[2][:], wqh_d[:, 16 * 640:24 * 640])
                    nc.gpsimd.dma_start(wqh[3][:], wqh_d[:, 24 * 640:32 * 640])
                    _load(1, nc.gpsimd)
                    nc.gpsimd.dma_start(wql[0][:], wql_d[:, 0:16 * 640])
                    nc.gpsimd.dma_start(wql[1][:], wql_d[:, 16 * 640:32 * 640])
                    _load(2, nc.gpsimd)
                    _load(3, nc.gpsimd)
                    # small consts, needed from the RoPE/attention stages,
                    # ride the parallel HWDGE path
                    nc.scalar.dma_start(cst_sb[:], cst_d[:])
                    nc.scalar.dma_start(msk_sb[:], msk_d[:])
                    nc.scalar.dma_start(ab_sb[:], ab_d[:])
                    nc.scalar.dma_start(idn_sb[:], idn_d[:])
                else:
                    _load(1, nc.sync)
                raw = [raw_pool.tile([128, 512], dt.bfloat16, tag=f"raw{m}",
                                     name=f"raw{m}_{n}_{b}") for m in range(5)]
                # b=0 (phase A, attention not running): one 5-bank sweep.
                # b=1 (phase B): two sweeps (m 0-2, then 3-4) so QKV holds
                # only qkv0..2 and attention keeps qkv3/qkv4/aux+cpsA/cpsB.
                # Either way 3 passes (M1 hi*hi, M2 lo-w*hi-x, M3 hi-w*lo-x):
                # the cold start only needs hi weights + hi activations.
                tag5 = ("qkv0", "qkv1", "qkv2", "cpsA", "cpsB")
                sweeps = ((0, 1, 2, 3, 4),) if b == 0 else ((0, 1, 2), (3, 4))
                for ms in sweeps:
                    ps = {m: mm.tile([128, 512], dt.float32, tag=tag5[j],
                                     name=f"qkv{m}_{n}_{b}")
                          for j, m in enumerate(ms)}
                    for pa in range(3):
                        for kp in range(16):
                            if ms[0] == 0 and pa == 0 and c > 0:
                                if kp == 4:
                                    _load(2, nc.gpsimd)
                                if kp == 10:
                                    _load(3, nc.sync)
                            half, i = kp // 8, kp % 8
                            mv = hst[(2 if pa == 2 else 0) + half][:, 2 * i:2 * i + 2, :]
                            for m in ms:
                                if pa == 1:
                                    wsl = wql[kp // 8][:, 2 * (kp % 8):2 * (kp % 8) + 2,
                                              m * 128:(m + 1) * 128]
                                else:
                                    wsl = wqh[kp // 4][:, 2 * (kp % 4):2 * (kp % 4) + 2,
                                              m * 128:(m + 1) * 128]
                                nc.tensor.matmul(ps[m][:], wsl, mv,
                                                 start=(pa == 0 and kp == 0),
                                                 stop=(pa == 2 and kp == 15),
                                                 perf_mode=DR)
                                if pa == 2 and kp == 15:
                                    # drain while PE finishes the rest
                                    nc.scalar.copy(raw[m][:], ps[m][:])
                            if kp % 2 == 1:
                                yield
                Cs = cst_sb[:, n * 512:(n + 1) * 512]
                Ss = cst_sb[:, 1024 + n * 512: 1024 + (n + 1) * 512]
                for grp in range(2):
                    A, Bb = raw[grp * 2], raw[grp * 2 + 1]
                    P1 = tmp_pool.tile([128, 512], dt.bfloat16, tag="P1")
                    P2 = tmp_pool.tile([128, 512], dt.bfloat16, tag="P2")
                    P3 = tmp_pool.tile([128, 512], dt.bfloat16, tag="P3")
                    P4 = tmp_pool.tile([128, 512], dt.bfloat16, tag="P4")
                    nc.vector.tensor_mul(P1[:], A[:], Cs)
                    nc.vector.tensor_mul(P2[:], Bb[:], Ss)
                    nc.vector.tensor_mul(P3[:], Bb[:], Cs)
                    nc.vector.tensor_mul(P4[:], A[:], Ss)
                    for i in range(4):
                        h = grp * 4 + i
                        sl = slice(32 * i, 32 * i + 32)
                        nc.vector.tensor_sub(qp[(b, h)][0:32, ncol], P1[sl, :], P2[sl, :])
                        nc.vector.tensor_add(qp[(b, h)][32:64, ncol], P3[sl, :], P4[sl, :])
                kvr = raw[4]
                # reuse the P tiles' space for the k-RoPE temporaries
                pk1 = tmp_pool.tile([128, 512], dt.bfloat16, tag="P1",
                                    name=f"pk1_{n}_{b}")[0:32, :]
                pk2 = tmp_pool.tile([128, 512], dt.bfloat16, tag="P2",
                                    name=f"pk2_{n}_{b}")[0:32, :]
                pk3 = tmp_pool.tile([128, 512], dt.bfloat16, tag="P3",
                                    name=f"pk3_{n}_{b}")[0:32, :]
                pk4 = tmp_pool.tile([128, 512], dt.bfloat16, tag="P4",
                                    name=f"pk4_{n}_{b}")[0:32, :]
                nc.vector.tensor_mul(pk1[:], kvr[0:32, :], Cs[0:32, :])
                nc.vector.tensor_mul(pk2[:], kvr[32:64, :], Ss[32:64, :])
                nc.vector.tensor_mul(pk3[:], kvr[32:64, :], Cs[32:64, :])
                nc.vector.tensor_mul(pk4[:], kvr[0:32, :], Ss[0:32, :])
                nc.vector.tensor_sub(kk[b][0:32, ncol], pk1[:], pk2[:])
                nc.vector.tensor_add(kk[b][32:64, ncol], pk3[:], pk4[:])
                nc.vector.tensor_copy(vt[:, ncol], kvr[64:128, :])
                # duplicate k into rows 64-127 for the odd (base-64) heads
                nc.gpsimd.tensor_copy(kk[b][64:128, ncol], kk[b][0:64, ncol])
                yield
            # V transpose + ones column (borrows the aux PSUM bank)
            va[b] = va_pool.tile([128, 8 * 72], dt.bfloat16, tag="va", name=f"va{b}")
            for ki in range(8):
                slot = mm.tile([128, 512], dt.float32, tag="aux", name=f"vps{ki}_{b}")
                vps = slot[:, 0:32].bitcast(dt.bfloat16)
                nc.tensor.transpose(vps, vt[0:64, ki * 128:(ki + 1) * 128],
                                    idn_sb[:, :])
                nc.vector.tensor_copy(va[b][:, ki * 72: ki * 72 + 64], vps)
                nc.vector.memset(va[b][:, ki * 72 + 64: ki * 72 + 65], ONES_VAL)
            yield

        def gen_attn(b):
            for pr in range(4):
                ctxt[(b, pr)] = ctx_pool.tile([128, 1024], dt.bfloat16,
                                              tag=f"ctxt{pr}", name=f"ctxt{pr}_{b}")
            cxh[b] = ctx_pool.tile([128, 4, 1024], dt.float8e4, tag="cxh",
                                   name=f"cxh{b}")
            cxl[b] = cxl_pool.tile([128, 4, 1024], dt.float8e4, tag="cxl",
                                   name=f"cxl{b}")
            # Both batches: heads are software-pipelined (scores of head h
            # interleave with PV/epi of head h-1) so the exp round-trip is
            # hidden; the co-running generator (qkv(1) in phase B, dense(0)
            # in phase C) packs the remaining PE gaps. Scores rotate through
            # qkv3/qkv4/aux, PV owns cpsA/cpsB, qkv/dense use qkv0..2.
            rot = ("qkv3", "qkv4", "aux")

            def make_head(h):
                st = {}
                st['et'] = et_pool.tile([128, ET_W], dt.bfloat16, tag="et",
                                        name=f"et{h}_{b}")
                rrb = rb_pool.tile([128, 1024], dt.bfloat16, tag="rb",
                                   name=f"rrb{h}_{b}")
                st['rr'] = rrb[0:1, :]
                st['rb'] = rrb[64:128, :]
                st['cph'] = [
                    mm.tile([128, 512], dt.float32, tag="cpsA", name=f"cpA{h}_{b}"),
                    mm.tile([128, 512], dt.float32, tag="cpsB", name=f"cpB{h}_{b}")]
                st['ci'] = 0
                return st

            def score_ki(st, h, ki):
                et = st['et']
                base = ki * 128
                nchunks = (1024 - base + 511) // 512
                row0 = 64 * (h % 2)
                for cj in range(nchunks):
                    c0 = base + cj * 512
                    cw = min(512, 1024 - c0)
                    sc = mm.tile([128, 512], dt.float32,
                                 tag=rot[st['ci'] % len(rot)],
                                 name=f"sc{h}{ki}{cj}_{b}")
                    st['ci'] += 1
                    nc.tensor.matmul(
                        sc[:, 0:cw],
                        kk[b][row0:row0 + 64, base:base + 128],
                        qp[(b, h)][0:64, c0:c0 + cw],
                        start=True, stop=True,
                    )
                    abc = b * 64 + ki * 8 + h
                    nc.scalar.activation(
                        et[:, koff[ki] + (c0 - base): koff[ki] + (c0 - base) + cw],
                        sc[:, 0:cw], AF.Exp,
                        bias=ab_sb[:, abc:abc + 1], scale=EXP_SCALE)
                    if cj == 0:
                        # causal mask: zero the upper triangle of the diag
                        # block via a 0/1 multiply (off the sc->exp chain).
                        # Pool is otherwise idle and ACT/DVE are saturated.
                        nc.gpsimd.tensor_mul(
                            et[:, koff[ki]: koff[ki] + 128],
                            et[:, koff[ki]: koff[ki] + 128], msk_sb[:])

            def pv_ki(st, h, ki):
                et, cph = st['et'], st['cph']
                g0 = ki * 128
                while g0 < 1024:
                    half = g0 // 512
                    g1 = min(1024, (half + 1) * 512)
                    loc = slice(g0 - half * 512, g1 - half * 512)
                    nc.tensor.matmul(
                        cph[half][0:65, loc],
                        va[b][:, ki * 72: ki * 72 + 65],
                        et[:, koff[ki] + g0 - ki * 128: koff[ki] + g1 - ki * 128],
                        start=(ki == 0), stop=(ki == (3 if half == 0 else 7)),
                        skip_group_check=True,
                    )
                    g0 = g1

            def epi(st, h, ki):
                # epilogue for the finished half: A after ki=3, B after 7
                rr, rb, cph = st['rr'], st['rb'], st['cph']
                pr, hh = h // 2, h % 2
                half = 0 if ki == 3 else 1
                hs_ = slice(half * 512, half * 512 + 512)
                # reciprocal straight from the PSUM ones-row
                # (f32r is fp32-width; the gate only knows dtype != f32)
                with nc.allow_low_precision(reason="1/l in bf16 is accurate enough"):
                    nc.vector.reciprocal(rr[0:1, hs_], cph[half][64:65, 0:512])
                slot = mm.tile([128, 512], dt.float32, tag="aux",
                               name=f"rps{h}{half}_{b}")
                nc.tensor.matmul(slot[0:64, :], ones_b[:], rr[:, hs_],
                                 start=True, stop=True)
                nc.vector.tensor_copy(rb[:, hs_], slot[0:64, :])
                nc.vector.tensor_mul(
                    ctxt[(b, pr)][hh * 64:(hh + 1) * 64, hs_],
                    cph[half][0:64, 0:512], rb[:, hs_])
                if ki == 7 and hh == 1:
                    # head pair done: split ctx into fp8 hi+lo planes for the
                    # DoubleRow dense. Both SBUF-only ops go to the idle Pool
                    # engine; ACT (exp) and DVE (recip/epi) are saturated.
                    nc.gpsimd.tensor_copy(cxh[b][:, pr, :], ctxt[(b, pr)][:])
                    nc.gpsimd.tensor_sub(cxl[b][:, pr, :], ctxt[(b, pr)][:],
                                         cxh[b][:, pr, :])

            if b == 0:
                wdx = wd_pool.tile([128, 4, 8192], dt.float8e4, tag="wdx",
                                   name="wdx")
                wdx_t.append(wdx)
            prev = None
            for slot in range(9):
                if b == 0 and 1 <= slot <= 4:
                    # dense weights stream during phase B; emitting them
                    # inside the slot loop keeps them queued behind phase-B
                    # work so they don't steal DMA bandwidth from phase A
                    kq = slot - 1
                    nc.scalar.dma_start(wdx_t[0][:, kq, :],
                                        wdx_d[:, kq * 8192:(kq + 1) * 8192])
                cur = make_head(slot) if slot < 8 else None
                for ki in range(8):
                    if cur is not None:
                        score_ki(cur, slot, ki)
                    if prev is not None:
                        pv_ki(prev, slot - 1, ki)
                        if ki == 3 or ki == 7:
                            epi(prev, slot - 1, ki)
                    yield
                prev = cur

        def gen_dense(b):
            wdx = wdx_t[0]
            # b=1 runs after attention is done, so the score-rotation banks
            # are free for deeper accumulate/drain pipelining
            slots = ("qkv0", "qkv1", "qkv2") if b == 0 else (
                "qkv0", "qkv1", "qkv2", "qkv3", "aux", "cpsA")
            for mt in range(32):
                dsb = dout_pool.tile([128, 1024], dt.bfloat16, tag="dsb",
                                     name=f"dsb{mt}_{b}")
                for n2 in range(2):
                    dps = mm.tile([128, 512], dt.float32,
                                  tag=slots[(mt * 2 + n2) % len(slots)],
                                  name=f"d{mt}{n2}_{b}")
                    n2s = slice(n2 * 512, (n2 + 1) * 512)
                    for t in range(2):
                        ks = slice(2 * t, 2 * t + 2)
                        w_hi = wdx[:, ks, mt * 128:(mt + 1) * 128]
                        w_lo = wdx[:, ks, 4096 + mt * 128:4096 + (mt + 1) * 128]
                        nc.tensor.matmul(dps[:], w_hi, cxh[b][:, ks, n2s],
                                         start=(t == 0), stop=False, perf_mode=DR)
                        nc.tensor.matmul(dps[:], w_lo, cxh[b][:, ks, n2s],
                                         start=False, stop=False, perf_mode=DR)
                        nc.tensor.matmul(dps[:], w_hi, cxl[b][:, ks, n2s],
                                         start=False, stop=(t == 1), perf_mode=DR)
                        if b == 0:
                            # fine-grained quanta so dense matmuls pack into
                            # the gaps of attn(1)'s serial score->exp chain
                            yield
                    # phase C: ACT is exp-bound, keep drains on DVE until the
                    # exps dry up (~mt 24); phase D and late C use ACT too
                    if (b == 1 or mt >= 24) and n2 == 0:
                        nc.scalar.mul(dsb[:, 0:512], dps[:], DRAIN_SCALE)
                    else:
                        nc.vector.tensor_scalar_mul(
                            dsb[:, n2 * 512:(n2 + 1) * 512], dps[:], DRAIN_SCALE)
                    if b == 1 and mt == 31:
                        # last tile: ship halves separately to shorten the
                        # final copy->DMA drain chain
                        nc.sync.dma_start(
                            outp_d[mt][:, b * 1024 + n2 * 512:
                                        b * 1024 + n2 * 512 + 512],
                            dsb[:, n2 * 512:(n2 + 1) * 512])
                    yield
                if not (b == 1 and mt == 31):
                    # SP queue is idle through C/D; keep Pool free for drains
                    nc.sync.dma_start(
                        outp_d[mt][:, b * 1024: b * 1024 + 1024], dsb[:])

        def _chain(*gens):
            for g in gens:
                yield from g

        _drive(gen_qkv(0))
        # merge phases B and C: as soon as qkv(1) finishes emitting, attn(1)
        # interleaves with attn(0)'s tail; when attn(0) ends, dense(0)
        # interleaves with attn(1)'s tail.
        _drive(_chain(gen_qkv(1), gen_attn(1)),
               _chain(gen_attn(0), gen_dense(0)))
        _drive(gen_dense(1))

    _CACHED_NC = nc
    return nc


def _split8(x):
    """x (f32) -> (hi, lo) fp8e4 with x ~= hi + lo."""
    hi = x.astype(f8)
    lo = (x - hi.astype(np.float32)).astype(f8)
    return hi, lo


def host_prep(hidden_states, alibi, attention_mask, W_qkv, W_dense):
    hsT = np.ascontiguousarray(hidden_states.reshape(NPOS, HID).T)  # [4096, 2048]
    hh, hl = _split8(hsT.astype(np.float32))
    # hsx[c][p][k*512+j] (hi) / 16384 + same (lo) = hsT[k*128+p, c*512+j]
    def _arr(x8):
        return np.ascontiguousarray(
            x8.reshape(32, 128, 4, 512).transpose(2, 1, 0, 3).reshape(4, 128, 16384))
    hsx = np.concatenate([_arr(hh), _arr(hl)], axis=2)  # [4, 128, 32768]

    j32 = np.arange(32)
    inv_freq = 1.0 / (10000.0 ** (2 * j32 / HD))
    t = np.arange(S, dtype=np.float64)
    fr = np.outer(inv_freq, t)                       # [32, S]
    cst = np.zeros((128, 2048), np.float32)
    cst[:, 0:1024] = np.tile(np.cos(fr), (4, 1))
    cst[:, 1024:2048] = np.tile(np.sin(fr), (4, 1))
    cst = cst.astype(bf16)

    # single causal diag block, [kpos, q] layout: 0 where kpos > q, else 1
    mf = np.where(attention_mask[0, 0, 0:128, 0:128], 0.0, 1.0).astype(np.float32)
    msk = np.ascontiguousarray(mf.T).astype(bf16)    # [kpos, q]

    al = alibi.reshape(B, NKV * G, S) * INV          # [B, 64, S]

    perm = []
    for i in range(4):
        perm += [i * 64 + d for d in range(32)]
    for i in range(4):
        perm += [i * 64 + 32 + d for d in range(32)]
    for i in range(4, 8):
        perm += [i * 64 + d for d in range(32)]
    for i in range(4, 8):
        perm += [i * 64 + 32 + d for d in range(32)]
    perm += [512 + d for d in range(64)] + [576 + d for d in range(64)]
    perm = np.array(perm)

    idn = np.eye(64, dtype=np.float32).astype(bf16)
    in_maps = []
    for c in range(NCORES):
        Wg = (W_qkv[c * 640:(c + 1) * 640][perm] * WS).astype(np.float32)
        WgT = np.ascontiguousarray(Wg.T)              # [4096, 640]
        wh, wl = _split8(WgT)
        wqh = np.ascontiguousarray(
            wh.reshape(32, 128, 640).transpose(1, 0, 2)).reshape(128, 32 * 640)
        wql = np.ascontiguousarray(
            wl.reshape(32, 128, 640).transpose(1, 0, 2)).reshape(128, 32 * 640)

        Wd = (W_dense[:, c * 512:(c + 1) * 512] * WS).astype(np.float32)
        WdT = np.ascontiguousarray(Wd.T)              # [512, 4096]
        dh, dl = _split8(WdT)
        wdx = np.concatenate(
            [dh.reshape(4, 128, 4096).transpose(1, 0, 2),
             dl.reshape(4, 128, 4096).transpose(1, 0, 2)], axis=2)  # [128,4,8192]
        wdx = np.ascontiguousarray(wdx).reshape(128, 4 * 8192)

        ab = np.zeros((128, 128), np.float32)
        for b in range(2):
            for ki in range(8):
                for h in range(8):
                    ab[:, b * 64 + ki * 8 + h] = al[b, c * 8 + h,
                                                    ki * 128:(ki + 1) * 128]
        in_maps.append({
            "hsx": hsx, "wqh": wqh, "wql": wql, "wdx": wdx, "cst": cst,
            "msk": msk, "ab": ab, "idn": idn,
        })
    return in_maps


def kernel(hidden_states, alibi, attention_mask, W_qkv, W_dense, _want_time=False):
    nc = build_program()
    in_maps = host_prep(np.asarray(hidden_states), np.asarray(alibi),
                        np.asarray(attention_mask), np.asarray(W_qkv),
                        np.asarray(W_dense))
    res = run_bass_kernel_spmd(nc, in_maps, list(range(NCORES)))
    acc = np.zeros((32, 128, NPOS), np.float32)
    for c in range(NCORES):
        acc += res.results[c]["outp"].astype(np.float32)
    out = acc.reshape(4096, NPOS).T.reshape(B, S, HID)
    if _want_time:
        return np.ascontiguousarray(out), res
    return np.ascontiguousarray(out)


# revision 85
# speedup vs baseline: 1.0988x; 1.0233x over previous
"""GQA attention block (dense_transformer) on 8 trn2 cores.

Sharding: tensor-parallel by kv-group. Core c owns kv-group c = 8 query
heads + 1 k + 1 v head (640 rows of W_qkv) and the matching 512 columns of
W_dense. hidden_states is replicated. Each core returns a bf16 partial
[4096, 2048] dense output; the host sums the 8 partials in f32.

v3: split-precision fp8 DoubleRow for the two big GEMMs. QKV and dense
weights/activations are decomposed host-side (and ctx on-chip) into
hi+lo fp8e4 planes; each K=256 pair runs as 3 DoubleRow matmuls
(hi*hi, lo-w*hi-x, hi-w*lo-x) at 0.5 cycles/col = 0.75x the bf16 PE
cost with bf16-level accuracy (dropped lo*lo term ~2^-8). Attention
internals (RoPE, scores, softmax, PV, epilogue) stay bf16. Scales:
W_qkv and W_dense x64 host-side (q,k,v 64x), exp activation scale
INV/4096, va ones column 4.0 (ctx 16x true), dense drains x2^-10.

Schedule: QKV holds only PSUM banks qkv0..2 (two m-sweeps for b=1, one
5-bank sweep in phase A) so attention permanently owns qkv3/qkv4/aux
score rotation + cpsA/cpsB PV accumulators. Attention heads are
software-pipelined (scores of head h emit interleaved with PV/epi of
head h-1) to hide the exp round-trip, and phases are fused by chaining
generators [qkv(1)->attn(1)] || [attn(0)->dense(0)] so a compute-dense
partner always fills the serial softmax chain's PE bubbles in the
in-order queues. Two heads share each 128-partition qp tile (kk is
duplicated to rows 64-127 so odd heads use base_partition 64). Engine
balance: exps+b1 drains on ACT, recip/epi/drains on DVE, masks and
ctx hi/lo splits on the otherwise idle Pool, output DMAs on SP.
Chunk 0's loads are emitted on a single SWDGE queue in exact demand
order (M1: wqh+hs-hi, M2: wql, M3: hs-lo) since the cold start is
supply-limited on the shared DMA engines (~28us of transfers vs 25.6us
of compute). Measured (TimelineSim): 305.1us vs 327.6us for the
session baseline; rel err 0.0035 (gate 2e-2). PE busy 265.7us (87%
occupancy); the residual idle is the supply-limited phase-A cold start
(~19us floor) and the attention ramp at the B/C seam (~7us).
"""
import numpy as np
import ml_dtypes
from contextlib import ExitStack

import bass_rust
import concourse.bass as bass
import concourse.mybir as mybir
from concourse import tile
from concourse.bass_utils import run_bass_kernel_spmd

dt = mybir.dt
bf16 = ml_dtypes.bfloat16
f8 = ml_dtypes.float8_e4m3

B, S, HID = 2, 1024, 4096
NKV, G, HD = 8, 8, 64
NPOS = B * S
INV = 0.125
WS = 64.0                      # host-side weight scale (q,k,v come out 64x)
EXP_SCALE = INV / (WS * WS)    # PSUM scores are 4096x true
ONES_VAL = 4.0                 # va ones column -> ctx = 16x true
DRAIN_SCALE = 1.0 / 1024.0     # dense psum = 16 * 64 = 1024x true
NCORES = 8
DR = mybir.MatmulPerfMode.DoubleRow

# ---------------------------------------------------------------------------
# walrus in this container takes at most ONE sync-wait per instruction; Tile
# attaches several (tail drain especially). Split extras onto same-engine nops.
_orig_exit = tile.TileContext.__exit__


def _split_waits(nc):
    for bb in nc.m.functions[0].blocks:
        out, extra = [], 0
        for inst in bb.instructions:
            si = inst.sync_info
            if si is not None and len(si.on_wait) > 1:
                waits = list(si.on_wait)
                for w in waits[:-1]:
                    nop = mybir.InstNoOp(name=f"I-wsplit-{nc.next_id()}")
                    nop.engine = inst.engine
                    nop.sync_info = bass_rust.SyncInfo(on_wait=[w], on_update=[])
                    nc.register_instruction(nop, overwrite=True)
                    out.append(nop)
                    extra += 1
                inst.sync_info = bass_rust.SyncInfo(
                    on_wait=[waits[-1]], on_update=list(si.on_update)
                )
            out.append(inst)
        if extra:
            bb.instructions = out


def _patched_exit(self, exc_type, exc_val, exc_tb):
    r = _orig_exit(self, exc_type, exc_val, exc_tb)
    _split_waits(self.nc)
    return r


tile.TileContext.__exit__ = _patched_exit
# ---------------------------------------------------------------------------

_CACHED_NC = None


def _drive(*gens):
    live = list(gens)
    while live:
        for g in list(live):
            try:
                next(g)
            except StopIteration:
                live.remove(g)


def build_program():
    global _CACHED_NC
    if _CACHED_NC is not None:
        return _CACHED_NC
    nc = bass.Bass()
    # per chunk c (=b*2+n, 512 positions): [hi 32x512 | lo 32x512] fp8 planes
    hsx_d = nc.declare_dram_parameter("hsx", [4, 128, 32768], dt.float8e4, isOutput=False)
    # [p][k][640] hi and lo planes as separate params (hi loads first)
    wqh_d = nc.declare_dram_parameter("wqh", [128, 32 * 640], dt.float8e4, isOutput=False)
    wql_d = nc.declare_dram_parameter("wql", [128, 32 * 640], dt.float8e4, isOutput=False)
    # [p][kt][hi 4096 | lo 4096]
    wdx_d = nc.declare_dram_parameter("wdx", [128, 4 * 8192], dt.float8e4, isOutput=False)
    cst_d = nc.declare_dram_parameter("cst", [128, 2048], dt.bfloat16, isOutput=False)
    msk_d = nc.declare_dram_parameter("msk", [128, 128], dt.bfloat16, isOutput=False)
    ab_d = nc.declare_dram_parameter("ab", [128, 128], dt.float32, isOutput=False)
    idn_d = nc.declare_dram_parameter("idn", [64, 64], dt.bfloat16, isOutput=False)
    outp_d = nc.declare_dram_parameter("outp", [32, 128, NPOS], dt.bfloat16, isOutput=True)

    AF = mybir.ActivationFunctionType
    # packed causal offsets for et: block ki has width 1024-128*ki
    koff = [0] * 8
    for ki in range(1, 8):
        koff[ki] = koff[ki - 1] + (1024 - 128 * (ki - 1))
    ET_W = koff[7] + (1024 - 128 * 7)  # 4608

    with ExitStack() as ctx:
        tc = ctx.enter_context(tile.TileContext(nc))
        cpool = ctx.enter_context(tc.tile_pool(name="const", bufs=1))
        cst_sb = cpool.tile([128, 2048], dt.bfloat16)
        msk_sb = cpool.tile([128, 128], dt.bfloat16)
        ab_sb = cpool.tile([128, 128], dt.float32)
        idn_sb = cpool.tile([64, 64], dt.bfloat16)
        ones_b = cpool.tile([1, 64], dt.bfloat16)
        nc.vector.memset(ones_b[:], 1.0)

        hs_pool = ctx.enter_context(tc.tile_pool(name="hs", bufs=4))
        raw_pool = ctx.enter_context(tc.tile_pool(name="raw", bufs=2))
        tmp_pool = ctx.enter_context(tc.tile_pool(name="tmp", bufs=1))
        qp_pool = ctx.enter_context(tc.tile_pool(name="qp", bufs=2))
        kv_pool = ctx.enter_context(tc.tile_pool(name="kv", bufs=2))
        va_pool = ctx.enter_context(tc.tile_pool(name="va", bufs=2))
        et_pool = ctx.enter_context(tc.tile_pool(name="et", bufs=2))
        l_pool = ctx.enter_context(tc.tile_pool(name="l", bufs=1))
        rb_pool = ctx.enter_context(tc.tile_pool(name="rb", bufs=1))
        ctx_pool = ctx.enter_context(tc.tile_pool(name="ctx", bufs=2))
        cxl_pool = ctx.enter_context(tc.tile_pool(name="cxl", bufs=2))
        dout_pool = ctx.enter_context(tc.tile_pool(name="dout", bufs=3))
        wd_pool = ctx.enter_context(tc.tile_pool(name="wdp", bufs=1))
        wq_pool = ctx.enter_context(tc.tile_pool(name="wqp", bufs=1))
        mm = ctx.enter_context(tc.tile_pool(name="mm", bufs=1, space="PSUM"))

        # [128, k=32, hi|lo 1280] fp8
        # separate tiles per lazily-loaded slice: a shared tile would add
        # false write-after-read deps from in-flight matmuls to later loads
        wqh = [wq_pool.tile([128, 8, 640], dt.float8e4, tag=f"wqh{q}",
                            name=f"wqh{q}") for q in range(4)]
        wql = [wq_pool.tile([128, 16, 640], dt.float8e4, tag=f"wql{q}",
                            name=f"wql{q}") for q in range(2)]
        # small first slices unblock the first matmuls asap
        nc.sync.dma_start(wqh[0][:, 0:4, :], wqh_d[:, 0:4 * 640])
        nc.sync.dma_start(wqh[0][:, 4:8, :], wqh_d[:, 4 * 640:8 * 640])

        # per-batch SBUF state, filled by gen_qkv, read by gen_attn/gen_dense
        qp = {}   # (b, h) -> [64, 1024] bf16
        kk = {}   # b -> [64, 1024] bf16
        va = {}   # b -> [128, 8*72] bf16
        ctxt = {}  # (b, pr) -> [128, 1024] bf16
        cxh = {}  # b -> [128, 4, 1024] fp8 hi
        cxl = {}  # b -> [128, 4, 1024] fp8 lo
        wdx_t = []

        def gen_qkv(b):
            # two heads share one 128-partition tile (h even: rows 0-63,
            # h odd: rows 64-127); kk is duplicated into both halves so the
            # odd-head score matmuls use matching base_partition 64
            for pr in range(4):
                qph = qp_pool.tile([128, 1024], dt.bfloat16, tag=f"qph{pr}",
                                   name=f"qph{pr}_{b}")
                qp[(b, 2 * pr)] = qph[0:64, :]
                qp[(b, 2 * pr + 1)] = qph[64:128, :]
            kk[b] = kv_pool.tile([128, 1024], dt.bfloat16, tag="kk", name=f"kk{b}")
            vt = kv_pool.tile([64, 1024], dt.bfloat16, tag="vt", name=f"vt{b}")
            for n in range(2):
                c = b * 2 + n
                ncol = slice(n * 512, n * 512 + 512)
                # 4 hs tiles per chunk: hiH0 (k0-15), hiH1 (k16-31), loH0, loH1
                hst = {}

                def _load(part, engs):
                    # part: 0=hiH0 1=hiH1 2=loH0 3=loH1
                    t = hs_pool.tile([128, 16, 512], dt.float8e4, tag="hs",
                                     name=f"hs{part}_{c}")
                    if c == 0 and part == 0:
                        # split first load so the first matmul unblocks early
                        for g in range(4):
                            nc.gpsimd.dma_start(
                                t[:, 4 * g:4 * g + 4, :],
                                hsx_d[c][:, g * 2048:(g + 1) * 2048])
                    else:
                        engs.dma_start(t[:], hsx_d[c][:, part * 8192:(part + 1) * 8192])
                    hst[part] = t

                _load(0, nc.gpsimd)
                if c == 0:
                    # chunk 0 is DMA-bound: emit everything on ONE queue in
                    # exact demand order (M1: wqh+hs-hi, M2: wql, M3: hs-lo)
                    # so the shared DMA engines serve it in priority order.
                    nc.gpsimd.dma_start(wqh[1][:], wqh_d[:, 8 * 640:16 * 640])
                    nc.gpsimd.dma_start(wqh[2][:], wqh_d[:, 16 * 640:24 * 640])
                    nc.gpsimd.dma_start(wqh[3][:], wqh_d[:, 24 * 640:32 * 640])
                    _load(1, nc.gpsimd)
                    nc.gpsimd.dma_start(wql[0][:], wql_d[:, 0:16 * 640])
                    nc.gpsimd.dma_start(wql[1][:], wql_d[:, 16 * 640:32 * 640])
                    _load(2, nc.gpsimd)
                    _load(3, nc.gpsimd)
                    # small consts (first needed by RoPE at ~30us) go LAST in
                    # the demand-ordered stream
                    nc.gpsimd.dma_start(cst_sb[:], cst_d[:])
                    nc.gpsimd.dma_start(msk_sb[:], msk_d[:])
                    nc.gpsimd.dma_start(ab_sb[:], ab_d[:])
                    nc.gpsimd.dma_start(idn_sb[:], idn_d[:])
                else:
                    _load(1, nc.sync)
                raw = [raw_pool.tile([128, 512], dt.bfloat16, tag=f"raw{m}",
                                     name=f"raw{m}_{n}_{b}") for m in range(5)]
                # b=0 (phase A, attention not running): one 5-bank sweep.
                # b=1 (phase B): two sweeps (m 0-2, then 3-4) so QKV holds
                # only qkv0..2 and attention keeps qkv3/qkv4/aux+cpsA/cpsB.
                # Either way 3 passes (M1 hi*hi, M2 lo-w*hi-x, M3 hi-w*lo-x):
                # the cold start only needs hi weights + hi activations.
                tag5 = ("qkv0", "qkv1", "qkv2", "cpsA", "cpsB")
                sweeps = ((0, 1, 2, 3, 4),) if b == 0 else ((0, 1, 2), (3, 4))
                for ms in sweeps:
                    ps = {m: mm.tile([128, 512], dt.float32, tag=tag5[j],
                                     name=f"qkv{m}_{n}_{b}")
                          for j, m in enumerate(ms)}
                    for pa in range(3):
                        for kp in range(16):
                            if ms[0] == 0 and pa == 0 and c > 0:
                                if kp == 4:
                                    _load(2, nc.gpsimd)
                                if kp == 10:
                                    _load(3, nc.sync)
                            half, i = kp // 8, kp % 8
                            mv = hst[(2 if pa == 2 else 0) + half][:, 2 * i:2 * i + 2, :]
                            for m in ms:
                                if pa == 1:
                                    wsl = wql[kp // 8][:, 2 * (kp % 8):2 * (kp % 8) + 2,
                                              m * 128:(m + 1) * 128]
                                else:
                                    wsl = wqh[kp // 4][:, 2 * (kp % 4):2 * (kp % 4) + 2,
                                              m * 128:(m + 1) * 128]
                                nc.tensor.matmul(ps[m][:], wsl, mv,
                                                 start=(pa == 0 and kp == 0),
                                                 stop=(pa == 2 and kp == 15),
                                                 perf_mode=DR)
                                if pa == 2 and kp == 15:
                                    # drain while PE finishes the rest
                                    nc.scalar.copy(raw[m][:], ps[m][:])
                            if kp % 2 == 1:
                                yield
                Cs = cst_sb[:, n * 512:(n + 1) * 512]
                Ss = cst_sb[:, 1024 + n * 512: 1024 + (n + 1) * 512]
                for grp in range(2):
                    A, Bb = raw[grp * 2], raw[grp * 2 + 1]
                    P1 = tmp_pool.tile([128, 512], dt.bfloat16, tag="P1")
                    P2 = tmp_pool.tile([128, 512], dt.bfloat16, tag="P2")
                    P3 = tmp_pool.tile([128, 512], dt.bfloat16, tag="P3")
                    P4 = tmp_pool.tile([128, 512], dt.bfloat16, tag="P4")
                    nc.vector.tensor_mul(P1[:], A[:], Cs)
                    nc.vector.tensor_mul(P2[:], Bb[:], Ss)
                    nc.vector.tensor_mul(P3[:], Bb[:], Cs)
                    nc.vector.tensor_mul(P4[:], A[:], Ss)
                    for i in range(4):
                        h = grp * 4 + i
                        sl = slice(32 * i, 32 * i + 32)
                        nc.vector.tensor_sub(qp[(b, h)][0:32, ncol], P1[sl, :], P2[sl, :])
                        nc.vector.tensor_add(qp[(b, h)][32:64, ncol], P3[sl, :], P4[sl, :])
                kvr = raw[4]
                # reuse the P tiles' space for the k-RoPE temporaries
                pk1 = tmp_pool.tile([128, 512], dt.bfloat16, tag="P1",
                                    name=f"pk1_{n}_{b}")[0:32, :]
                pk2 = tmp_pool.tile([128, 512], dt.bfloat16, tag="P2",
                                    name=f"pk2_{n}_{b}")[0:32, :]
                pk3 = tmp_pool.tile([128, 512], dt.bfloat16, tag="P3",
                                    name=f"pk3_{n}_{b}")[0:32, :]
                pk4 = tmp_pool.tile([128, 512], dt.bfloat16, tag="P4",
                                    name=f"pk4_{n}_{b}")[0:32, :]
                nc.vector.tensor_mul(pk1[:], kvr[0:32, :], Cs[0:32, :])
                nc.vector.tensor_mul(pk2[:], kvr[32:64, :], Ss[32:64, :])
                nc.vector.tensor_mul(pk3[:], kvr[32:64, :], Cs[32:64, :])
                nc.vector.tensor_mul(pk4[:], kvr[0:32, :], Ss[0:32, :])
                nc.vector.tensor_sub(kk[b][0:32, ncol], pk1[:], pk2[:])
                nc.vector.tensor_add(kk[b][32:64, ncol], pk3[:], pk4[:])
                nc.vector.tensor_copy(vt[:, ncol], kvr[64:128, :])
                # duplicate k into rows 64-127 for the odd (base-64) heads
                nc.gpsimd.tensor_copy(kk[b][64:128, ncol], kk[b][0:64, ncol])
                yield
            # V transpose + ones column (borrows the aux PSUM bank)
            va[b] = va_pool.tile([128, 8 * 72], dt.bfloat16, tag="va", name=f"va{b}")
            for ki in range(8):
                slot = mm.tile([128, 512], dt.float32, tag="aux", name=f"vps{ki}_{b}")
                vps = slot[:, 0:32].bitcast(dt.bfloat16)
                nc.tensor.transpose(vps, vt[0:64, ki * 128:(ki + 1) * 128],
                                    idn_sb[:, :])
                nc.vector.tensor_copy(va[b][:, ki * 72: ki * 72 + 64], vps)
                nc.vector.memset(va[b][:, ki * 72 + 64: ki * 72 + 65], ONES_VAL)
            yield

        def gen_attn(b):
            for pr in range(4):
                ctxt[(b, pr)] = ctx_pool.tile([128, 1024], dt.bfloat16,
                                              tag=f"ctxt{pr}", name=f"ctxt{pr}_{b}")
            cxh[b] = ctx_pool.tile([128, 4, 1024], dt.float8e4, tag="cxh",
                                   name=f"cxh{b}")
            cxl[b] = cxl_pool.tile([128, 4, 1024], dt.float8e4, tag="cxl",
                                   name=f"cxl{b}")
            # Both batches: heads are software-pipelined (scores of head h
            # interleave with PV/epi of head h-1) so the exp round-trip is
            # hidden; the co-running generator (qkv(1) in phase B, dense(0)
            # in phase C) packs the remaining PE gaps. Scores rotate through
            # qkv3/qkv4/aux, PV owns cpsA/cpsB, qkv/dense use qkv0..2.
            rot = ("qkv3", "qkv4", "aux")

            def make_head(h):
                st = {}
                st['et'] = et_pool.tile([128, ET_W], dt.bfloat16, tag="et",
                                        name=f"et{h}_{b}")
                rrb = rb_pool.tile([128, 1024], dt.bfloat16, tag="rb",
                                   name=f"rrb{h}_{b}")
                st['rr'] = rrb[0:1, :]
                st['rb'] = rrb[64:128, :]
                st['cph'] = [
                    mm.tile([128, 512], dt.float32, tag="cpsA", name=f"cpA{h}_{b}"),
                    mm.tile([128, 512], dt.float32, tag="cpsB", name=f"cpB{h}_{b}")]
                st['ci'] = 0
                return st

            def score_ki(st, h, ki):
                et = st['et']
                base = ki * 128
                nchunks = (1024 - base + 511) // 512
                row0 = 64 * (h % 2)
                for cj in range(nchunks):
                    c0 = base + cj * 512
                    cw = min(512, 1024 - c0)
                    sc = mm.tile([128, 512], dt.float32,
                                 tag=rot[st['ci'] % len(rot)],
                                 name=f"sc{h}{ki}{cj}_{b}")
                    st['ci'] += 1
                    nc.tensor.matmul(
                        sc[:, 0:cw],
                        kk[b][row0:row0 + 64, base:base + 128],
                        qp[(b, h)][0:64, c0:c0 + cw],
                        start=True, stop=True,
                    )
                    abc = b * 64 + ki * 8 + h
                    nc.scalar.activation(
                        et[:, koff[ki] + (c0 - base): koff[ki] + (c0 - base) + cw],
                        sc[:, 0:cw], AF.Exp,
                        bias=ab_sb[:, abc:abc + 1], scale=EXP_SCALE)
                    if cj == 0:
                        # causal mask: zero the upper triangle of the diag
                        # block via a 0/1 multiply (off the sc->exp chain).
                        # Pool is otherwise idle and ACT/DVE are saturated.
                        nc.gpsimd.tensor_mul(
                            et[:, koff[ki]: koff[ki] + 128],
                            et[:, koff[ki]: koff[ki] + 128], msk_sb[:])

            def pv_ki(st, h, ki):
                et, cph = st['et'], st['cph']
                g0 = ki * 128
                while g0 < 1024:
                    half = g0 // 512
                    g1 = min(1024, (half + 1) * 512)
                    loc = slice(g0 - half * 512, g1 - half * 512)
                    nc.tensor.matmul(
                        cph[half][0:65, loc],
                        va[b][:, ki * 72: ki * 72 + 65],
                        et[:, koff[ki] + g0 - ki * 128: koff[ki] + g1 - ki * 128],
                        start=(ki == 0), stop=(ki == (3 if half == 0 else 7)),
                        skip_group_check=True,
                    )
                    g0 = g1

            def epi(st, h, ki):
                # epilogue for the finished half: A after ki=3, B after 7
                rr, rb, cph = st['rr'], st['rb'], st['cph']
                pr, hh = h // 2, h % 2
                half = 0 if ki == 3 else 1
                hs_ = slice(half * 512, half * 512 + 512)
                # reciprocal straight from the PSUM ones-row
                # (f32r is fp32-width; the gate only knows dtype != f32)
                with nc.allow_low_precision(reason="1/l in bf16 is accurate enough"):
                    nc.vector.reciprocal(rr[0:1, hs_], cph[half][64:65, 0:512])
                slot = mm.tile([128, 512], dt.float32, tag="aux",
                               name=f"rps{h}{half}_{b}")
                nc.tensor.matmul(slot[0:64, :], ones_b[:], rr[:, hs_],
                                 start=True, stop=True)
                nc.vector.tensor_copy(rb[:, hs_], slot[0:64, :])
                nc.vector.tensor_mul(
                    ctxt[(b, pr)][hh * 64:(hh + 1) * 64, hs_],
                    cph[half][0:64, 0:512], rb[:, hs_])
                if ki == 7 and hh == 1:
                    # head pair done: split ctx into fp8 hi+lo planes for the
                    # DoubleRow dense. Both SBUF-only ops go to the idle Pool
                    # engine; ACT (exp) and DVE (recip/epi) are saturated.
                    nc.gpsimd.tensor_copy(cxh[b][:, pr, :], ctxt[(b, pr)][:])
                    nc.gpsimd.tensor_sub(cxl[b][:, pr, :], ctxt[(b, pr)][:],
                                         cxh[b][:, pr, :])

            if b == 0:
                wdx = wd_pool.tile([128, 4, 8192], dt.float8e4, tag="wdx",
                                   name="wdx")
                wdx_t.append(wdx)
            prev = None
            for slot in range(9):
                if b == 0 and 1 <= slot <= 4:
                    # dense weights stream during phase B; emitting them
                    # inside the slot loop keeps them queued behind phase-B
                    # work so they don't steal DMA bandwidth from phase A
                    kq = slot - 1
                    nc.scalar.dma_start(wdx_t[0][:, kq, :],
                                        wdx_d[:, kq * 8192:(kq + 1) * 8192])
                cur = make_head(slot) if slot < 8 else None
                for ki in range(8):
                    if cur is not None:
                        score_ki(cur, slot, ki)
                    if prev is not None:
                        pv_ki(prev, slot - 1, ki)
                        if ki == 3 or ki == 7:
                            epi(prev, slot - 1, ki)
                    yield
                prev = cur

        def gen_dense(b):
            wdx = wdx_t[0]
            # b=1 runs after attention is done, so the score-rotation banks
            # are free for deeper accumulate/drain pipelining
            slots = ("qkv0", "qkv1", "qkv2") if b == 0 else (
                "qkv0", "qkv1", "qkv2", "qkv3", "aux", "cpsA")
            for mt in range(32):
                dsb = dout_pool.tile([128, 1024], dt.bfloat16, tag="dsb",
                                     name=f"dsb{mt}_{b}")
                for n2 in range(2):
                    dps = mm.tile([128, 512], dt.float32,
                                  tag=slots[(mt * 2 + n2) % len(slots)],
                                  name=f"d{mt}{n2}_{b}")
                    n2s = slice(n2 * 512, (n2 + 1) * 512)
                    for t in range(2):
                        ks = slice(2 * t, 2 * t + 2)
                        w_hi = wdx[:, ks, mt * 128:(mt + 1) * 128]
                        w_lo = wdx[:, ks, 4096 + mt * 128:4096 + (mt + 1) * 128]
                        nc.tensor.matmul(dps[:], w_hi, cxh[b][:, ks, n2s],
                                         start=(t == 0), stop=False, perf_mode=DR)
                        nc.tensor.matmul(dps[:], w_lo, cxh[b][:, ks, n2s],
                                         start=False, stop=False, perf_mode=DR)
                        nc.tensor.matmul(dps[:], w_hi, cxl[b][:, ks, n2s],
                                         start=False, stop=(t == 1), perf_mode=DR)
                        if b == 0:
                            # fine-grained quanta so dense matmuls pack into
                            # the gaps of attn(1)'s serial score->exp chain
                            yield
                    # phase C: ACT is exp-bound, keep drains on DVE until the
                    # exps dry up (~mt 24); phase D and late C use ACT too
                    if (b == 1 or mt >= 24) and n2 == 0:
                        nc.scalar.mul(dsb[:, 0:512], dps[:], DRAIN_SCALE)
                    else:
                        nc.vector.tensor_scalar_mul(
                            dsb[:, n2 * 512:(n2 + 1) * 512], dps[:], DRAIN_SCALE)
                    if b == 1 and mt == 31:
                        # last tile: ship halves separately to shorten the
                        # final copy->DMA drain chain
                        nc.sync.dma_start(
                            outp_d[mt][:, b * 1024 + n2 * 512:
                                        b * 1024 + n2 * 512 + 512],
                            dsb[:, n2 * 512:(n2 + 1) * 512])
                    yield
                if not (b == 1 and mt == 31):
                    # SP queue is idle through C/D; keep Pool free for drains
                    nc.sync.dma_start(
                        outp_d[mt][:, b * 1024: b * 1024 + 1024], dsb[:])

        def _chain(*gens):
            for g in gens:
                yield from g

        _drive(gen_qkv(0))
        # merge phases B and C: as soon as qkv(1) finishes emitting, attn(1)
        # interleaves with attn(0)'s tail; when attn(0) ends, dense(0)
        # interleaves with attn(1)'s tail.
        _drive(_chain(gen_qkv(1), gen_attn(1)),
               _chain(gen_attn(0), gen_dense(0)))
        _drive(gen_dense(1))

    _CACHED_NC = nc
    return nc


def _split8(x):
    """x (f32) -> (hi, lo) fp8e4 with x ~= hi + lo."""
    hi = x.astype(f8)
    lo = (x - hi.astype(np.float32)).astype(f8)
    return hi, lo


def host_prep(hidden_states, alibi, attention_mask, W_qkv, W_dense):
    hsT = np.ascontiguousarray(hidden_states.reshape(NPOS, HID).T)  # [4096, 2048]
    hh, hl = _split8(hsT.astype(np.float32))
    # hsx[c][p][k*512+j] (hi) / 16384 + same (lo) = hsT[k*128+p, c*512+j]
    def _arr(x8):
        return np.ascontiguousarray(
            x8.reshape(32, 128, 4, 512).transpose(2, 1, 0, 3).reshape(4, 128, 16384))
    hsx = np.concatenate([_arr(hh), _arr(hl)], axis=2)  # [4, 128, 32768]

    j32 = np.arange(32)
    inv_freq = 1.0 / (10000.0 ** (2 * j32 / HD))
    t = np.arange(S, dtype=np.float64)
    fr = np.outer(inv_freq, t)                       # [32, S]
    cst = np.zeros((128, 2048), np.float32)
    cst[:, 0:1024] = np.tile(np.cos(fr), (4, 1))
    cst[:, 1024:2048] = np.tile(np.sin(fr), (4, 1))
    cst = cst.astype(bf16)

    # single causal diag block, [kpos, q] layout: 0 where kpos > q, else 1
    mf = np.where(attention_mask[0, 0, 0:128, 0:128], 0.0, 1.0).astype(np.float32)
    msk = np.ascontiguousarray(mf.T).astype(bf16)    # [kpos, q]

    al = alibi.reshape(B, NKV * G, S) * INV          # [B, 64, S]

    perm = []
    for i in range(4):
        perm += [i * 64 + d for d in range(32)]
    for i in range(4):
        perm += [i * 64 + 32 + d for d in range(32)]
    for i in range(4, 8):
        perm += [i * 64 + d for d in range(32)]
    for i in range(4, 8):
        perm += [i * 64 + 32 + d for d in range(32)]
    perm += [512 + d for d in range(64)] + [576 + d for d in range(64)]
    perm = np.array(perm)

    idn = np.eye(64, dtype=np.float32).astype(bf16)
    in_maps = []
    for c in range(NCORES):
        Wg = (W_qkv[c * 640:(c + 1) * 640][perm] * WS).astype(np.float32)
        WgT = np.ascontiguousarray(Wg.T)              # [4096, 640]
        wh, wl = _split8(WgT)
        wqh = np.ascontiguousarray(
            wh.reshape(32, 128, 640).transpose(1, 0, 2)).reshape(128, 32 * 640)
        wql = np.ascontiguousarray(
            wl.reshape(32, 128, 640).transpose(1, 0, 2)).reshape(128, 32 * 640)

        Wd = (W_dense[:, c * 512:(c + 1) * 512] * WS).astype(np.float32)
        WdT = np.ascontiguousarray(Wd.T)              # [512, 4096]
        dh, dl = _split8(WdT)
        wdx = np.concatenate(
            [dh.reshape(4, 128, 4096).transpose(1, 0, 2),
             dl.reshape(4, 128, 4096).transpose(1, 0, 2)], axis=2)  # [128,4,8192]
        wdx = np.ascontiguousarray(wdx).reshape(128, 4 * 8192)

        ab = np.zeros((128, 128), np.float32)
        for b in range(2):
            for ki in range(8):
                for h in range(8):
                    ab[:, b * 64 + ki * 8 + h] = al[b, c * 8 + h,
                                                    ki * 128:(ki + 1) * 128]
        in_maps.append({
            "hsx": hsx, "wqh": wqh, "wql": wql, "wdx": wdx, "cst": cst,
            "msk": msk, "ab": ab, "idn": idn,
        })
    return in_maps


def kernel(hidden_states, alibi, attention_mask, W_qkv, W_dense, _want_time=False):
    nc = build_program()
    in_maps = host_prep(np.asarray(hidden_states), np.asarray(alibi),
                        np.asarray(attention_mask), np.asarray(W_qkv),
                        np.asarray(W_dense))
    res = run_bass_kernel_spmd(nc, in_maps, list(range(NCORES)))
    acc = np.zeros((32, 128, NPOS), np.float32)
    for c in range(NCORES):
        acc += res.results[c]["outp"].astype(np.float32)
    out = acc.reshape(4096, NPOS).T.reshape(B, S, HID)
    if _want_time:
        return np.ascontiguousarray(out), res
    return np.ascontiguousarray(out)


# revision 89
# speedup vs baseline: 1.1096x; 1.0098x over previous
"""GQA attention block (dense_transformer) on 8 trn2 cores.

Sharding: tensor-parallel by kv-group. Core c owns kv-group c = 8 query
heads + 1 k + 1 v head (640 rows of W_qkv) and the matching 512 columns of
W_dense. hidden_states is replicated. Each core returns a bf16 partial
[4096, 2048] dense output; the host sums the 8 partials in f32.

v3: split-precision fp8 DoubleRow for the two big GEMMs. QKV and dense
weights/activations are decomposed host-side (and ctx on-chip) into
hi+lo fp8e4 planes; each K=256 pair runs as 3 DoubleRow matmuls
(hi*hi, lo-w*hi-x, hi-w*lo-x) at 0.5 cycles/col = 0.75x the bf16 PE
cost with bf16-level accuracy (dropped lo*lo term ~2^-8). Attention
internals (RoPE, scores, softmax, PV, epilogue) stay bf16. Scales:
W_qkv and W_dense x64 host-side (q,k,v 64x), exp activation scale
INV/4096, va ones column 4.0 (ctx 16x true), dense drains x2^-10.

Schedule: QKV holds only PSUM banks qkv0..2 (two m-sweeps for b=1, one
5-bank sweep in phase A) so attention permanently owns qkv3/qkv4/aux
score rotation + cpsA/cpsB PV accumulators. Attention heads are
software-pipelined (scores of head h emit interleaved with PV/epi of
head h-1) to hide the exp round-trip, and phases are fused by chaining
generators [qkv(1)->attn(1)] || [attn(0)->dense(0)] so a compute-dense
partner always fills the serial softmax chain's PE bubbles in the
in-order queues. Two heads share each 128-partition qp tile (kk is
duplicated to rows 64-127 so odd heads use base_partition 64). Engine
balance: exps+b1 drains on ACT, recip/epi/drains on DVE, masks and
ctx hi/lo splits on the otherwise idle Pool, output DMAs on SP.
Chunk 0's loads are emitted on a single SWDGE queue in exact demand
order (M1: wqh+hs-hi, M2: wql, M3: hs-lo, then consts) since the cold
start is supply-limited on the shared DMA engines (~28us of transfers
vs 25.6us of compute); even the small consts matter there. Measured
(TimelineSim): 298.2us vs 327.6us for the session baseline; rel err
0.0035 (gate 2e-2). PE busy 265.7us (89% occupancy); the residual
idle is the supply-limited phase-A cold start and the attention ramp
at the B/C seam.
"""
import numpy as np
import ml_dtypes
from contextlib import ExitStack

import bass_rust
import concourse.bass as bass
import concourse.mybir as mybir
from concourse import tile
from concourse.bass_utils import run_bass_kernel_spmd

dt = mybir.dt
bf16 = ml_dtypes.bfloat16
f8 = ml_dtypes.float8_e4m3

B, S, HID = 2, 1024, 4096
NKV, G, HD = 8, 8, 64
NPOS = B * S
INV = 0.125
WS = 64.0                      # host-side weight scale (q,k,v come out 64x)
EXP_SCALE = INV / (WS * WS)    # PSUM scores are 4096x true
ONES_VAL = 4.0                 # va ones column -> ctx = 16x true
DRAIN_SCALE = 1.0 / 1024.0     # dense psum = 16 * 64 = 1024x true
NCORES = 8
DR = mybir.MatmulPerfMode.DoubleRow

# ---------------------------------------------------------------------------
# walrus in this container takes at most ONE sync-wait per instruction; Tile
# attaches several (tail drain especially). Split extras onto same-engine nops.
_orig_exit = tile.TileContext.__exit__


def _split_waits(nc):
    for bb in nc.m.functions[0].blocks:
        out, extra = [], 0
        for inst in bb.instructions:
            si = inst.sync_info
            if si is not None and len(si.on_wait) > 1:
                waits = list(si.on_wait)
                for w in waits[:-1]:
                    nop = mybir.InstNoOp(name=f"I-wsplit-{nc.next_id()}")
                    nop.engine = inst.engine
                    nop.sync_info = bass_rust.SyncInfo(on_wait=[w], on_update=[])
                    nc.register_instruction(nop, overwrite=True)
                    out.append(nop)
                    extra += 1
                inst.sync_info = bass_rust.SyncInfo(
                    on_wait=[waits[-1]], on_update=list(si.on_update)
                )
            out.append(inst)
        if extra:
            bb.instructions = out


def _patched_exit(self, exc_type, exc_val, exc_tb):
    r = _orig_exit(self, exc_type, exc_val, exc_tb)
    _split_waits(self.nc)
    return r


tile.TileContext.__exit__ = _patched_exit
# ---------------------------------------------------------------------------

_CACHED_NC = None


def _drive(*gens):
    live = list(gens)
    while live:
        for g in list(live):
            try:
                next(g)
            except StopIteration:
                live.remove(g)


def build_program():
    global _CACHED_NC
    if _CACHED_NC is not None:
        return _CACHED_NC
    nc = bass.Bass()
    # per chunk c (=b*2+n, 512 positions): [hi 32x512 | lo 32x512] fp8 planes
    hsx_d = nc.declare_dram_parameter("hsx", [4, 128, 32768], dt.float8e4, isOutput=False)
    # [p][k][640] hi and lo planes as separate params (hi loads first)
    wqh_d = nc.declare_dram_parameter("wqh", [128, 32 * 640], dt.float8e4, isOutput=False)
    wql_d = nc.declare_dram_parameter("wql", [128, 32 * 640], dt.float8e4, isOutput=False)
    # [p][kt][hi 4096 | lo 4096]
    wdx_d = nc.declare_dram_parameter("wdx", [128, 4 * 8192], dt.float8e4, isOutput=False)
    cst_d = nc.declare_dram_parameter("cst", [128, 2048], dt.bfloat16, isOutput=False)
    msk_d = nc.declare_dram_parameter("msk", [128, 128], dt.bfloat16, isOutput=False)
    ab_d = nc.declare_dram_parameter("ab", [128, 128], dt.float32, isOutput=False)
    idn_d = nc.declare_dram_parameter("idn", [64, 64], dt.bfloat16, isOutput=False)
    outp_d = nc.declare_dram_parameter("outp", [32, 128, NPOS], dt.bfloat16, isOutput=True)

    AF = mybir.ActivationFunctionType
    # packed causal offsets for et: block ki has width 1024-128*ki
    koff = [0] * 8
    for ki in range(1, 8):
        koff[ki] = koff[ki - 1] + (1024 - 128 * (ki - 1))
    ET_W = koff[7] + (1024 - 128 * 7)  # 4608

    with ExitStack() as ctx:
        tc = ctx.enter_context(tile.TileContext(nc))
        cpool = ctx.enter_context(tc.tile_pool(name="const", bufs=1))
        cst_sb = cpool.tile([128, 2048], dt.bfloat16)
        msk_sb = cpool.tile([128, 128], dt.bfloat16)
        ab_sb = cpool.tile([128, 128], dt.float32)
        idn_sb = cpool.tile([64, 64], dt.bfloat16)
        ones_b = cpool.tile([1, 64], dt.bfloat16)
        nc.vector.memset(ones_b[:], 1.0)

        hs_pool = ctx.enter_context(tc.tile_pool(name="hs", bufs=4))
        raw_pool = ctx.enter_context(tc.tile_pool(name="raw", bufs=2))
        tmp_pool = ctx.enter_context(tc.tile_pool(name="tmp", bufs=1))
        qp_pool = ctx.enter_context(tc.tile_pool(name="qp", bufs=2))
        kv_pool = ctx.enter_context(tc.tile_pool(name="kv", bufs=2))
        va_pool = ctx.enter_context(tc.tile_pool(name="va", bufs=2))
        et_pool = ctx.enter_context(tc.tile_pool(name="et", bufs=2))
        l_pool = ctx.enter_context(tc.tile_pool(name="l", bufs=1))
        rb_pool = ctx.enter_context(tc.tile_pool(name="rb", bufs=1))
        ctx_pool = ctx.enter_context(tc.tile_pool(name="ctx", bufs=2))
        cxl_pool = ctx.enter_context(tc.tile_pool(name="cxl", bufs=2))
        dout_pool = ctx.enter_context(tc.tile_pool(name="dout", bufs=3))
        wd_pool = ctx.enter_context(tc.tile_pool(name="wdp", bufs=1))
        wq_pool = ctx.enter_context(tc.tile_pool(name="wqp", bufs=1))
        mm = ctx.enter_context(tc.tile_pool(name="mm", bufs=1, space="PSUM"))

        # [128, k=32, hi|lo 1280] fp8
        # separate tiles per lazily-loaded slice: a shared tile would add
        # false write-after-read deps from in-flight matmuls to later loads
        wqh = [wq_pool.tile([128, 8, 640], dt.float8e4, tag=f"wqh{q}",
                            name=f"wqh{q}") for q in range(4)]
        wql = [wq_pool.tile([128, 16, 640], dt.float8e4, tag=f"wql{q}",
                            name=f"wql{q}") for q in range(2)]
        # small first slices unblock the first matmuls asap
        nc.sync.dma_start(wqh[0][:, 0:4, :], wqh_d[:, 0:4 * 640])
        nc.sync.dma_start(wqh[0][:, 4:8, :], wqh_d[:, 4 * 640:8 * 640])

        # per-batch SBUF state, filled by gen_qkv, read by gen_attn/gen_dense
        qp = {}   # (b, h) -> [64, 1024] bf16
        kk = {}   # b -> [64, 1024] bf16
        va = {}   # b -> [128, 8*72] bf16
        ctxt = {}  # (b, pr) -> [128, 1024] bf16
        cxh = {}  # b -> [128, 4, 1024] fp8 hi
        cxl = {}  # b -> [128, 4, 1024] fp8 lo
        wdx_t = []

        def gen_qkv(b):
            # two heads share one 128-partition tile (h even: rows 0-63,
            # h odd: rows 64-127); kk is duplicated into both halves so the
            # odd-head score matmuls use matching base_partition 64
            for pr in range(4):
                qph = qp_pool.tile([128, 1024], dt.bfloat16, tag=f"qph{pr}",
                                   name=f"qph{pr}_{b}")
                qp[(b, 2 * pr)] = qph[0:64, :]
                qp[(b, 2 * pr + 1)] = qph[64:128, :]
            kk[b] = kv_pool.tile([128, 1024], dt.bfloat16, tag="kk", name=f"kk{b}")
            vt = kv_pool.tile([64, 1024], dt.bfloat16, tag="vt", name=f"vt{b}")
            for n in range(2):
                c = b * 2 + n
                ncol = slice(n * 512, n * 512 + 512)
                # 4 hs tiles per chunk: hiH0 (k0-15), hiH1 (k16-31), loH0, loH1
                hst = {}

                def _load(part, engs):
                    # part: 0=hiH0 1=hiH1 2=loH0 3=loH1
                    t = hs_pool.tile([128, 16, 512], dt.float8e4, tag="hs",
                                     name=f"hs{part}_{c}")
                    if c == 0 and part == 0:
                        # split first load so the first matmul unblocks early;
                        # ride SP/HWDGE so desc-gen runs parallel to the Pool
                        # queue's weight stream
                        for g in range(4):
                            nc.sync.dma_start(
                                t[:, 4 * g:4 * g + 4, :],
                                hsx_d[c][:, g * 2048:(g + 1) * 2048])
                    else:
                        engs.dma_start(t[:], hsx_d[c][:, part * 8192:(part + 1) * 8192])
                    hst[part] = t

                _load(0, nc.gpsimd)
                if c == 0:
                    # chunk 0 is DMA-bound: emit everything on ONE queue in
                    # exact demand order (M1: wqh+hs-hi, M2: wql, M3: hs-lo)
                    # so the shared DMA engines serve it in priority order.
                    nc.gpsimd.dma_start(wqh[1][:], wqh_d[:, 8 * 640:16 * 640])
                    nc.gpsimd.dma_start(wqh[2][:], wqh_d[:, 16 * 640:24 * 640])
                    nc.gpsimd.dma_start(wqh[3][:], wqh_d[:, 24 * 640:32 * 640])
                    _load(1, nc.gpsimd)
                    nc.gpsimd.dma_start(wql[0][:], wql_d[:, 0:16 * 640])
                    nc.gpsimd.dma_start(wql[1][:], wql_d[:, 16 * 640:32 * 640])
                    _load(2, nc.gpsimd)
                    _load(3, nc.gpsimd)
                    # small consts (first needed by RoPE at ~30us) go LAST in
                    # the demand-ordered stream
                    nc.gpsimd.dma_start(cst_sb[:], cst_d[:])
                    nc.gpsimd.dma_start(msk_sb[:], msk_d[:])
                    nc.gpsimd.dma_start(ab_sb[:], ab_d[:])
                    nc.gpsimd.dma_start(idn_sb[:], idn_d[:])
                else:
                    _load(1, nc.sync)
                raw = [raw_pool.tile([128, 512], dt.bfloat16, tag=f"raw{m}",
                                     name=f"raw{m}_{n}_{b}") for m in range(5)]
                # b=0 (phase A, attention not running): one 5-bank sweep.
                # b=1 (phase B): two sweeps (m 0-2, then 3-4) so QKV holds
                # only qkv0..2 and attention keeps qkv3/qkv4/aux+cpsA/cpsB.
                # Either way 3 passes (M1 hi*hi, M2 lo-w*hi-x, M3 hi-w*lo-x):
                # the cold start only needs hi weights + hi activations.
                tag5 = ("qkv0", "qkv1", "qkv2", "cpsA", "cpsB")
                sweeps = ((0, 1, 2, 3, 4),) if b == 0 else ((0, 1, 2), (3, 4))
                for ms in sweeps:
                    ps = {m: mm.tile([128, 512], dt.float32, tag=tag5[j],
                                     name=f"qkv{m}_{n}_{b}")
                          for j, m in enumerate(ms)}
                    for pa in range(3):
                        for kp in range(16):
                            if ms[0] == 0 and pa == 0 and c > 0:
                                if kp == 4:
                                    _load(2, nc.gpsimd)
                                if kp == 10:
                                    _load(3, nc.sync)
                            half, i = kp // 8, kp % 8
                            mv = hst[(2 if pa == 2 else 0) + half][:, 2 * i:2 * i + 2, :]
                            for m in ms:
                                if pa == 1:
                                    wsl = wql[kp // 8][:, 2 * (kp % 8):2 * (kp % 8) + 2,
                                              m * 128:(m + 1) * 128]
                                else:
                                    wsl = wqh[kp // 4][:, 2 * (kp % 4):2 * (kp % 4) + 2,
                                              m * 128:(m + 1) * 128]
                                nc.tensor.matmul(ps[m][:], wsl, mv,
                                                 start=(pa == 0 and kp == 0),
                                                 stop=(pa == 2 and kp == 15),
                                                 perf_mode=DR)
                                if pa == 2 and kp == 15:
                                    # drain while PE finishes the rest
                                    nc.scalar.copy(raw[m][:], ps[m][:])
                            if kp % 2 == 1:
                                yield
                Cs = cst_sb[:, n * 512:(n + 1) * 512]
                Ss = cst_sb[:, 1024 + n * 512: 1024 + (n + 1) * 512]
                for grp in range(2):
                    A, Bb = raw[grp * 2], raw[grp * 2 + 1]
                    P1 = tmp_pool.tile([128, 512], dt.bfloat16, tag="P1")
                    P2 = tmp_pool.tile([128, 512], dt.bfloat16, tag="P2")
                    P3 = tmp_pool.tile([128, 512], dt.bfloat16, tag="P3")
                    P4 = tmp_pool.tile([128, 512], dt.bfloat16, tag="P4")
                    nc.vector.tensor_mul(P1[:], A[:], Cs)
                    nc.vector.tensor_mul(P2[:], Bb[:], Ss)
                    nc.vector.tensor_mul(P3[:], Bb[:], Cs)
                    nc.vector.tensor_mul(P4[:], A[:], Ss)
                    for i in range(4):
                        h = grp * 4 + i
                        sl = slice(32 * i, 32 * i + 32)
                        nc.vector.tensor_sub(qp[(b, h)][0:32, ncol], P1[sl, :], P2[sl, :])
                        nc.vector.tensor_add(qp[(b, h)][32:64, ncol], P3[sl, :], P4[sl, :])
                kvr = raw[4]
                # reuse the P tiles' space for the k-RoPE temporaries
                pk1 = tmp_pool.tile([128, 512], dt.bfloat16, tag="P1",
                                    name=f"pk1_{n}_{b}")[0:32, :]
                pk2 = tmp_pool.tile([128, 512], dt.bfloat16, tag="P2",
                                    name=f"pk2_{n}_{b}")[0:32, :]
                pk3 = tmp_pool.tile([128, 512], dt.bfloat16, tag="P3",
                                    name=f"pk3_{n}_{b}")[0:32, :]
                pk4 = tmp_pool.tile([128, 512], dt.bfloat16, tag="P4",
                                    name=f"pk4_{n}_{b}")[0:32, :]
                nc.vector.tensor_mul(pk1[:], kvr[0:32, :], Cs[0:32, :])
                nc.vector.tensor_mul(pk2[:], kvr[32:64, :], Ss[32:64, :])
                nc.vector.tensor_mul(pk3[:], kvr[32:64, :], Cs[32:64, :])
                nc.vector.tensor_mul(pk4[:], kvr[0:32, :], Ss[0:32, :])
                nc.vector.tensor_sub(kk[b][0:32, ncol], pk1[:], pk2[:])
                nc.vector.tensor_add(kk[b][32:64, ncol], pk3[:], pk4[:])
                nc.vector.tensor_copy(vt[:, ncol], kvr[64:128, :])
                # duplicate k into rows 64-127 for the odd (base-64) heads
                nc.gpsimd.tensor_copy(kk[b][64:128, ncol], kk[b][0:64, ncol])
                yield
            # V transpose + ones column (borrows the aux PSUM bank)
            va[b] = va_pool.tile([128, 8 * 72], dt.bfloat16, tag="va", name=f"va{b}")
            for ki in range(8):
                slot = mm.tile([128, 512], dt.float32, tag="aux", name=f"vps{ki}_{b}")
                vps = slot[:, 0:32].bitcast(dt.bfloat16)
                nc.tensor.transpose(vps, vt[0:64, ki * 128:(ki + 1) * 128],
                                    idn_sb[:, :])
                nc.vector.tensor_copy(va[b][:, ki * 72: ki * 72 + 64], vps)
                nc.vector.memset(va[b][:, ki * 72 + 64: ki * 72 + 65], ONES_VAL)
            yield

        def gen_attn(b):
            for pr in range(4):
                ctxt[(b, pr)] = ctx_pool.tile([128, 1024], dt.bfloat16,
                                              tag=f"ctxt{pr}", name=f"ctxt{pr}_{b}")
            cxh[b] = ctx_pool.tile([128, 4, 1024], dt.float8e4, tag="cxh",
                                   name=f"cxh{b}")
            cxl[b] = cxl_pool.tile([128, 4, 1024], dt.float8e4, tag="cxl",
                                   name=f"cxl{b}")
            # Both batches: heads are software-pipelined (scores of head h
            # interleave with PV/epi of head h-1) so the exp round-trip is
            # hidden; the co-running generator (qkv(1) in phase B, dense(0)
            # in phase C) packs the remaining PE gaps. Scores rotate through
            # qkv3/qkv4/aux, PV owns cpsA/cpsB, qkv/dense use qkv0..2.
            rot = ("qkv3", "qkv4", "aux")

            def make_head(h):
                st = {}
                st['et'] = et_pool.tile([128, ET_W], dt.bfloat16, tag="et",
                                        name=f"et{h}_{b}")
                rrb = rb_pool.tile([128, 1024], dt.bfloat16, tag="rb",
                                   name=f"rrb{h}_{b}")
                st['rr'] = rrb[0:1, :]
                st['rb'] = rrb[64:128, :]
                st['cph'] = [
                    mm.tile([128, 512], dt.float32, tag="cpsA", name=f"cpA{h}_{b}"),
                    mm.tile([128, 512], dt.float32, tag="cpsB", name=f"cpB{h}_{b}")]
                st['ci'] = 0
                return st

            def score_ki(st, h, ki):
                et = st['et']
                base = ki * 128
                nchunks = (1024 - base + 511) // 512
                row0 = 64 * (h % 2)
                for cj in range(nchunks):
                    c0 = base + cj * 512
                    cw = min(512, 1024 - c0)
                    sc = mm.tile([128, 512], dt.float32,
                                 tag=rot[st['ci'] % len(rot)],
                                 name=f"sc{h}{ki}{cj}_{b}")
                    st['ci'] += 1
                    nc.tensor.matmul(
                        sc[:, 0:cw],
                        kk[b][row0:row0 + 64, base:base + 128],
                        qp[(b, h)][0:64, c0:c0 + cw],
                        start=True, stop=True,
                    )
                    abc = b * 64 + ki * 8 + h
                    nc.scalar.activation(
                        et[:, koff[ki] + (c0 - base): koff[ki] + (c0 - base) + cw],
                        sc[:, 0:cw], AF.Exp,
                        bias=ab_sb[:, abc:abc + 1], scale=EXP_SCALE)
                    if cj == 0:
                        # causal mask: zero the upper triangle of the diag
                        # block via a 0/1 multiply (off the sc->exp chain).
                        # Pool is otherwise idle and ACT/DVE are saturated.
                        nc.gpsimd.tensor_mul(
                            et[:, koff[ki]: koff[ki] + 128],
                            et[:, koff[ki]: koff[ki] + 128], msk_sb[:])

            def pv_ki(st, h, ki):
                et, cph = st['et'], st['cph']
                g0 = ki * 128
                while g0 < 1024:
                    half = g0 // 512
                    g1 = min(1024, (half + 1) * 512)
                    loc = slice(g0 - half * 512, g1 - half * 512)
                    nc.tensor.matmul(
                        cph[half][0:65, loc],
                        va[b][:, ki * 72: ki * 72 + 65],
                        et[:, koff[ki] + g0 - ki * 128: koff[ki] + g1 - ki * 128],
                        start=(ki == 0), stop=(ki == (3 if half == 0 else 7)),
                        skip_group_check=True,
                    )
                    g0 = g1

            def epi(st, h, ki):
                # epilogue for the finished half: A after ki=3, B after 7
                rr, rb, cph = st['rr'], st['rb'], st['cph']
                pr, hh = h // 2, h % 2
                half = 0 if ki == 3 else 1
                hs_ = slice(half * 512, half * 512 + 512)
                # reciprocal straight from the PSUM ones-row
                # (f32r is fp32-width; the gate only knows dtype != f32)
                with nc.allow_low_precision(reason="1/l in bf16 is accurate enough"):
                    nc.vector.reciprocal(rr[0:1, hs_], cph[half][64:65, 0:512])
                slot = mm.tile([128, 512], dt.float32, tag="aux",
                               name=f"rps{h}{half}_{b}")
                nc.tensor.matmul(slot[0:64, :], ones_b[:], rr[:, hs_],
                                 start=True, stop=True)
                nc.vector.tensor_copy(rb[:, hs_], slot[0:64, :])
                nc.vector.tensor_mul(
                    ctxt[(b, pr)][hh * 64:(hh + 1) * 64, hs_],
                    cph[half][0:64, 0:512], rb[:, hs_])
                if ki == 7 and hh == 1:
                    # head pair done: split ctx into fp8 hi+lo planes for the
                    # DoubleRow dense. Both SBUF-only ops go to the idle Pool
                    # engine; ACT (exp) and DVE (recip/epi) are saturated.
                    nc.gpsimd.tensor_copy(cxh[b][:, pr, :], ctxt[(b, pr)][:])
                    nc.gpsimd.tensor_sub(cxl[b][:, pr, :], ctxt[(b, pr)][:],
                                         cxh[b][:, pr, :])

            if b == 0:
                wdx = wd_pool.tile([128, 4, 8192], dt.float8e4, tag="wdx",
                                   name="wdx")
                wdx_t.append(wdx)
            prev = None
            for slot in range(9):
                if b == 0 and 1 <= slot <= 4:
                    # dense weights stream during phase B; emitting them
                    # inside the slot loop keeps them queued behind phase-B
                    # work so they don't steal DMA bandwidth from phase A
                    kq = slot - 1
                    nc.scalar.dma_start(wdx_t[0][:, kq, :],
                                        wdx_d[:, kq * 8192:(kq + 1) * 8192])
                cur = make_head(slot) if slot < 8 else None
                for ki in range(8):
                    if cur is not None:
                        score_ki(cur, slot, ki)
                    if prev is not None:
                        pv_ki(prev, slot - 1, ki)
                        if ki == 3 or ki == 7:
                            epi(prev, slot - 1, ki)
                    yield
                prev = cur

        def gen_dense(b):
            wdx = wdx_t[0]
            # b=1 runs after attention is done, so the score-rotation banks
            # are free for deeper accumulate/drain pipelining
            slots = ("qkv0", "qkv1", "qkv2") if b == 0 else (
                "qkv0", "qkv1", "qkv2", "qkv3", "aux", "cpsA")
            for mt in range(32):
                dsb = dout_pool.tile([128, 1024], dt.bfloat16, tag="dsb",
                                     name=f"dsb{mt}_{b}")
                for n2 in range(2):
                    dps = mm.tile([128, 512], dt.float32,
                                  tag=slots[(mt * 2 + n2) % len(slots)],
                                  name=f"d{mt}{n2}_{b}")
                    n2s = slice(n2 * 512, (n2 + 1) * 512)
                    for t in range(2):
                        ks = slice(2 * t, 2 * t + 2)
                        w_hi = wdx[:, ks, mt * 128:(mt + 1) * 128]
                        w_lo = wdx[:, ks, 4096 + mt * 128:4096 + (mt + 1) * 128]
                        nc.tensor.matmul(dps[:], w_hi, cxh[b][:, ks, n2s],
                                         start=(t == 0), stop=False, perf_mode=DR)
                        nc.tensor.matmul(dps[:], w_lo, cxh[b][:, ks, n2s],
                                         start=False, stop=False, perf_mode=DR)
                        nc.tensor.matmul(dps[:], w_hi, cxl[b][:, ks, n2s],
                                         start=False, stop=(t == 1), perf_mode=DR)
                        if b == 0:
                            # fine-grained quanta so dense matmuls pack into
                            # the gaps of attn(1)'s serial score->exp chain
                            yield
                    # phase C: ACT is exp-bound, keep drains on DVE until the
                    # exps dry up (~mt 24); phase D and late C use ACT too
                    if (b == 1 or mt >= 24) and n2 == 0:
                        nc.scalar.mul(dsb[:, 0:512], dps[:], DRAIN_SCALE)
                    else:
                        nc.vector.tensor_scalar_mul(
                            dsb[:, n2 * 512:(n2 + 1) * 512], dps[:], DRAIN_SCALE)
                    if b == 1 and mt == 31:
                        # last tile: ship halves separately to shorten the
                        # final copy->DMA drain chain
                        nc.sync.dma_start(
                            outp_d[mt][:, b * 1024 + n2 * 512:
                                        b * 1024 + n2 * 512 + 512],
                            dsb[:, n2 * 512:(n2 + 1) * 512])
                    yield
                if not (b == 1 and mt == 31):
                    # SP queue is idle through C/D; keep Pool free for drains
                    nc.sync.dma_start(
                        outp_d[mt][:, b * 1024: b * 1024 + 1024], dsb[:])

        def _chain(*gens):
            for g in gens:
                yield from g

        _drive(gen_qkv(0))
        # merge phases B and C: as soon as qkv(1) finishes emitting, attn(1)
        # interleaves with attn(0)'s tail; when attn(0) ends, dense(0)
        # interleaves with attn(1)'s tail.
        _drive(_chain(gen_qkv(1), gen_attn(1)),
               _chain(gen_attn(0), gen_dense(0)))
        _drive(gen_dense(1))

    _CACHED_NC = nc
    return nc


def _split8(x):
    """x (f32) -> (hi, lo) fp8e4 with x ~= hi + lo."""
    hi = x.astype(f8)
    lo = (x - hi.astype(np.float32)).astype(f8)
    return hi, lo


def host_prep(hidden_states, alibi, attention_mask, W_qkv, W_dense):
    hsT = np.ascontiguousarray(hidden_states.reshape(NPOS, HID).T)  # [4096, 2048]
    hh, hl = _split8(hsT.astype(np.float32))
    # hsx[c][p][k*512+j] (hi) / 16384 + same (lo) = hsT[k*128+p, c*512+j]
    def _arr(x8):
        return np.ascontiguousarray(
            x8.reshape(32, 128, 4, 512).transpose(2, 1, 0, 3).reshape(4, 128, 16384))
    hsx = np.concatenate([_arr(hh), _arr(hl)], axis=2)  # [4, 128, 32768]

    j32 = np.arange(32)
    inv_freq = 1.0 / (10000.0 ** (2 * j32 / HD))
    t = np.arange(S, dtype=np.float64)
    fr = np.outer(inv_freq, t)                       # [32, S]
    cst = np.zeros((128, 2048), np.float32)
    cst[:, 0:1024] = np.tile(np.cos(fr), (4, 1))
    cst[:, 1024:2048] = np.tile(np.sin(fr), (4, 1))
    cst = cst.astype(bf16)

    # single causal diag block, [kpos, q] layout: 0 where kpos > q, else 1
    mf = np.where(attention_mask[0, 0, 0:128, 0:128], 0.0, 1.0).astype(np.float32)
    msk = np.ascontiguousarray(mf.T).astype(bf16)    # [kpos, q]

    al = alibi.reshape(B, NKV * G, S) * INV          # [B, 64, S]

    perm = []
    for i in range(4):
        perm += [i * 64 + d for d in range(32)]
    for i in range(4):
        perm += [i * 64 + 32 + d for d in range(32)]
    for i in range(4, 8):
        perm += [i * 64 + d for d in range(32)]
    for i in range(4, 8):
        perm += [i * 64 + 32 + d for d in range(32)]
    perm += [512 + d for d in range(64)] + [576 + d for d in range(64)]
    perm = np.array(perm)

    idn = np.eye(64, dtype=np.float32).astype(bf16)
    in_maps = []
    for c in range(NCORES):
        Wg = (W_qkv[c * 640:(c + 1) * 640][perm] * WS).astype(np.float32)
        WgT = np.ascontiguousarray(Wg.T)              # [4096, 640]
        wh, wl = _split8(WgT)
        wqh = np.ascontiguousarray(
            wh.reshape(32, 128, 640).transpose(1, 0, 2)).reshape(128, 32 * 640)
        wql = np.ascontiguousarray(
            wl.reshape(32, 128, 640).transpose(1, 0, 2)).reshape(128, 32 * 640)

        Wd = (W_dense[:, c * 512:(c + 1) * 512] * WS).astype(np.float32)
        WdT = np.ascontiguousarray(Wd.T)              # [512, 4096]
        dh, dl = _split8(WdT)
        wdx = np.concatenate(
            [dh.reshape(4, 128, 4096).transpose(1, 0, 2),
             dl.reshape(4, 128, 4096).transpose(1, 0, 2)], axis=2)  # [128,4,8192]
        wdx = np.ascontiguousarray(wdx).reshape(128, 4 * 8192)

        ab = np.zeros((128, 128), np.float32)
        for b in range(2):
            for ki in range(8):
                for h in range(8):
                    ab[:, b * 64 + ki * 8 + h] = al[b, c * 8 + h,
                                                    ki * 128:(ki + 1) * 128]
        in_maps.append({
            "hsx": hsx, "wqh": wqh, "wql": wql, "wdx": wdx, "cst": cst,
            "msk": msk, "ab": ab, "idn": idn,
        })
    return in_maps


def kernel(hidden_states, alibi, attention_mask, W_qkv, W_dense, _want_time=False):
    nc = build_program()
    in_maps = host_prep(np.asarray(hidden_states), np.asarray(alibi),
                        np.asarray(attention_mask), np.asarray(W_qkv),
                        np.asarray(W_dense))
    res = run_bass_kernel_spmd(nc, in_maps, list(range(NCORES)))
    acc = np.zeros((32, 128, NPOS), np.float32)
    for c in range(NCORES):
        acc += res.results[c]["outp"].astype(np.float32)
    out = acc.reshape(4096, NPOS).T.reshape(B, S, HID)
    if _want_time:
        return np.ascontiguousarray(out), res
    return np.ascontiguousarray(out)


# revision 93
# speedup vs baseline: 1.1140x; 1.0040x over previous
"""GQA attention block (dense_transformer) on 8 trn2 cores.

Sharding: tensor-parallel by kv-group. Core c owns kv-group c = 8 query
heads + 1 k + 1 v head (640 rows of W_qkv) and the matching 512 columns of
W_dense. hidden_states is replicated. Each core returns a bf16 partial
[4096, 2048] dense output; the host sums the 8 partials in f32.

v3: split-precision fp8 DoubleRow for the two big GEMMs. QKV and dense
weights/activations are decomposed host-side (and ctx on-chip) into
hi+lo fp8e4 planes; each K=256 pair runs as 3 DoubleRow matmuls
(hi*hi, lo-w*hi-x, hi-w*lo-x) at 0.5 cycles/col = 0.75x the bf16 PE
cost with bf16-level accuracy (dropped lo*lo term ~2^-8). Attention
internals (RoPE, scores, softmax, PV, epilogue) stay bf16. Scales:
W_qkv and W_dense x64 host-side (q,k,v 64x), exp activation scale
INV/4096, va ones column 4.0 (ctx 16x true), dense drains x2^-10.

Schedule: QKV holds only PSUM banks qkv0..2 (two m-sweeps for b=1, one
5-bank sweep in phase A) so attention permanently owns qkv3/qkv4/aux
score rotation + cpsA/cpsB PV accumulators. Attention heads are
software-pipelined (scores of head h emit interleaved with PV/epi of
head h-1) to hide the exp round-trip, and phases are fused by chaining
generators [qkv(1)->attn(1)] || [attn(0)->dense(0)] so a compute-dense
partner always fills the serial softmax chain's PE bubbles in the
in-order queues. Two heads share each 128-partition qp tile (kk is
duplicated to rows 64-127 so odd heads use base_partition 64). Engine
balance: exps+b1 drains on ACT, recip/epi/drains on DVE, masks and
ctx hi/lo splits on the otherwise idle Pool, output DMAs on SP.
Chunk 0's loads are emitted on a single SWDGE queue in exact demand
order (M1: wqh+hs-hi, M2: wql, M3: hs-lo, then consts) since the cold
start is supply-limited on the shared DMA engines (~28us of transfers
vs 25.6us of compute); even the small consts matter there, and the
first hs sub-loads ride SP/HWDGE so their descriptor generation runs
parallel to the Pool queue's weight stream. Measured (TimelineSim):
295.3us vs 327.6us for the session baseline; rel err 0.0035 (gate
2e-2). PE busy 265.7us (90% occupancy); the residual idle is the
supply-limited phase-A cold start and the attention ramp at the B/C
seam.
"""
import numpy as np
import ml_dtypes
from contextlib import ExitStack

import bass_rust
import concourse.bass as bass
import concourse.mybir as mybir
from concourse import tile
from concourse.bass_utils import run_bass_kernel_spmd

dt = mybir.dt
bf16 = ml_dtypes.bfloat16
f8 = ml_dtypes.float8_e4m3

B, S, HID = 2, 1024, 4096
NKV, G, HD = 8, 8, 64
NPOS = B * S
INV = 0.125
WS = 64.0                      # host-side weight scale (q,k,v come out 64x)
EXP_SCALE = INV / (WS * WS)    # PSUM scores are 4096x true
ONES_VAL = 4.0                 # va ones column -> ctx = 16x true
DRAIN_SCALE = 1.0 / 1024.0     # dense psum = 16 * 64 = 1024x true
NCORES = 8
DR = mybir.MatmulPerfMode.DoubleRow

# ---------------------------------------------------------------------------
# walrus in this container takes at most ONE sync-wait per instruction; Tile
# attaches several (tail drain especially). Split extras onto same-engine nops.
_orig_exit = tile.TileContext.__exit__


def _split_waits(nc):
    for bb in nc.m.functions[0].blocks:
        out, extra = [], 0
        for inst in bb.instructions:
            si = inst.sync_info
            if si is not None and len(si.on_wait) > 1:
                waits = list(si.on_wait)
                for w in waits[:-1]:
                    nop = mybir.InstNoOp(name=f"I-wsplit-{nc.next_id()}")
                    nop.engine = inst.engine
                    nop.sync_info = bass_rust.SyncInfo(on_wait=[w], on_update=[])
                    nc.register_instruction(nop, overwrite=True)
                    out.append(nop)
                    extra += 1
                inst.sync_info = bass_rust.SyncInfo(
                    on_wait=[waits[-1]], on_update=list(si.on_update)
                )
            out.append(inst)
        if extra:
            bb.instructions = out


def _patched_exit(self, exc_type, exc_val, exc_tb):
    r = _orig_exit(self, exc_type, exc_val, exc_tb)
    _split_waits(self.nc)
    return r


tile.TileContext.__exit__ = _patched_exit
# ---------------------------------------------------------------------------

_CACHED_NC = None


def _drive(*gens):
    live = list(gens)
    while live:
        for g in list(live):
            try:
                next(g)
            except StopIteration:
                live.remove(g)


def build_program():
    global _CACHED_NC
    if _CACHED_NC is not None:
        return _CACHED_NC
    nc = bass.Bass()
    # per chunk c (=b*2+n, 512 positions): [hi 32x512 | lo 32x512] fp8 planes
    hsx_d = nc.declare_dram_parameter("hsx", [4, 128, 32768], dt.float8e4, isOutput=False)
    # [p][k][640] hi and lo planes as separate params (hi loads first)
    wqh_d = nc.declare_dram_parameter("wqh", [128, 32 * 640], dt.float8e4, isOutput=False)
    wql_d = nc.declare_dram_parameter("wql", [128, 32 * 640], dt.float8e4, isOutput=False)
    # [p][kt][hi 4096 | lo 4096]
    wdx_d = nc.declare_dram_parameter("wdx", [128, 4 * 8192], dt.float8e4, isOutput=False)
    cst_d = nc.declare_dram_parameter("cst", [128, 2048], dt.bfloat16, isOutput=False)
    msk_d = nc.declare_dram_parameter("msk", [128, 128], dt.bfloat16, isOutput=False)
    ab_d = nc.declare_dram_parameter("ab", [128, 128], dt.float32, isOutput=False)
    idn_d = nc.declare_dram_parameter("idn", [64, 64], dt.bfloat16, isOutput=False)
    outp_d = nc.declare_dram_parameter("outp", [32, 128, NPOS], dt.bfloat16, isOutput=True)

    AF = mybir.ActivationFunctionType
    # packed causal offsets for et: block ki has width 1024-128*ki
    koff = [0] * 8
    for ki in range(1, 8):
        koff[ki] = koff[ki - 1] + (1024 - 128 * (ki - 1))
    ET_W = koff[7] + (1024 - 128 * 7)  # 4608

    with ExitStack() as ctx:
        tc = ctx.enter_context(tile.TileContext(nc))
        cpool = ctx.enter_context(tc.tile_pool(name="const", bufs=1))
        cst_sb = cpool.tile([128, 2048], dt.bfloat16)
        msk_sb = cpool.tile([128, 128], dt.bfloat16)
        ab_sb = cpool.tile([128, 128], dt.float32)
        idn_sb = cpool.tile([64, 64], dt.bfloat16)
        ones_b = cpool.tile([1, 64], dt.bfloat16)
        nc.vector.memset(ones_b[:], 1.0)

        hs_pool = ctx.enter_context(tc.tile_pool(name="hs", bufs=4))
        raw_pool = ctx.enter_context(tc.tile_pool(name="raw", bufs=2))
        tmp_pool = ctx.enter_context(tc.tile_pool(name="tmp", bufs=1))
        qp_pool = ctx.enter_context(tc.tile_pool(name="qp", bufs=2))
        kv_pool = ctx.enter_context(tc.tile_pool(name="kv", bufs=2))
        va_pool = ctx.enter_context(tc.tile_pool(name="va", bufs=2))
        et_pool = ctx.enter_context(tc.tile_pool(name="et", bufs=2))
        l_pool = ctx.enter_context(tc.tile_pool(name="l", bufs=1))
        rb_pool = ctx.enter_context(tc.tile_pool(name="rb", bufs=1))
        ctx_pool = ctx.enter_context(tc.tile_pool(name="ctx", bufs=2))
        cxl_pool = ctx.enter_context(tc.tile_pool(name="cxl", bufs=2))
        dout_pool = ctx.enter_context(tc.tile_pool(name="dout", bufs=3))
        wd_pool = ctx.enter_context(tc.tile_pool(name="wdp", bufs=1))
        wq_pool = ctx.enter_context(tc.tile_pool(name="wqp", bufs=1))
        mm = ctx.enter_context(tc.tile_pool(name="mm", bufs=1, space="PSUM"))

        # [128, k=32, hi|lo 1280] fp8
        # separate tiles per lazily-loaded slice: a shared tile would add
        # false write-after-read deps from in-flight matmuls to later loads
        wqh = [wq_pool.tile([128, 8, 640], dt.float8e4, tag=f"wqh{q}",
                            name=f"wqh{q}") for q in range(4)]
        wql = [wq_pool.tile([128, 16, 640], dt.float8e4, tag=f"wql{q}",
                            name=f"wql{q}") for q in range(2)]
        # small first slices unblock the first matmuls asap
        nc.sync.dma_start(wqh[0][:, 0:4, :], wqh_d[:, 0:4 * 640])
        nc.sync.dma_start(wqh[0][:, 4:8, :], wqh_d[:, 4 * 640:8 * 640])

        # per-batch SBUF state, filled by gen_qkv, read by gen_attn/gen_dense
        qp = {}   # (b, h) -> [64, 1024] bf16
        kk = {}   # b -> [64, 1024] bf16
        va = {}   # b -> [128, 8*72] bf16
        ctxt = {}  # (b, pr) -> [128, 1024] bf16
        cxh = {}  # b -> [128, 4, 1024] fp8 hi
        cxl = {}  # b -> [128, 4, 1024] fp8 lo
        wdx_t = []

        def gen_qkv(b):
            # two heads share one 128-partition tile (h even: rows 0-63,
            # h odd: rows 64-127); kk is duplicated into both halves so the
            # odd-head score matmuls use matching base_partition 64
            for pr in range(4):
                qph = qp_pool.tile([128, 1024], dt.bfloat16, tag=f"qph{pr}",
                                   name=f"qph{pr}_{b}")
                qp[(b, 2 * pr)] = qph[0:64, :]
                qp[(b, 2 * pr + 1)] = qph[64:128, :]
            kk[b] = kv_pool.tile([128, 1024], dt.bfloat16, tag="kk", name=f"kk{b}")
            vt = kv_pool.tile([64, 1024], dt.bfloat16, tag="vt", name=f"vt{b}")
            for n in range(2):
                c = b * 2 + n
                ncol = slice(n * 512, n * 512 + 512)
                # 4 hs tiles per chunk: hiH0 (k0-15), hiH1 (k16-31), loH0, loH1
                hst = {}

                def _load(part, engs):
                    # part: 0=hiH0 1=hiH1 2=loH0 3=loH1
                    t = hs_pool.tile([128, 16, 512], dt.float8e4, tag="hs",
                                     name=f"hs{part}_{c}")
                    if c == 0 and part == 0:
                        # split first load so the first matmul unblocks early;
                        # ride SP/HWDGE so desc-gen runs parallel to the Pool
                        # queue's weight stream
                        for g in range(4):
                            nc.sync.dma_start(
                                t[:, 4 * g:4 * g + 4, :],
                                hsx_d[c][:, g * 2048:(g + 1) * 2048])
                    else:
                        engs.dma_start(t[:], hsx_d[c][:, part * 8192:(part + 1) * 8192])
                    hst[part] = t

                _load(0, nc.gpsimd)
                if c == 0:
                    # chunk 0 is DMA-bound: emit everything on ONE queue in
                    # exact demand order (M1: wqh+hs-hi, M2: wql, M3: hs-lo)
                    # so the shared DMA engines serve it in priority order.
                    nc.gpsimd.dma_start(wqh[1][:], wqh_d[:, 8 * 640:16 * 640])
                    nc.gpsimd.dma_start(wqh[2][:], wqh_d[:, 16 * 640:24 * 640])
                    nc.gpsimd.dma_start(wqh[3][:], wqh_d[:, 24 * 640:32 * 640])
                    _load(1, nc.gpsimd)
                    nc.gpsimd.dma_start(wql[0][:], wql_d[:, 0:16 * 640])
                    nc.gpsimd.dma_start(wql[1][:], wql_d[:, 16 * 640:32 * 640])
                    _load(2, nc.gpsimd)
                    _load(3, nc.gpsimd)
                    # small consts (first needed by RoPE at ~30us) go LAST in
                    # the demand-ordered stream
                    nc.gpsimd.dma_start(cst_sb[:], cst_d[:])
                    nc.gpsimd.dma_start(msk_sb[:], msk_d[:])
                    nc.gpsimd.dma_start(ab_sb[:], ab_d[:])
                    nc.gpsimd.dma_start(idn_sb[:], idn_d[:])
                else:
                    _load(1, nc.sync)
                raw = [raw_pool.tile([128, 512], dt.bfloat16, tag=f"raw{m}",
                                     name=f"raw{m}_{n}_{b}") for m in range(5)]
                # b=0 (phase A, attention not running): one 5-bank sweep.
                # b=1 (phase B): two sweeps (m 0-2, then 3-4) so QKV holds
                # only qkv0..2 and attention keeps qkv3/qkv4/aux+cpsA/cpsB.
                # Either way 3 passes (M1 hi*hi, M2 lo-w*hi-x, M3 hi-w*lo-x):
                # the cold start only needs hi weights + hi activations.
                tag5 = ("qkv0", "qkv1", "qkv2", "cpsA", "cpsB")
                sweeps = ((0, 1, 2, 3, 4),) if b == 0 else ((0, 1, 2), (3, 4))
                for ms in sweeps:
                    ps = {m: mm.tile([128, 512], dt.float32, tag=tag5[j],
                                     name=f"qkv{m}_{n}_{b}")
                          for j, m in enumerate(ms)}
                    for pa in range(3):
                        for kp in range(16):
                            if ms[0] == 0 and pa == 0 and c > 0:
                                if kp == 4:
                                    _load(2, nc.gpsimd)
                                if kp == 10:
                                    _load(3, nc.sync)
                            half, i = kp // 8, kp % 8
                            mv = hst[(2 if pa == 2 else 0) + half][:, 2 * i:2 * i + 2, :]
                            for m in ms:
                                if pa == 1:
                                    wsl = wql[kp // 8][:, 2 * (kp % 8):2 * (kp % 8) + 2,
                                              m * 128:(m + 1) * 128]
                                else:
                                    wsl = wqh[kp // 4][:, 2 * (kp % 4):2 * (kp % 4) + 2,
                                              m * 128:(m + 1) * 128]
                                nc.tensor.matmul(ps[m][:], wsl, mv,
                                                 start=(pa == 0 and kp == 0),
                                                 stop=(pa == 2 and kp == 15),
                                                 perf_mode=DR)
                                if pa == 2 and kp == 15:
                                    # drain while PE finishes the rest
                                    nc.scalar.copy(raw[m][:], ps[m][:])
                            if kp % 2 == 1:
                                yield
                Cs = cst_sb[:, n * 512:(n + 1) * 512]
                Ss = cst_sb[:, 1024 + n * 512: 1024 + (n + 1) * 512]
                for grp in range(2):
                    A, Bb = raw[grp * 2], raw[grp * 2 + 1]
                    P1 = tmp_pool.tile([128, 512], dt.bfloat16, tag="P1")
                    P2 = tmp_pool.tile([128, 512], dt.bfloat16, tag="P2")
                    P3 = tmp_pool.tile([128, 512], dt.bfloat16, tag="P3")
                    P4 = tmp_pool.tile([128, 512], dt.bfloat16, tag="P4")
                    nc.vector.tensor_mul(P1[:], A[:], Cs)
                    nc.vector.tensor_mul(P2[:], Bb[:], Ss)
                    nc.vector.tensor_mul(P3[:], Bb[:], Cs)
                    nc.vector.tensor_mul(P4[:], A[:], Ss)
                    for i in range(4):
                        h = grp * 4 + i
                        sl = slice(32 * i, 32 * i + 32)
                        nc.vector.tensor_sub(qp[(b, h)][0:32, ncol], P1[sl, :], P2[sl, :])
                        nc.vector.tensor_add(qp[(b, h)][32:64, ncol], P3[sl, :], P4[sl, :])
                kvr = raw[4]
                # reuse the P tiles' space for the k-RoPE temporaries
                pk1 = tmp_pool.tile([128, 512], dt.bfloat16, tag="P1",
                                    name=f"pk1_{n}_{b}")[0:32, :]
                pk2 = tmp_pool.tile([128, 512], dt.bfloat16, tag="P2",
                                    name=f"pk2_{n}_{b}")[0:32, :]
                pk3 = tmp_pool.tile([128, 512], dt.bfloat16, tag="P3",
                                    name=f"pk3_{n}_{b}")[0:32, :]
                pk4 = tmp_pool.tile([128, 512], dt.bfloat16, tag="P4",
                                    name=f"pk4_{n}_{b}")[0:32, :]
                nc.vector.tensor_mul(pk1[:], kvr[0:32, :], Cs[0:32, :])
                nc.vector.tensor_mul(pk2[:], kvr[32:64, :], Ss[32:64, :])
                nc.vector.tensor_mul(pk3[:], kvr[32:64, :], Cs[32:64, :])
                nc.vector.tensor_mul(pk4[:], kvr[0:32, :], Ss[0:32, :])
                nc.vector.tensor_sub(kk[b][0:32, ncol], pk1[:], pk2[:])
                nc.vector.tensor_add(kk[b][32:64, ncol], pk3[:], pk4[:])
                nc.vector.tensor_copy(vt[:, ncol], kvr[64:128, :])
                # duplicate k into rows 64-127 for the odd (base-64) heads
                nc.gpsimd.tensor_copy(kk[b][64:128, ncol], kk[b][0:64, ncol])
                yield
            # V transpose + ones column (borrows the aux PSUM bank)
            va[b] = va_pool.tile([128, 8 * 72], dt.bfloat16, tag="va", name=f"va{b}")
            for ki in range(8):
                slot = mm.tile([128, 512], dt.float32, tag="aux", name=f"vps{ki}_{b}")
                vps = slot[:, 0:32].bitcast(dt.bfloat16)
                nc.tensor.transpose(vps, vt[0:64, ki * 128:(ki + 1) * 128],
                                    idn_sb[:, :])
                nc.vector.tensor_copy(va[b][:, ki * 72: ki * 72 + 64], vps)
                nc.vector.memset(va[b][:, ki * 72 + 64: ki * 72 + 65], ONES_VAL)
            yield

        def gen_attn(b):
            for pr in range(4):
                ctxt[(b, pr)] = ctx_pool.tile([128, 1024], dt.bfloat16,
                                              tag=f"ctxt{pr}", name=f"ctxt{pr}_{b}")
            cxh[b] = ctx_pool.tile([128, 4, 1024], dt.float8e4, tag="cxh",
                                   name=f"cxh{b}")
            cxl[b] = cxl_pool.tile([128, 4, 1024], dt.float8e4, tag="cxl",
                                   name=f"cxl{b}")
            # Both batches: heads are software-pipelined (scores of head h
            # interleave with PV/epi of head h-1) so the exp round-trip is
            # hidden; the co-running generator (qkv(1) in phase B, dense(0)
            # in phase C) packs the remaining PE gaps. Scores rotate through
            # qkv3/qkv4/aux, PV owns cpsA/cpsB, qkv/dense use qkv0..2.
            rot = ("qkv3", "qkv4", "aux")

            def make_head(h):
                st = {}
                st['et'] = et_pool.tile([128, ET_W], dt.bfloat16, tag="et",
                                        name=f"et{h}_{b}")
                rrb = rb_pool.tile([128, 1024], dt.bfloat16, tag="rb",
                                   name=f"rrb{h}_{b}")
                st['rr'] = rrb[0:1, :]
                st['rb'] = rrb[64:128, :]
                st['cph'] = [
                    mm.tile([128, 512], dt.float32, tag="cpsA", name=f"cpA{h}_{b}"),
                    mm.tile([128, 512], dt.float32, tag="cpsB", name=f"cpB{h}_{b}")]
                st['ci'] = 0
                return st

            def score_ki(st, h, ki):
                et = st['et']
                base = ki * 128
                nchunks = (1024 - base + 511) // 512
                row0 = 64 * (h % 2)
                for cj in range(nchunks):
                    c0 = base + cj * 512
                    cw = min(512, 1024 - c0)
                    sc = mm.tile([128, 512], dt.float32,
                                 tag=rot[st['ci'] % len(rot)],
                                 name=f"sc{h}{ki}{cj}_{b}")
                    st['ci'] += 1
                    nc.tensor.matmul(
                        sc[:, 0:cw],
                        kk[b][row0:row0 + 64, base:base + 128],
                        qp[(b, h)][0:64, c0:c0 + cw],
                        start=True, stop=True,
                    )
                    abc = b * 64 + ki * 8 + h
                    nc.scalar.activation(
                        et[:, koff[ki] + (c0 - base): koff[ki] + (c0 - base) + cw],
                        sc[:, 0:cw], AF.Exp,
                        bias=ab_sb[:, abc:abc + 1], scale=EXP_SCALE)
                    if cj == 0:
                        # causal mask: zero the upper triangle of the diag
                        # block via a 0/1 multiply (off the sc->exp chain).
                        # Pool is otherwise idle and ACT/DVE are saturated.
                        nc.gpsimd.tensor_mul(
                            et[:, koff[ki]: koff[ki] + 128],
                            et[:, koff[ki]: koff[ki] + 128], msk_sb[:])

            def pv_ki(st, h, ki):
                et, cph = st['et'], st['cph']
                g0 = ki * 128
                while g0 < 1024:
                    half = g0 // 512
                    g1 = min(1024, (half + 1) * 512)
                    loc = slice(g0 - half * 512, g1 - half * 512)
                    nc.tensor.matmul(
                        cph[half][0:65, loc],
                        va[b][:, ki * 72: ki * 72 + 65],
                        et[:, koff[ki] + g0 - ki * 128: koff[ki] + g1 - ki * 128],
                        start=(ki == 0), stop=(ki == (3 if half == 0 else 7)),
                        skip_group_check=True,
                    )
                    g0 = g1

            def epi(st, h, ki):
                # epilogue for the finished half: A after ki=3, B after 7
                rr, rb, cph = st['rr'], st['rb'], st['cph']
                pr, hh = h // 2, h % 2
                half = 0 if ki == 3 else 1
                hs_ = slice(half * 512, half * 512 + 512)
                # reciprocal straight from the PSUM ones-row
                # (f32r is fp32-width; the gate only knows dtype != f32)
                with nc.allow_low_precision(reason="1/l in bf16 is accurate enough"):
                    nc.vector.reciprocal(rr[0:1, hs_], cph[half][64:65, 0:512])
                slot = mm.tile([128, 512], dt.float32, tag="aux",
                               name=f"rps{h}{half}_{b}")
                nc.tensor.matmul(slot[0:64, :], ones_b[:], rr[:, hs_],
                                 start=True, stop=True)
                nc.vector.tensor_copy(rb[:, hs_], slot[0:64, :])
                nc.vector.tensor_mul(
                    ctxt[(b, pr)][hh * 64:(hh + 1) * 64, hs_],
                    cph[half][0:64, 0:512], rb[:, hs_])
                if ki == 7 and hh == 1:
                    # head pair done: split ctx into fp8 hi+lo planes for the
                    # DoubleRow dense. Both SBUF-only ops go to the idle Pool
                    # engine; ACT (exp) and DVE (recip/epi) are saturated.
                    nc.gpsimd.tensor_copy(cxh[b][:, pr, :], ctxt[(b, pr)][:])
                    nc.gpsimd.tensor_sub(cxl[b][:, pr, :], ctxt[(b, pr)][:],
                                         cxh[b][:, pr, :])

            if b == 0:
                wdx = wd_pool.tile([128, 4, 8192], dt.float8e4, tag="wdx",
                                   name="wdx")
                wdx_t.append(wdx)
            prev = None
            for slot in range(9):
                if b == 0 and 1 <= slot <= 4:
                    # dense weights stream during phase B; emitting them
                    # inside the slot loop keeps them queued behind phase-B
                    # work so they don't steal DMA bandwidth from phase A
                    kq = slot - 1
                    nc.scalar.dma_start(wdx_t[0][:, kq, :],
                                        wdx_d[:, kq * 8192:(kq + 1) * 8192])
                cur = make_head(slot) if slot < 8 else None
                for ki in range(8):
                    if cur is not None:
                        score_ki(cur, slot, ki)
                    if prev is not None:
                        pv_ki(prev, slot - 1, ki)
                        if ki == 3 or ki == 7:
                            epi(prev, slot - 1, ki)
                    yield
                prev = cur

        def gen_dense(b):
            wdx = wdx_t[0]
            # b=1 runs after attention is done, so the score-rotation banks
            # are free for deeper accumulate/drain pipelining
            slots = ("qkv0", "qkv1", "qkv2") if b == 0 else (
                "qkv0", "qkv1", "qkv2", "qkv3", "aux", "cpsA")
            for mt in range(32):
                dsb = dout_pool.tile([128, 1024], dt.bfloat16, tag="dsb",
                                     name=f"dsb{mt}_{b}")
                for n2 in range(2):
                    dps = mm.tile([128, 512], dt.float32,
                                  tag=slots[(mt * 2 + n2) % len(slots)],
                                  name=f"d{mt}{n2}_{b}")
                    n2s = slice(n2 * 512, (n2 + 1) * 512)
                    for t in range(2):
                        ks = slice(2 * t, 2 * t + 2)
                        w_hi = wdx[:, ks, mt * 128:(mt + 1) * 128]
                        w_lo = wdx[:, ks, 4096 + mt * 128:4096 + (mt + 1) * 128]
                        nc.tensor.matmul(dps[:], w_hi, cxh[b][:, ks, n2s],
                                         start=(t == 0), stop=False, perf_mode=DR)
                        nc.tensor.matmul(dps[:], w_lo, cxh[b][:, ks, n2s],
                                         start=False, stop=False, perf_mode=DR)
                        nc.tensor.matmul(dps[:], w_hi, cxl[b][:, ks, n2s],
                                         start=False, stop=(t == 1), perf_mode=DR)
                        if b == 0:
                            # fine-grained quanta so dense matmuls pack into
                            # the gaps of attn(1)'s serial score->exp chain
                            yield
                    # phase C: ACT is exp-bound mid-phase, so drains ride DVE
                    # there; but during the attn(1) ramp (early mt, exps just
                    # starting) and tail (exps done) ACT has the slack
                    if (b == 1 or mt < 8 or mt >= 24) and n2 == 0:
                        nc.scalar.mul(dsb[:, 0:512], dps[:], DRAIN_SCALE)
                    else:
                        nc.vector.tensor_scalar_mul(
                            dsb[:, n2 * 512:(n2 + 1) * 512], dps[:], DRAIN_SCALE)
                    if b == 1 and mt == 31:
                        # last tile: ship halves separately to shorten the
                        # final copy->DMA drain chain
                        nc.sync.dma_start(
                            outp_d[mt][:, b * 1024 + n2 * 512:
                                        b * 1024 + n2 * 512 + 512],
                            dsb[:, n2 * 512:(n2 + 1) * 512])
                    yield
                if not (b == 1 and mt == 31):
                    # SP queue is idle through C/D; keep Pool free for drains
                    nc.sync.dma_start(
                        outp_d[mt][:, b * 1024: b * 1024 + 1024], dsb[:])

        def _chain(*gens):
            for g in gens:
                yield from g

        _drive(gen_qkv(0))
        # merge phases B and C: as soon as qkv(1) finishes emitting, attn(1)
        # interleaves with attn(0)'s tail; when attn(0) ends, dense(0)
        # interleaves with attn(1)'s tail.
        _drive(_chain(gen_qkv(1), gen_attn(1)),
               _chain(gen_attn(0), gen_dense(0)))
        _drive(gen_dense(1))

    _CACHED_NC = nc
    return nc


def _split8(x):
    """x (f32) -> (hi, lo) fp8e4 with x ~= hi + lo."""
    hi = x.astype(f8)
    lo = (x - hi.astype(np.float32)).astype(f8)
    return hi, lo


def host_prep(hidden_states, alibi, attention_mask, W_qkv, W_dense):
    hsT = np.ascontiguousarray(hidden_states.reshape(NPOS, HID).T)  # [4096, 2048]
    hh, hl = _split8(hsT.astype(np.float32))
    # hsx[c][p][k*512+j] (hi) / 16384 + same (lo) = hsT[k*128+p, c*512+j]
    def _arr(x8):
        return np.ascontiguousarray(
            x8.reshape(32, 128, 4, 512).transpose(2, 1, 0, 3).reshape(4, 128, 16384))
    hsx = np.concatenate([_arr(hh), _arr(hl)], axis=2)  # [4, 128, 32768]

    j32 = np.arange(32)
    inv_freq = 1.0 / (10000.0 ** (2 * j32 / HD))
    t = np.arange(S, dtype=np.float64)
    fr = np.outer(inv_freq, t)                       # [32, S]
    cst = np.zeros((128, 2048), np.float32)
    cst[:, 0:1024] = np.tile(np.cos(fr), (4, 1))
    cst[:, 1024:2048] = np.tile(np.sin(fr), (4, 1))
    cst = cst.astype(bf16)

    # single causal diag block, [kpos, q] layout: 0 where kpos > q, else 1
    mf = np.where(attention_mask[0, 0, 0:128, 0:128], 0.0, 1.0).astype(np.float32)
    msk = np.ascontiguousarray(mf.T).astype(bf16)    # [kpos, q]

    al = alibi.reshape(B, NKV * G, S) * INV          # [B, 64, S]

    perm = []
    for i in range(4):
        perm += [i * 64 + d for d in range(32)]
    for i in range(4):
        perm += [i * 64 + 32 + d for d in range(32)]
    for i in range(4, 8):
        perm += [i * 64 + d for d in range(32)]
    for i in range(4, 8):
        perm += [i * 64 + 32 + d for d in range(32)]
    perm += [512 + d for d in range(64)] + [576 + d for d in range(64)]
    perm = np.array(perm)

    idn = np.eye(64, dtype=np.float32).astype(bf16)
    in_maps = []
    for c in range(NCORES):
        Wg = (W_qkv[c * 640:(c + 1) * 640][perm] * WS).astype(np.float32)
        WgT = np.ascontiguousarray(Wg.T)              # [4096, 640]
        wh, wl = _split8(WgT)
        wqh = np.ascontiguousarray(
            wh.reshape(32, 128, 640).transpose(1, 0, 2)).reshape(128, 32 * 640)
        wql = np.ascontiguousarray(
            wl.reshape(32, 128, 640).transpose(1, 0, 2)).reshape(128, 32 * 640)

        Wd = (W_dense[:, c * 512:(c + 1) * 512] * WS).astype(np.float32)
        WdT = np.ascontiguousarray(Wd.T)              # [512, 4096]
        dh, dl = _split8(WdT)
        wdx = np.concatenate(
            [dh.reshape(4, 128, 4096).transpose(1, 0, 2),
             dl.reshape(4, 128, 4096).transpose(1, 0, 2)], axis=2)  # [128,4,8192]
        wdx = np.ascontiguousarray(wdx).reshape(128, 4 * 8192)

        ab = np.zeros((128, 128), np.float32)
        for b in range(2):
            for ki in range(8):
                for h in range(8):
                    ab[:, b * 64 + ki * 8 + h] = al[b, c * 8 + h,
                                                    ki * 128:(ki + 1) * 128]
        in_maps.append({
            "hsx": hsx, "wqh": wqh, "wql": wql, "wdx": wdx, "cst": cst,
            "msk": msk, "ab": ab, "idn": idn,
        })
    return in_maps


def kernel(hidden_states, alibi, attention_mask, W_qkv, W_dense, _want_time=False):
    nc = build_program()
    in_maps = host_prep(np.asarray(hidden_states), np.asarray(alibi),
                        np.asarray(attention_mask), np.asarray(W_qkv),
                        np.asarray(W_dense))
    res = run_bass_kernel_spmd(nc, in_maps, list(range(NCORES)))
    acc = np.zeros((32, 128, NPOS), np.float32)
    for c in range(NCORES):
        acc += res.results[c]["outp"].astype(np.float32)
    out = acc.reshape(4096, NPOS).T.reshape(B, S, HID)
    if _want_time:
        return np.ascontiguousarray(out), res
    return np.ascontiguousarray(out)


# revision 103
# speedup vs baseline: 1.1143x; 1.0003x over previous
"""GQA attention block (dense_transformer) on 8 trn2 cores.

Sharding: tensor-parallel by kv-group. Core c owns kv-group c = 8 query
heads + 1 k + 1 v head (640 rows of W_qkv) and the matching 512 columns of
W_dense. hidden_states is replicated. Each core returns a bf16 partial
[4096, 2048] dense output; the host sums the 8 partials in f32.

v3: split-precision fp8 DoubleRow for the two big GEMMs. QKV and dense
weights/activations are decomposed host-side (and ctx on-chip) into
hi+lo fp8e4 planes; each K=256 pair runs as 3 DoubleRow matmuls
(hi*hi, lo-w*hi-x, hi-w*lo-x) at 0.5 cycles/col = 0.75x the bf16 PE
cost with bf16-level accuracy (dropped lo*lo term ~2^-8). Attention
internals (RoPE, scores, softmax, PV, epilogue) stay bf16. Scales:
W_qkv and W_dense x64 host-side (q,k,v 64x), exp activation scale
INV/4096, va ones column 4.0 (ctx 16x true), dense drains x2^-10.

Schedule: QKV holds only PSUM banks qkv0..2 (two m-sweeps for b=1, one
5-bank sweep in phase A) so attention permanently owns qkv3/qkv4/aux
score rotation + cpsA/cpsB PV accumulators. Attention heads are
software-pipelined (scores of head h emit interleaved with PV/epi of
head h-1) to hide the exp round-trip, and phases are fused by chaining
generators [qkv(1)->attn(1)] || [attn(0)->dense(0)] so a compute-dense
partner always fills the serial softmax chain's PE bubbles in the
in-order queues. Two heads share each 128-partition qp tile (kk is
duplicated to rows 64-127 so odd heads use base_partition 64). Engine
balance: exps+b1 drains on ACT, recip/epi/drains on DVE, masks and
ctx hi/lo splits on the otherwise idle Pool, output DMAs on SP.
Chunk 0's loads are emitted on a single SWDGE queue in exact demand
order (M1: wqh+hs-hi, M2: wql, M3: hs-lo, then consts) since the cold
start is supply-limited on the shared DMA engines (~28us of transfers
vs 25.6us of compute); even the small consts matter there, and the
first hs sub-loads ride SP/HWDGE so their descriptor generation runs
parallel to the Pool queue's weight stream. Measured (TimelineSim):
294.1us vs 327.6us for the session baseline; rel err 0.0035 (gate
2e-2). PE busy 265.7us (90% occupancy); the residual idle is the
supply-limited phase-A cold start and the attention ramp at the B/C
seam.
"""
import numpy as np
import ml_dtypes
from contextlib import ExitStack

import bass_rust
import concourse.bass as bass
import concourse.mybir as mybir
from concourse import tile
from concourse.bass_utils import run_bass_kernel_spmd

dt = mybir.dt
bf16 = ml_dtypes.bfloat16
f8 = ml_dtypes.float8_e4m3

B, S, HID = 2, 1024, 4096
NKV, G, HD = 8, 8, 64
NPOS = B * S
INV = 0.125
WS = 64.0                      # host-side weight scale (q,k,v come out 64x)
EXP_SCALE = INV / (WS * WS)    # PSUM scores are 4096x true
ONES_VAL = 4.0                 # va ones column -> ctx = 16x true
DRAIN_SCALE = 1.0 / 1024.0     # dense psum = 16 * 64 = 1024x true
NCORES = 8
DR = mybir.MatmulPerfMode.DoubleRow

# ---------------------------------------------------------------------------
# walrus in this container takes at most ONE sync-wait per instruction; Tile
# attaches several (tail drain especially). Split extras onto same-engine nops.
_orig_exit = tile.TileContext.__exit__


def _split_waits(nc):
    for bb in nc.m.functions[0].blocks:
        out, extra = [], 0
        for inst in bb.instructions:
            si = inst.sync_info
            if si is not None and len(si.on_wait) > 1:
                waits = list(si.on_wait)
                for w in waits[:-1]:
                    nop = mybir.InstNoOp(name=f"I-wsplit-{nc.next_id()}")
                    nop.engine = inst.engine
                    nop.sync_info = bass_rust.SyncInfo(on_wait=[w], on_update=[])
                    nc.register_instruction(nop, overwrite=True)
                    out.append(nop)
                    extra += 1
                inst.sync_info = bass_rust.SyncInfo(
                    on_wait=[waits[-1]], on_update=list(si.on_update)
                )
            out.append(inst)
        if extra:
            bb.instructions = out


def _patched_exit(self, exc_type, exc_val, exc_tb):
    r = _orig_exit(self, exc_type, exc_val, exc_tb)
    _split_waits(self.nc)
    return r


tile.TileContext.__exit__ = _patched_exit
# ---------------------------------------------------------------------------

_CACHED_NC = None


def _drive(*gens):
    live = list(gens)
    while live:
        for g in list(live):
            try:
                next(g)
            except StopIteration:
                live.remove(g)


def build_program():
    global _CACHED_NC
    if _CACHED_NC is not None:
        return _CACHED_NC
    nc = bass.Bass()
    # per chunk c (=b*2+n, 512 positions): [hi 32x512 | lo 32x512] fp8 planes
    hsx_d = nc.declare_dram_parameter("hsx", [4, 128, 32768], dt.float8e4, isOutput=False)
    # [p][k][640] hi and lo planes as separate params (hi loads first)
    wqh_d = nc.declare_dram_parameter("wqh", [128, 32 * 640], dt.float8e4, isOutput=False)
    wql_d = nc.declare_dram_parameter("wql", [128, 32 * 640], dt.float8e4, isOutput=False)
    # [p][kt][hi 4096 | lo 4096]
    wdx_d = nc.declare_dram_parameter("wdx", [128, 4 * 8192], dt.float8e4, isOutput=False)
    cst_d = nc.declare_dram_parameter("cst", [128, 2048], dt.bfloat16, isOutput=False)
    msk_d = nc.declare_dram_parameter("msk", [128, 128], dt.bfloat16, isOutput=False)
    ab_d = nc.declare_dram_parameter("ab", [128, 128], dt.float32, isOutput=False)
    idn_d = nc.declare_dram_parameter("idn", [64, 64], dt.bfloat16, isOutput=False)
    outp_d = nc.declare_dram_parameter("outp", [32, 128, NPOS], dt.bfloat16, isOutput=True)

    AF = mybir.ActivationFunctionType
    # packed causal offsets for et: block ki has width 1024-128*ki
    koff = [0] * 8
    for ki in range(1, 8):
        koff[ki] = koff[ki - 1] + (1024 - 128 * (ki - 1))
    ET_W = koff[7] + (1024 - 128 * 7)  # 4608

    with ExitStack() as ctx:
        tc = ctx.enter_context(tile.TileContext(nc))
        cpool = ctx.enter_context(tc.tile_pool(name="const", bufs=1))
        cst_sb = cpool.tile([128, 2048], dt.bfloat16)
        msk_sb = cpool.tile([128, 128], dt.bfloat16)
        ab_sb = cpool.tile([128, 128], dt.float32)
        idn_sb = cpool.tile([64, 64], dt.bfloat16)
        ones_b = cpool.tile([1, 64], dt.bfloat16)
        nc.vector.memset(ones_b[:], 1.0)

        hs_pool = ctx.enter_context(tc.tile_pool(name="hs", bufs=4))
        raw_pool = ctx.enter_context(tc.tile_pool(name="raw", bufs=2))
        tmp_pool = ctx.enter_context(tc.tile_pool(name="tmp", bufs=1))
        qp_pool = ctx.enter_context(tc.tile_pool(name="qp", bufs=2))
        kv_pool = ctx.enter_context(tc.tile_pool(name="kv", bufs=2))
        va_pool = ctx.enter_context(tc.tile_pool(name="va", bufs=2))
        et_pool = ctx.enter_context(tc.tile_pool(name="et", bufs=2))
        l_pool = ctx.enter_context(tc.tile_pool(name="l", bufs=1))
        rb_pool = ctx.enter_context(tc.tile_pool(name="rb", bufs=1))
        ctx_pool = ctx.enter_context(tc.tile_pool(name="ctx", bufs=2))
        cxl_pool = ctx.enter_context(tc.tile_pool(name="cxl", bufs=2))
        dout_pool = ctx.enter_context(tc.tile_pool(name="dout", bufs=3))
        wd_pool = ctx.enter_context(tc.tile_pool(name="wdp", bufs=1))
        wq_pool = ctx.enter_context(tc.tile_pool(name="wqp", bufs=1))
        mm = ctx.enter_context(tc.tile_pool(name="mm", bufs=1, space="PSUM"))

        # [128, k=32, hi|lo 1280] fp8
        # separate tiles per lazily-loaded slice: a shared tile would add
        # false write-after-read deps from in-flight matmuls to later loads
        wqh = [wq_pool.tile([128, 8, 640], dt.float8e4, tag=f"wqh{q}",
                            name=f"wqh{q}") for q in range(4)]
        wql = [wq_pool.tile([128, 16, 640], dt.float8e4, tag=f"wql{q}",
                            name=f"wql{q}") for q in range(2)]
        # small first slices unblock the first matmuls asap
        nc.sync.dma_start(wqh[0][:, 0:4, :], wqh_d[:, 0:4 * 640])
        nc.sync.dma_start(wqh[0][:, 4:8, :], wqh_d[:, 4 * 640:8 * 640])

        # per-batch SBUF state, filled by gen_qkv, read by gen_attn/gen_dense
        qp = {}   # (b, h) -> [64, 1024] bf16
        kk = {}   # b -> [64, 1024] bf16
        va = {}   # b -> [128, 8*72] bf16
        ctxt = {}  # (b, pr) -> [128, 1024] bf16
        cxh = {}  # b -> [128, 4, 1024] fp8 hi
        cxl = {}  # b -> [128, 4, 1024] fp8 lo
        wdx_t = []

        def gen_qkv(b):
            # two heads share one 128-partition tile (h even: rows 0-63,
            # h odd: rows 64-127); kk is duplicated into both halves so the
            # odd-head score matmuls use matching base_partition 64
            for pr in range(4):
                qph = qp_pool.tile([128, 1024], dt.bfloat16, tag=f"qph{pr}",
                                   name=f"qph{pr}_{b}")
                qp[(b, 2 * pr)] = qph[0:64, :]
                qp[(b, 2 * pr + 1)] = qph[64:128, :]
            kk[b] = kv_pool.tile([128, 1024], dt.bfloat16, tag="kk", name=f"kk{b}")
            vt = kv_pool.tile([64, 1024], dt.bfloat16, tag="vt", name=f"vt{b}")
            for n in range(2):
                c = b * 2 + n
                ncol = slice(n * 512, n * 512 + 512)
                # 4 hs tiles per chunk: hiH0 (k0-15), hiH1 (k16-31), loH0, loH1
                hst = {}

                def _load(part, engs):
                    # part: 0=hiH0 1=hiH1 2=loH0 3=loH1
                    t = hs_pool.tile([128, 16, 512], dt.float8e4, tag="hs",
                                     name=f"hs{part}_{c}")
                    if c == 0 and part == 0:
                        # split first load so the first matmul unblocks early;
                        # ride SP/HWDGE so desc-gen runs parallel to the Pool
                        # queue's weight stream
                        for g in range(4):
                            nc.sync.dma_start(
                                t[:, 4 * g:4 * g + 4, :],
                                hsx_d[c][:, g * 2048:(g + 1) * 2048])
                    else:
                        engs.dma_start(t[:], hsx_d[c][:, part * 8192:(part + 1) * 8192])
                    hst[part] = t

                _load(0, nc.gpsimd)
                if c == 0:
                    # chunk 0 is DMA-bound: emit everything on ONE queue in
                    # exact demand order (M1: wqh+hs-hi, M2: wql, M3: hs-lo)
                    # so the shared DMA engines serve it in priority order.
                    nc.gpsimd.dma_start(wqh[1][:], wqh_d[:, 8 * 640:16 * 640])
                    nc.gpsimd.dma_start(wqh[2][:], wqh_d[:, 16 * 640:24 * 640])
                    nc.gpsimd.dma_start(wqh[3][:], wqh_d[:, 24 * 640:32 * 640])
                    _load(1, nc.gpsimd)
                    nc.gpsimd.dma_start(wql[0][:], wql_d[:, 0:16 * 640])
                    nc.gpsimd.dma_start(wql[1][:], wql_d[:, 16 * 640:32 * 640])
                    _load(2, nc.gpsimd)
                    _load(3, nc.gpsimd)
                    # small consts (first needed by RoPE at ~30us) go LAST in
                    # the demand-ordered stream
                    nc.gpsimd.dma_start(cst_sb[:], cst_d[:])
                    nc.gpsimd.dma_start(msk_sb[:], msk_d[:])
                    nc.gpsimd.dma_start(ab_sb[:], ab_d[:])
                    nc.gpsimd.dma_start(idn_sb[:], idn_d[:])
                else:
                    _load(1, nc.sync)
                raw = [raw_pool.tile([128, 512], dt.bfloat16, tag=f"raw{m}",
                                     name=f"raw{m}_{n}_{b}") for m in range(5)]
                # b=0 (phase A, attention not running): one 5-bank sweep.
                # b=1 (phase B): two sweeps (m 0-2, then 3-4) so QKV holds
                # only qkv0..2 and attention keeps qkv3/qkv4/aux+cpsA/cpsB.
                # Either way 3 passes (M1 hi*hi, M2 lo-w*hi-x, M3 hi-w*lo-x):
                # the cold start only needs hi weights + hi activations.
                tag5 = ("qkv0", "qkv1", "qkv2", "cpsA", "cpsB")
                sweeps = ((0, 1, 2, 3, 4),) if b == 0 else ((0, 1, 2), (3, 4))
                for ms in sweeps:
                    ps = {m: mm.tile([128, 512], dt.float32, tag=tag5[j],
                                     name=f"qkv{m}_{n}_{b}")
                          for j, m in enumerate(ms)}
                    for pa in range(3):
                        for kp in range(16):
                            if ms[0] == 0 and pa == 0 and c > 0:
                                if kp == 4:
                                    _load(2, nc.gpsimd)
                                if kp == 10:
                                    _load(3, nc.sync)
                            half, i = kp // 8, kp % 8
                            mv = hst[(2 if pa == 2 else 0) + half][:, 2 * i:2 * i + 2, :]
                            for m in ms:
                                if pa == 1:
                                    wsl = wql[kp // 8][:, 2 * (kp % 8):2 * (kp % 8) + 2,
                                              m * 128:(m + 1) * 128]
                                else:
                                    wsl = wqh[kp // 4][:, 2 * (kp % 4):2 * (kp % 4) + 2,
                                              m * 128:(m + 1) * 128]
                                nc.tensor.matmul(ps[m][:], wsl, mv,
                                                 start=(pa == 0 and kp == 0),
                                                 stop=(pa == 2 and kp == 15),
                                                 perf_mode=DR)
                                if pa == 2 and kp == 15:
                                    # drain while PE finishes the rest
                                    nc.scalar.copy(raw[m][:], ps[m][:])
                            if kp % 2 == 1:
                                yield
                Cs = cst_sb[:, n * 512:(n + 1) * 512]
                Ss = cst_sb[:, 1024 + n * 512: 1024 + (n + 1) * 512]
                for grp in range(2):
                    A, Bb = raw[grp * 2], raw[grp * 2 + 1]
                    P1 = tmp_pool.tile([128, 512], dt.bfloat16, tag="P1")
                    P2 = tmp_pool.tile([128, 512], dt.bfloat16, tag="P2")
                    P3 = tmp_pool.tile([128, 512], dt.bfloat16, tag="P3")
                    P4 = tmp_pool.tile([128, 512], dt.bfloat16, tag="P4")
                    nc.vector.tensor_mul(P1[:], A[:], Cs)
                    nc.vector.tensor_mul(P2[:], Bb[:], Ss)
                    nc.vector.tensor_mul(P3[:], Bb[:], Cs)
                    nc.vector.tensor_mul(P4[:], A[:], Ss)
                    for i in range(4):
                        h = grp * 4 + i
                        sl = slice(32 * i, 32 * i + 32)
                        nc.vector.tensor_sub(qp[(b, h)][0:32, ncol], P1[sl, :], P2[sl, :])
                        nc.vector.tensor_add(qp[(b, h)][32:64, ncol], P3[sl, :], P4[sl, :])
                kvr = raw[4]
                # reuse the P tiles' space for the k-RoPE temporaries
                pk1 = tmp_pool.tile([128, 512], dt.bfloat16, tag="P1",
                                    name=f"pk1_{n}_{b}")[0:32, :]
                pk2 = tmp_pool.tile([128, 512], dt.bfloat16, tag="P2",
                                    name=f"pk2_{n}_{b}")[0:32, :]
                pk3 = tmp_pool.tile([128, 512], dt.bfloat16, tag="P3",
                                    name=f"pk3_{n}_{b}")[0:32, :]
                pk4 = tmp_pool.tile([128, 512], dt.bfloat16, tag="P4",
                                    name=f"pk4_{n}_{b}")[0:32, :]
                nc.vector.tensor_mul(pk1[:], kvr[0:32, :], Cs[0:32, :])
                nc.vector.tensor_mul(pk2[:], kvr[32:64, :], Ss[32:64, :])
                nc.vector.tensor_mul(pk3[:], kvr[32:64, :], Cs[32:64, :])
                nc.vector.tensor_mul(pk4[:], kvr[0:32, :], Ss[0:32, :])
                nc.vector.tensor_sub(kk[b][0:32, ncol], pk1[:], pk2[:])
                nc.vector.tensor_add(kk[b][32:64, ncol], pk3[:], pk4[:])
                nc.vector.tensor_copy(vt[:, ncol], kvr[64:128, :])
                # duplicate k into rows 64-127 for the odd (base-64) heads
                nc.gpsimd.tensor_copy(kk[b][64:128, ncol], kk[b][0:64, ncol])
                yield
            # V transpose + ones column (borrows the aux PSUM bank)
            va[b] = va_pool.tile([128, 8 * 72], dt.bfloat16, tag="va", name=f"va{b}")
            for ki in range(8):
                slot = mm.tile([128, 512], dt.float32, tag="aux", name=f"vps{ki}_{b}")
                vps = slot[:, 0:32].bitcast(dt.bfloat16)
                nc.tensor.transpose(vps, vt[0:64, ki * 128:(ki + 1) * 128],
                                    idn_sb[:, :])
                nc.vector.tensor_copy(va[b][:, ki * 72: ki * 72 + 64], vps)
                nc.vector.memset(va[b][:, ki * 72 + 64: ki * 72 + 65], ONES_VAL)
            yield

        def gen_attn(b):
            for pr in range(4):
                ctxt[(b, pr)] = ctx_pool.tile([128, 1024], dt.bfloat16,
                                              tag=f"ctxt{pr}", name=f"ctxt{pr}_{b}")
            cxh[b] = ctx_pool.tile([128, 4, 1024], dt.float8e4, tag="cxh",
                                   name=f"cxh{b}")
            cxl[b] = cxl_pool.tile([128, 4, 1024], dt.float8e4, tag="cxl",
                                   name=f"cxl{b}")
            # Both batches: heads are software-pipelined (scores of head h
            # interleave with PV/epi of head h-1) so the exp round-trip is
            # hidden; the co-running generator (qkv(1) in phase B, dense(0)
            # in phase C) packs the remaining PE gaps. Scores rotate through
            # qkv3/qkv4/aux, PV owns cpsA/cpsB, qkv/dense use qkv0..2.
            rot = ("qkv3", "qkv4", "aux")

            def make_head(h):
                st = {}
                st['et'] = et_pool.tile([128, ET_W], dt.bfloat16, tag="et",
                                        name=f"et{h}_{b}")
                rrb = rb_pool.tile([128, 1024], dt.bfloat16, tag="rb",
                                   name=f"rrb{h}_{b}")
                st['rr'] = rrb[0:1, :]
                st['rb'] = rrb[64:128, :]
                st['cph'] = [
                    mm.tile([128, 512], dt.float32, tag="cpsA", name=f"cpA{h}_{b}"),
                    mm.tile([128, 512], dt.float32, tag="cpsB", name=f"cpB{h}_{b}")]
                st['ci'] = 0
                return st

            def score_ki(st, h, ki):
                et = st['et']
                base = ki * 128
                nchunks = (1024 - base + 511) // 512
                row0 = 64 * (h % 2)
                for cj in range(nchunks):
                    c0 = base + cj * 512
                    cw = min(512, 1024 - c0)
                    sc = mm.tile([128, 512], dt.float32,
                                 tag=rot[st['ci'] % len(rot)],
                                 name=f"sc{h}{ki}{cj}_{b}")
                    st['ci'] += 1
                    nc.tensor.matmul(
                        sc[:, 0:cw],
                        kk[b][row0:row0 + 64, base:base + 128],
                        qp[(b, h)][0:64, c0:c0 + cw],
                        start=True, stop=True,
                    )
                    abc = b * 64 + ki * 8 + h
                    nc.scalar.activation(
                        et[:, koff[ki] + (c0 - base): koff[ki] + (c0 - base) + cw],
                        sc[:, 0:cw], AF.Exp,
                        bias=ab_sb[:, abc:abc + 1], scale=EXP_SCALE)
                    if cj == 0:
                        # causal mask: zero the upper triangle of the diag
                        # block via a 0/1 multiply (off the sc->exp chain).
                        # Pool is otherwise idle and ACT/DVE are saturated.
                        nc.gpsimd.tensor_mul(
                            et[:, koff[ki]: koff[ki] + 128],
                            et[:, koff[ki]: koff[ki] + 128], msk_sb[:])

            def pv_ki(st, h, ki):
                et, cph = st['et'], st['cph']
                g0 = ki * 128
                while g0 < 1024:
                    half = g0 // 512
                    g1 = min(1024, (half + 1) * 512)
                    loc = slice(g0 - half * 512, g1 - half * 512)
                    nc.tensor.matmul(
                        cph[half][0:65, loc],
                        va[b][:, ki * 72: ki * 72 + 65],
                        et[:, koff[ki] + g0 - ki * 128: koff[ki] + g1 - ki * 128],
                        start=(ki == 0), stop=(ki == (3 if half == 0 else 7)),
                        skip_group_check=True,
                    )
                    g0 = g1

            def epi(st, h, ki):
                # epilogue for the finished half: A after ki=3, B after 7
                rr, rb, cph = st['rr'], st['rb'], st['cph']
                pr, hh = h // 2, h % 2
                half = 0 if ki == 3 else 1
                hs_ = slice(half * 512, half * 512 + 512)
                # reciprocal straight from the PSUM ones-row
                # (f32r is fp32-width; the gate only knows dtype != f32)
                with nc.allow_low_precision(reason="1/l in bf16 is accurate enough"):
                    nc.vector.reciprocal(rr[0:1, hs_], cph[half][64:65, 0:512])
                slot = mm.tile([128, 512], dt.float32, tag="aux",
                               name=f"rps{h}{half}_{b}")
                nc.tensor.matmul(slot[0:64, :], ones_b[:], rr[:, hs_],
                                 start=True, stop=True)
                nc.vector.tensor_copy(rb[:, hs_], slot[0:64, :])
                nc.vector.tensor_mul(
                    ctxt[(b, pr)][hh * 64:(hh + 1) * 64, hs_],
                    cph[half][0:64, 0:512], rb[:, hs_])
                if ki == 7 and hh == 1:
                    # head pair done: split ctx into fp8 hi+lo planes for the
                    # DoubleRow dense. Both SBUF-only ops go to the idle Pool
                    # engine; ACT (exp) and DVE (recip/epi) are saturated.
                    nc.gpsimd.tensor_copy(cxh[b][:, pr, :], ctxt[(b, pr)][:])
                    nc.gpsimd.tensor_sub(cxl[b][:, pr, :], ctxt[(b, pr)][:],
                                         cxh[b][:, pr, :])

            if b == 0:
                wdx = wd_pool.tile([128, 4, 8192], dt.float8e4, tag="wdx",
                                   name="wdx")
                wdx_t.append(wdx)
            prev = None
            for slot in range(9):
                if b == 0 and 1 <= slot <= 4:
                    # dense weights stream during phase B; emitting them
                    # inside the slot loop keeps them queued behind phase-B
                    # work so they don't steal DMA bandwidth from phase A
                    kq = slot - 1
                    nc.scalar.dma_start(wdx_t[0][:, kq, :],
                                        wdx_d[:, kq * 8192:(kq + 1) * 8192])
                cur = make_head(slot) if slot < 8 else None
                for ki in range(8):
                    if cur is not None:
                        score_ki(cur, slot, ki)
                    if prev is not None:
                        pv_ki(prev, slot - 1, ki)
                        if ki == 3 or ki == 7:
                            epi(prev, slot - 1, ki)
                    yield
                prev = cur

        def gen_dense(b):
            wdx = wdx_t[0]
            # b=1 runs after attention is done, so the score-rotation banks
            # are free for deeper accumulate/drain pipelining
            slots = ("qkv0", "qkv1", "qkv2") if b == 0 else (
                "qkv0", "qkv1", "qkv2", "qkv3", "aux", "cpsA")
            for mt in range(32):
                dsb = dout_pool.tile([128, 1024], dt.bfloat16, tag="dsb",
                                     name=f"dsb{mt}_{b}")
                for n2 in range(2):
                    dps = mm.tile([128, 512], dt.float32,
                                  tag=slots[(mt * 2 + n2) % len(slots)],
                                  name=f"d{mt}{n2}_{b}")
                    n2s = slice(n2 * 512, (n2 + 1) * 512)
                    for t in range(2):
                        ks = slice(2 * t, 2 * t + 2)
                        w_hi = wdx[:, ks, mt * 128:(mt + 1) * 128]
                        w_lo = wdx[:, ks, 4096 + mt * 128:4096 + (mt + 1) * 128]
                        nc.tensor.matmul(dps[:], w_hi, cxh[b][:, ks, n2s],
                                         start=(t == 0), stop=False, perf_mode=DR)
                        nc.tensor.matmul(dps[:], w_lo, cxh[b][:, ks, n2s],
                                         start=False, stop=False, perf_mode=DR)
                        nc.tensor.matmul(dps[:], w_hi, cxl[b][:, ks, n2s],
                                         start=False, stop=(t == 1), perf_mode=DR)
                        if b == 0:
                            # fine-grained quanta so dense matmuls pack into
                            # the gaps of attn(1)'s serial score->exp chain
                            yield
                    # phase C: ACT is exp-bound mid-phase, so drains ride DVE
                    # there; but during the attn(1) ramp (early mt, exps just
                    # starting) and tail (exps done) ACT has the slack
                    if (b == 1 or mt < 8 or mt >= 20) and n2 == 0:
                        nc.scalar.mul(dsb[:, 0:512], dps[:], DRAIN_SCALE)
                    else:
                        nc.vector.tensor_scalar_mul(
                            dsb[:, n2 * 512:(n2 + 1) * 512], dps[:], DRAIN_SCALE)
                    if b == 1 and mt == 31:
                        # last tile: ship halves separately to shorten the
                        # final copy->DMA drain chain
                        nc.sync.dma_start(
                            outp_d[mt][:, b * 1024 + n2 * 512:
                                        b * 1024 + n2 * 512 + 512],
                            dsb[:, n2 * 512:(n2 + 1) * 512])
                    yield
                if not (b == 1 and mt == 31):
                    # SP queue is idle through C/D; keep Pool free for drains
                    nc.sync.dma_start(
                        outp_d[mt][:, b * 1024: b * 1024 + 1024], dsb[:])

        def _chain(*gens):
            for g in gens:
                yield from g

        _drive(gen_qkv(0))
        # merge phases B and C: as soon as qkv(1) finishes emitting, attn(1)
        # interleaves with attn(0)'s tail; when attn(0) ends, dense(0)
        # interleaves with attn(1)'s tail.
        _drive(_chain(gen_qkv(1), gen_attn(1)),
               _chain(gen_attn(0), gen_dense(0)))
        _drive(gen_dense(1))

    _CACHED_NC = nc
    return nc


def _split8(x):
    """x (f32) -> (hi, lo) fp8e4 with x ~= hi + lo."""
    hi = x.astype(f8)
    lo = (x - hi.astype(np.float32)).astype(f8)
    return hi, lo


def host_prep(hidden_states, alibi, attention_mask, W_qkv, W_dense):
    hsT = np.ascontiguousarray(hidden_states.reshape(NPOS, HID).T)  # [4096, 2048]
    hh, hl = _split8(hsT.astype(np.float32))
    # hsx[c][p][k*512+j] (hi) / 16384 + same (lo) = hsT[k*128+p, c*512+j]
    def _arr(x8):
        return np.ascontiguousarray(
            x8.reshape(32, 128, 4, 512).transpose(2, 1, 0, 3).reshape(4, 128, 16384))
    hsx = np.concatenate([_arr(hh), _arr(hl)], axis=2)  # [4, 128, 32768]

    j32 = np.arange(32)
    inv_freq = 1.0 / (10000.0 ** (2 * j32 / HD))
    t = np.arange(S, dtype=np.float64)
    fr = np.outer(inv_freq, t)                       # [32, S]
    cst = np.zeros((128, 2048), np.float32)
    cst[:, 0:1024] = np.tile(np.cos(fr), (4, 1))
    cst[:, 1024:2048] = np.tile(np.sin(fr), (4, 1))
    cst = cst.astype(bf16)

    # single causal diag block, [kpos, q] layout: 0 where kpos > q, else 1
    mf = np.where(attention_mask[0, 0, 0:128, 0:128], 0.0, 1.0).astype(np.float32)
    msk = np.ascontiguousarray(mf.T).astype(bf16)    # [kpos, q]

    al = alibi.reshape(B, NKV * G, S) * INV          # [B, 64, S]

    perm = []
    for i in range(4):
        perm += [i * 64 + d for d in range(32)]
    for i in range(4):
        perm += [i * 64 + 32 + d for d in range(32)]
    for i in range(4, 8):
        perm += [i * 64 + d for d in range(32)]
    for i in range(4, 8):
        perm += [i * 64 + 32 + d for d in range(32)]
    perm += [512 + d for d in range(64)] + [576 + d for d in range(64)]
    perm = np.array(perm)

    idn = np.eye(64, dtype=np.float32).astype(bf16)
    in_maps = []
    for c in range(NCORES):
        Wg = (W_qkv[c * 640:(c + 1) * 640][perm] * WS).astype(np.float32)
        WgT = np.ascontiguousarray(Wg.T)              # [4096, 640]
        wh, wl = _split8(WgT)
        wqh = np.ascontiguousarray(
            wh.reshape(32, 128, 640).transpose(1, 0, 2)).reshape(128, 32 * 640)
        wql = np.ascontiguousarray(
            wl.reshape(32, 128, 640).transpose(1, 0, 2)).reshape(128, 32 * 640)

        Wd = (W_dense[:, c * 512:(c + 1) * 512] * WS).astype(np.float32)
        WdT = np.ascontiguousarray(Wd.T)              # [512, 4096]
        dh, dl = _split8(WdT)
        wdx = np.concatenate(
            [dh.reshape(4, 128, 4096).transpose(1, 0, 2),
             dl.reshape(4, 128, 4096).transpose(1, 0, 2)], axis=2)  # [128,4,8192]
        wdx = np.ascontiguousarray(wdx).reshape(128, 4 * 8192)

        ab = np.zeros((128, 128), np.float32)
        for b in range(2):
            for ki in range(8):
                for h in range(8):
                    ab[:, b * 64 + ki * 8 + h] = al[b, c * 8 + h,
                                                    ki * 128:(ki + 1) * 128]
        in_maps.append({
            "hsx": hsx, "wqh": wqh, "wql": wql, "wdx": wdx, "cst": cst,
            "msk": msk, "ab": ab, "idn": idn,
        })
    return in_maps


def kernel(hidden_states, alibi, attention_mask, W_qkv, W_dense, _want_time=False):
    nc = build_program()
    in_maps = host_prep(np.asarray(hidden_states), np.asarray(alibi),
                        np.asarray(attention_mask), np.asarray(W_qkv),
                        np.asarray(W_dense))
    res = run_bass_kernel_spmd(nc, in_maps, list(range(NCORES)))
    acc = np.zeros((32, 128, NPOS), np.float32)
    for c in range(NCORES):
        acc += res.results[c]["outp"].astype(np.float32)
    out = acc.reshape(4096, NPOS).T.reshape(B, S, HID)
    if _want_time:
        return np.ascontiguousarray(out), res
    return np.ascontiguousarray(out)
